# revision 2
# baseline (speedup 1.0000x reference)
"""Distributed multi-head attention kernel for one TRN2 chip (8 NeuronCores).

Problem: y = Attention(x) with b=2, n=2048, dim=1024, heads=16, dim_head=64.

Sharding (data + tensor parallel):
  core c: batch g = c // 4, head-group r = c % 4 (4 heads = 256 inner dims).

Design (v2):
  - Host pre-converts x and Wq/Wk/Wv to bf16 (SCALE folded into Wq).
  - x^T lands in SBUF via dma_start_transpose (xT[p, c, t] = x[t, 128c+p];
    no PE transposes, no DVE copies).
  - Attention in single-head strips (h, ib): scores psS [128 j, 2*512 i]
    (two j-chunks per PSUM tile), exp on ACT -> e bf16 tiles. The ACT
    engine is the attention-phase pacer; projections and the output matmul
    are interleaved into the PE stream one step per exp slot.
  - AV transposed: out [i=128 tok, 65] per i-chunk with a ones-column in
    v_ext giving the softmax denominator in col 64 (full 128-partition
    outputs halve PE cost vs the [65, i] layout, and normalization becomes
    a per-partition DVE scalar op).
  - Transpose-back to [d, tok] bf16 runs twice per block with gate-scaled
    identity matrices (the wrong-batch copy emits zeros), DMA'd straight
    from PSUM into the A2A staging buffer.
  - Exchange: four 8-core AllToAll collectives (one per head, fired as
    each head completes) so only the smallest possible transfer sits in
    the tail. Receivers sum the two batch halves (one is zero) with a
    plain + SWDGE-accumulate DMA pair.
  - Output projection in two stages: even aT chunks (heads 0/1) overlap
    the later collectives; odd chunks run in the tail.
"""

import os
import sys

if "/opt/trn_rl_repo" not in sys.path:
    sys.path.insert(0, "/opt/trn_rl_repo")

ABL_AV = os.environ.get("ABL_AV") == "1"    # skip AV+norm+a2a (timing probe)
ABL_POP = os.environ.get("ABL_POP") == "1"  # skip pop steps (timing probe)

from contextlib import ExitStack

import ml_dtypes
import numpy as np

import concourse.bass as bass
from concourse import bacc
import concourse.mybir as mybir
import concourse.tile as tile
from concourse.masks import make_identity

F32 = mybir.dt.float32
F32R = mybir.dt.float32r
BF16 = mybir.dt.bfloat16
EXP = mybir.ActivationFunctionType.Exp

B, N, DIM = 2, 2048, 1024
HEADS, DH = 16, 64
INNER = HEADS * DH            # 1024
SCALE = DH ** -0.5
GROUP = 4                     # tensor-parallel group size (cores per batch)
IC = INNER // GROUP           # 256 inner dims per core (4 heads)
NQ = N // GROUP               # 512 output tokens per core
NEG = -1.0e30

P = 128
TB = 512                      # i-block (moving-dim max)
NT = N // P                   # 16 j-chunks
NJP = NT // 2                 # 8 j-chunk pairs (one psS/exp per pair)
ND = DIM // P                 # 8 contraction chunks
NTB = N // TB                 # 4 token blocks
LAGP = 2                      # AV trails exp by LAGP j-chunk-pairs

_CACHE = {}


def _mm(nc, out, lhsT, rhs, start=True, stop=True, tile_position=None):
    nc.tensor.matmul(
        out, lhsT, rhs, start=start, stop=stop, tile_position=tile_position
    )


def _build(mask_any: bool) -> bass.Bass:
    nc = bacc.Bacc()

    xt_in = nc.declare_dram_parameter("xt_b", [NTB, P, ND, TB], BF16, False)
    wq = nc.declare_dram_parameter("wq_s", [DIM, IC], BF16, False)
    wk = nc.declare_dram_parameter("wk_s", [DIM, IC], BF16, False)
    wv = nc.declare_dram_parameter("wv_s", [DIM, IC], BF16, False)
    wo = nc.declare_dram_parameter("wo", [INNER, DIM], BF16, False)
    bo = nc.declare_dram_parameter("bo", [1, DIM], F32R, False)
    # gate[:, g] = 1.0 iff this core handles batch g (replicated down the
    # partition axis); scales the transpose-back identity so cross-batch
    # A2A chunks carry zeros.
    gate = nc.declare_dram_parameter("gate", [P, 2], F32, False)
    if mask_any:
        mb = nc.declare_dram_parameter("mbias", [P, NT], F32, False)
    y = nc.declare_dram_parameter("y", [NQ, DIM], F32, True)

    with ExitStack() as ctx:
        tc = ctx.enter_context(tile.TileContext(nc))

        const = ctx.enter_context(tc.tile_pool(name="const", bufs=1))
        ident = const.tile([P, P], BF16, tag="ident")
        make_identity(nc, ident[:])
        gate_sb = const.tile([P, 2], F32, tag="gate_sb")
        nc.scalar.dma_start(gate_sb[:], gate.ap())
        ones_f = const.tile([P, P], F32, tag="ones_f")
        nc.vector.memset(ones_f[:], 1.0)
        ones_r = const.tile([P, P], F32R, tag="ones_r")
        nc.vector.tensor_copy(ones_r[:], ones_f[:])
        bo_sb = const.tile([1, DIM], F32R, tag="bo_sb")
        nc.scalar.dma_start(bo_sb[:], bo.ap())
        if mask_any:
            mb_sb = const.tile([P, NT], F32, tag="mb_sb")
            nc.scalar.dma_start(mb_sb[:], mb.ap())

        # ---- persistent SBUF ----
        big = ctx.enter_context(tc.tile_pool(name="big", bufs=1))
        wk_all = big.tile([P, ND, IC], BF16, tag="w", bufs=3, name="wk_all")
        wq_all = big.tile([P, ND, IC], BF16, tag="w", bufs=3, name="wq_all")
        wv_all = big.tile([P, ND, IC], BF16, tag="w", bufs=3, name="wv_all")
        wo_all = big.tile([P, ND, DIM], BF16, tag="woa", name="wo_all")
        wk_sb = [wk_all[:, c, :] for c in range(ND)]
        wq_sb = [wq_all[:, c, :] for c in range(ND)]
        wv_sb = [wv_all[:, c, :] for c in range(ND)]
        wo_sb = [wo_all[:, c, :] for c in range(ND)]
        xTb = [big.tile([P, ND, TB], BF16, tag="xT", bufs=4, name=f"xT{tb}")
               for tb in range(NTB)]

        def xT(c, lo, hi):
            # feature-chunk c, token range [lo, hi) (within one tb block)
            tb, off = divmod(lo, TB)
            return xTb[tb][:, c, off : off + (hi - lo)]
        q2 = [big.tile([P, N], BF16, tag="qk", bufs=4, name=f"q2_{hp}")
              for hp in range(2)]
        k2 = [big.tile([P, N], BF16, tag="qk", bufs=4, name=f"k2_{hp}")
              for hp in range(2)]
        v_all = big.tile([P, 4 * NT * (DH + 1)], BF16, tag="vx",
                         name="v_all")
        va = v_all[:].rearrange("p (h t c) -> p h t c", h=4, t=NT)
        aT = [big.tile([P, NQ], BF16, tag="aT", bufs=8, name=f"aT_{c}")
              for c in range(ND)]
        partial = [big.tile([P, TB], F32, tag="part", bufs=8, name=f"part{i}")
                   for i in range(8)]
        bo_rep = const.tile([P, DIM], F32, tag="bo_rep")

        # DMA issue order: one DMA per weight tensor on the scalar (ACT)
        # HWDGE queue - 4 configs total so neither the ACT sequencer nor the
        # queue's in-flight window ever backs up; x transposes on SP.
        nc.sync.dma_start(xTb[0][:], xt_in.ap()[0])
        nc.scalar.dma_start(wk_all[:], wk.ap().rearrange("(c p) f -> p c f",
                                                         c=ND))
        nc.sync.dma_start(xTb[1][:], xt_in.ap()[1])
        nc.scalar.dma_start(wq_all[:], wq.ap().rearrange("(c p) f -> p c f",
                                                         c=ND))
        nc.sync.dma_start(xTb[2][:], xt_in.ap()[2])
        nc.sync.dma_start(xTb[3][:], xt_in.ap()[3])
        nc.scalar.dma_start(wv_all[:], wv.ap().rearrange("(c p) f -> p c f",
                                                         c=ND))
        nc.scalar.dma_start(wo_all[:], wo.ap().rearrange("(c p) f -> p c f",
                                                         c=ND))

        nc.vector.memset(va[:, :, :, DH], 1.0)

        # A2A buffers (DRAM), one per head: chunk j (64 rows) -> core j;
        # chunk content = this head's [64 d, 512 tok] for quarter j%4,
        # zeros unless j//4 == my batch.
        dram = ctx.enter_context(tc.tile_pool(name="dram", bufs=1, space="DRAM"))
        a2a_in = [dram.tile([8 * DH, NQ], BF16, tag="a2a_in", bufs=4,
                            name=f"a2a_in{h}") for h in range(4)]
        a2a_out = [dram.tile([8 * DH, NQ], BF16, tag="a2a_out", bufs=4,
                             name=f"a2a_out{h}") for h in range(4)]

        ps_s = ctx.enter_context(tc.tile_pool(name="ps_s", bufs=2, space="PSUM"))
        ps_av = ctx.enter_context(tc.tile_pool(name="ps_av", bufs=2, space="PSUM"))
        ps_m = ctx.enter_context(tc.tile_pool(name="ps_m", bufs=2, space="PSUM"))

        sm = ctx.enter_context(tc.tile_pool(name="sm", bufs=1))

        # bias broadcast rows (uses the scores PSUM arena before attention)
        for nb in range(DIM // TB):
            psb = ps_s.tile([P, 2 * TB], F32, tag="s", name="psb")
            _mm(nc, psb[:, 0:TB], ones_r[0:1, :],
                bo_sb[:, nb * TB : (nb + 1) * TB])
            nc.vector.tensor_copy(bo_rep[:, nb * TB : (nb + 1) * TB],
                                  psb[:, 0:TB])

        # ---- projection / wo step closures (popped one per exp slot) ----
        def proj_step(kind, hp, tb):
            def run():
                pj = ps_m.tile([P, TB], F32, tag="m", name="pj")
                wsb = wq_sb if kind == "q" else wk_sb
                dest = q2[hp] if kind == "q" else k2[hp]
                for c in range(ND):
                    _mm(nc, pj[:], wsb[c][:, hp * P : (hp + 1) * P],
                        xT(c, tb * TB, (tb + 1) * TB),
                        start=(c == 0), stop=(c == ND - 1))
                nc.vector.tensor_copy(dest[:, tb * TB : (tb + 1) * TB], pj[:])
            return run

        def v_step(t):
            def run():
                pv = ps_m.tile([P, IC], F32, tag="m", name="pv")
                for c in range(ND):
                    _mm(nc, pv[:], xT(c, t * P, (t + 1) * P), wv_sb[c],
                        start=(c == 0), stop=(c == ND - 1))
                nc.vector.tensor_copy(
                    va[:, :, t, 0:DH],
                    pv[:].rearrange("p (h c) -> p h c", h=4))
            return run

        def wo_step(parity, t, nb, alt):
            def run():
                if alt:
                    pw_t = ps_s.tile([P, 2 * TB], F32, tag="s", name="pw")
                    pw = pw_t[:, 0:TB]
                else:
                    pw = ps_m.tile([P, TB], F32, tag="m", name="pw")[:]
                for m in range(4):
                    _mm(nc, pw,
                        aT[2 * m + parity][:, t * P : (t + 1) * P],
                        wo_sb[2 * m + parity][:, nb * TB : (nb + 1) * TB],
                        start=(m == 0), stop=(m == 3))
                if parity == 0:
                    nc.vector.tensor_add(
                        partial[t * 2 + nb][:], pw,
                        bo_rep[:, nb * TB : (nb + 1) * TB])
                else:
                    fo = sm.tile([P, TB], F32, tag="fo", bufs=3, name="fo")
                    nc.vector.tensor_add(fo[:], pw, partial[t * 2 + nb][:])
                    nc.sync.dma_start(
                        y.ap()[t * P : (t + 1) * P, nb * TB : (nb + 1) * TB],
                        fo[:])
            return run

        # stage 1: only the projections that gate the first scores
        proj_step("k", 0, 0)()
        proj_step("k", 0, 1)()
        proj_step("q", 0, 0)()
        proj_step("q", 0, 1)()
        proj_step("k", 0, 2)()
        proj_step("k", 0, 3)()

        # pop queue: exp-slot index -> closure (16 strips x 8 slots = 128).
        # v chunk t is consumed by the (full-strip-lagged) AV at absolute
        # slot 8 + t//2; popping v(t) at slot t-1 stays ahead (same-engine
        # program order guarantees no deadlock even at zero slack).
        popq = {0: [v_step(0), v_step(1)]}
        for t in range(2, NT):
            popq.setdefault(t - 1, []).append(v_step(t))
        popq.setdefault(15, []).append(proj_step("q", 0, 2))
        popq.setdefault(17, []).append(proj_step("q", 0, 3))
        for i in range(NTB):
            popq.setdefault(18 + 4 * i, []).append(proj_step("k", 1, i))
            popq.setdefault(34 + 4 * i, []).append(proj_step("q", 1, i))

        # ---- attention strips: one head at a time; AV for strip s runs
        # during strip s+1 (one pair per exp slot) so the interleaved
        # v/k/q projection steps never sit behind a waiting AV matmul.
        def do_av(st, jt):
            et = st["es"][jt // 2]
            off = (jt % 2) * TB
            if st["av"] is None:
                st["av"] = ps_av.tile([P, 4 * (DH + 1)], F32, tag="av",
                                      name="av")
                # four accumulation groups share this bank: zero once and
                # accumulate with start=False so their start bits can't
                # clobber each other's partials
                nc.vector.memset(st["av"][:], 0.0)
            av = st["av"]
            for c in range(4):
                nc.tensor.matmul(
                    av[:, c * (DH + 1) : (c + 1) * (DH + 1)],
                    et[:, off + c * P : off + (c + 1) * P],
                    va[:, st["h"], jt, :],
                    start=False, stop=(jt == NT - 1),
                    skip_group_check=True)

        def norm_dve(st):
            """Normalize (DVE); returns deferred PE-transpose + staging."""
            av, h, ib = st["av"], st["h"], st["ib"]
            rcp = sm.tile([P, 4], F32, tag="rcp", bufs=2, name="rcp")
            with nc.allow_low_precision("softmax denom"):
                nc.vector.reciprocal(rcp[:], av[:, DH :: DH + 1])
            onrm = sm.tile([P, 4 * DH], BF16, tag="onrm", bufs=2, name="onrm")
            for c in range(4):
                nc.vector.tensor_scalar_mul(
                    onrm[:, c * DH : (c + 1) * DH],
                    av[:, c * (DH + 1) : c * (DH + 1) + DH],
                    rcp[:, c : c + 1])

            def part2():
                oT = ps_m.tile([DH, TB], BF16, tag="m", name="oT")
                for c in range(4):
                    nc.tensor.transpose(oT[:, c * P : (c + 1) * P],
                                        onrm[:, c * DH : (c + 1) * DH],
                                        ident[:])
                for g in range(2):
                    stg = sm.tile([DH, TB], BF16, tag="stg", bufs=3,
                                  name="stg")
                    nc.vector.tensor_scalar_mul(stg[:], oT[:],
                                                gate_sb[0:DH, g : g + 1])
                    row = (4 * g + ib) * DH
                    nc.sync.dma_start(a2a_in[h][row : row + DH, :], stg[:])
                if ib == NTB - 1:
                    fire_a2a(h)
            return part2

        def fire_a2a(h):
            nc.gpsimd.collective_compute(
                "AllToAll",
                mybir.AluOpType.bypass,
                replica_groups=[[0, 1, 2, 3, 4, 5, 6, 7]],
                ins=[a2a_in[h].opt()],
                outs=[a2a_out[h].opt()],
            )

        def recv_a2a(h):
            # receive: aT[2m+hp] rows [e*64, e*64+64) = chunk m + chunk 4+m
            # (exactly one is nonzero; SWDGE accumulate adds the other).
            # Emitted only after the LAST collective so the SP/Pool queues
            # never block a later collective's issue while waiting.
            hp, e = divmod(h, 2)
            for m in range(GROUP):
                dst = aT[2 * m + hp][e * DH : (e + 1) * DH, :]
                nc.sync.dma_start(dst, a2a_out[h][m * DH : (m + 1) * DH, :])
                nc.gpsimd.dma_start(
                    dst, a2a_out[h][(4 + m) * DH : (5 + m) * DH, :],
                    accum_op=mybir.AluOpType.add)

        slot = 0
        prev = None
        pend2 = None
        for h in range(4):
            hp, e = divmod(h, 2)
            ksb = k2[hp][e * DH : (e + 1) * DH, :]
            qsb = q2[hp][e * DH : (e + 1) * DH, :]
            tp = (e * DH, 0)
            for ib in range(NTB):
                isl = slice(ib * TB, (ib + 1) * TB)
                cur = {"h": h, "ib": ib, "es": [None] * NJP, "av": None}
                for jp in range(NJP):
                    psS = ps_s.tile([P, 2 * TB], F32, tag="s", name="psS")
                    for sub in range(2):
                        jt = 2 * jp + sub
                        jsl = slice(jt * P, (jt + 1) * P)
                        _mm(nc, psS[:, sub * TB : (sub + 1) * TB],
                            ksb[:, jsl], qsb[:, isl], tile_position=tp)
                        if mask_any:
                            nc.vector.tensor_scalar_add(
                                psS[:, sub * TB : (sub + 1) * TB],
                                psS[:, sub * TB : (sub + 1) * TB],
                                mb_sb[:, jt : jt + 1])
                    e_t = sm.tile([P, 2 * TB], BF16, tag="e", bufs=12,
                                  name="e")
                    nc.scalar.activation(e_t[:], psS[:], EXP)
                    cur["es"][jp] = e_t
                    for fn in popq.pop(slot, ()):
                        if not ABL_POP:
                            fn()
                    if jp == 2 and pend2 is not None:
                        pend2()
                        pend2 = None
                    if prev is not None and not ABL_AV:
                        do_av(prev, 2 * jp)
                        do_av(prev, 2 * jp + 1)
                    slot += 1
                if prev is not None and not ABL_AV:
                    pend2 = norm_dve(prev)
                prev = cur

        # tail: AV + norm for the final strip, then its head's A2A
        if pend2 is not None:
            pend2()
        if not ABL_AV:
            for jp in range(NJP):
                do_av(prev, 2 * jp)
                do_av(prev, 2 * jp + 1)
            p2 = norm_dve(prev)
            p2()
            for h in range(4):
                recv_a2a(h)

        # wo even stage: runs inside the final collective's window
        wi = 0
        for t in range(NQ // P):
            for nb in range(DIM // TB):
                wo_step(0, t, nb, alt=(wi % 2 == 1))()
                wi += 1
        # drain any unfired pop steps (none expected)
        for i in sorted(popq):
            for fn in popq.pop(i):
                fn()

        # wo odd stage (alternate PSUM arenas to avoid WAR stalls)
        wi = 0
        for t in range(NQ // P):
            for nb in range(DIM // TB):
                wo_step(1, t, nb, alt=(wi % 2 == 1))()
                wi += 1

    nc.compile()
    return nc


def _get_nc(mask_any: bool) -> bass.Bass:
    if mask_any not in _CACHE:
        _CACHE[mask_any] = _build(mask_any)
    return _CACHE[mask_any]


def _in_maps(x, mask, Wq, Wkv, Wo, bo, mask_any):
    bf = ml_dtypes.bfloat16
    bo2 = np.ascontiguousarray(np.asarray(bo, np.float32).reshape(1, DIM))
    wo_bf = np.ascontiguousarray(np.asarray(Wo, np.float32).astype(bf))
    maps = []
    for c in range(8):
        g, r = divmod(c, GROUP)
        csl = slice(r * IC, (r + 1) * IC)
        m = {
            "xt_b": np.ascontiguousarray(
                x[g].reshape(NTB, TB, ND, P).transpose(0, 3, 2, 1).astype(bf)),
            "wq_s": np.ascontiguousarray(
                (Wq[:, csl] * np.float32(SCALE)).astype(bf)),
            "wk_s": np.ascontiguousarray(Wkv[:, csl].astype(bf)),
            "wv_s": np.ascontiguousarray(
                Wkv[:, INNER + r * IC : INNER + (r + 1) * IC].astype(bf)),
            "wo": wo_bf,
            "bo": bo2,
            "gate": np.ascontiguousarray(
                np.tile(np.array([[1.0 - g, float(g)]], np.float32), (P, 1))),
        }
        if mask_any:
            mvec = np.where(mask[g], np.float32(NEG), np.float32(0.0)).astype(
                np.float32)
            m["mbias"] = np.ascontiguousarray(mvec.reshape(NT, P).T)
        maps.append(m)
    return maps


_RUNNER = {}


def _get_runner(mask_any: bool):
    """Cached jax-jitted SPMD executor for the Bass module."""
    if mask_any in _RUNNER:
        return _RUNNER[mask_any]
    import jax
    from jax.sharding import Mesh, PartitionSpec
    from jax.experimental.shard_map import shard_map
    from concourse import bass2jax

    nc = _get_nc(mask_any)
    bass2jax.install_neuronx_cc_hook()

    partition_name = (
        nc.partition_id_tensor.name if nc.partition_id_tensor else None
    )
    in_names, out_names, out_avals = [], [], []
    for alloc in nc.m.functions[0].allocations:
        if not isinstance(alloc, mybir.MemoryLocationSet):
            continue
        name = alloc.memorylocations[0].name
        if alloc.kind == "ExternalInput":
            if name != partition_name:
                in_names.append(name)
        elif alloc.kind == "ExternalOutput":
            shape = tuple(alloc.tensor_shape)
            dtype = mybir.dt.np(alloc.dtype)
            out_names.append(name)
            out_avals.append(jax.core.ShapedArray(shape, dtype))
    n_params = len(in_names)
    n_outs = len(out_avals)
    all_names = list(in_names) + list(out_names)
    if partition_name is not None:
        all_names.append(partition_name)
    donate = tuple(range(n_params, n_params + n_outs))

    def _body(*args):
        operands = list(args)
        if partition_name is not None:
            operands.append(bass2jax.partition_id_tensor())
        outs = bass2jax._bass_exec_p.bind(
            *operands,
            out_avals=tuple(out_avals),
            in_names=tuple(all_names),
            out_names=tuple(out_names),
            lowering_input_output_aliases=(),
            sim_require_finite=True,
            sim_require_nnan=True,
            nc=nc,
        )
        return tuple(outs)

    devices = jax.devices()[:8]
    mesh = Mesh(np.asarray(devices), ("core",))
    in_specs = (PartitionSpec("core"),) * (n_params + n_outs)
    out_specs = (PartitionSpec("core"),) * n_outs
    sharded = jax.jit(
        shard_map(_body, mesh=mesh, in_specs=in_specs, out_specs=out_specs,
                  check_rep=False),
        donate_argnums=donate,
        keep_unused=True,
    )
    zero_shapes = [tuple(a.shape) for a in out_avals]
    zero_dtypes = [a.dtype for a in out_avals]

    def call(maps):
        concat_in = [
            np.concatenate([np.asarray(maps[c][nm]) for c in range(8)], axis=0)
            for nm in in_names
        ]
        concat_zeros = [
            np.zeros((8 * s[0], *s[1:]), d)
            for s, d in zip(zero_shapes, zero_dtypes)
        ]
        out_arrs = sharded(*concat_in, *concat_zeros)
        return [
            {
                nm: np.asarray(out_arrs[i]).reshape(8, *zero_shapes[i])[c]
                for i, nm in enumerate(out_names)
            }
            for c in range(8)
        ]

    _RUNNER[mask_any] = call
    return call


def run(x, mask, Wq, Wkv, Wo, bo, trace=False):
    x = np.asarray(x, np.float32)
    mask = np.asarray(mask, bool)
    Wq = np.asarray(Wq, np.float32)
    Wkv = np.asarray(Wkv, np.float32)
    Wo = np.asarray(Wo, np.float32)
    bo = np.asarray(bo, np.float32)
    mask_any = bool(mask.any())
    maps = _in_maps(x, mask, Wq, Wkv, Wo, bo, mask_any)
    results = _get_runner(mask_any)(maps)
    out = np.empty((B, N, DIM), np.float32)
    for c in range(8):
        g, r = divmod(c, GROUP)
        out[g, r * NQ : (r + 1) * NQ, :] = results[c]["y"]
    return out, results


def kernel(x, mask, Wq, Wkv, Wo, bo):
    out, _ = run(x, mask, Wq, Wkv, Wo, bo)
    return out


# revision 3
# speedup vs baseline: 1.0442x; 1.0442x over previous
"""Distributed multi-head attention kernel for one TRN2 chip (8 NeuronCores).

Problem: y = Attention(x) with b=2, n=2048, dim=1024, heads=16, dim_head=64.

Sharding (data + tensor parallel):
  core c: batch g = c // 4, head-group r = c % 4 (4 heads = 256 inner dims).

Design (v2):
  - Host pre-converts x (transposed to feature-major xT[p, c, t] =
    x[t, 128c+p]) and Wq/Wk/Wv/Wo to bf16, with SCALE folded into Wq, so
    no on-device transposes or conversions are needed.
  - Attention in single-head strips (h, ib): scores psS [128 j, 2*512 i]
    (two j-chunks per PSUM tile pair), exp on ACT -> e bf16 tiles. The
    ACT engine (exp throughput) paces the attention phase; q/k/v
    projections are interleaved into the PE stream one step per exp slot.
  - AV in transposed layout: out [i=128 tok, 65] per i-chunk with a
    ones-column in the v tile giving the softmax denominator in col 64.
    Full 128-partition outputs halve PE cost vs the [65, i] layout, and
    normalization becomes a cheap per-partition DVE scalar op. The four
    i-chunk accumulation groups share one PSUM bank, so the bank is
    zeroed once and all matmuls accumulate with start=False.
  - AV runs in-strip with a 2-pair lag (strips 0-1 lag a full strip while
    the v projections stream in); normalize+transpose-back+staging for
    strip s is deferred into strip s+1 so the PE never stalls on the DVE
    normalization chain at strip boundaries.
  - Transpose-back to [d, tok] via PE with a bf16 identity; two
    gate-scaled DVE copies stage it into the A2A buffer (the wrong-batch
    copy writes zeros).
  - Exchange: four 8-core AllToAll collectives (one per head, fired
    inline as each head's last strip closes) so only the smallest
    possible transfer sits in the tail; receives (plain + SWDGE
    accumulate, summing the two batch halves) are deferred until after
    the last collective so no queue ever blocks a later collective's
    issue.
  - Output projection in two stages: even aT chunks (head pairs 0/1)
    run inside the later collectives' windows; odd chunks in the tail,
    alternating PSUM arenas to avoid write-after-read stalls.
"""

import sys

if "/opt/trn_rl_repo" not in sys.path:
    sys.path.insert(0, "/opt/trn_rl_repo")

from contextlib import ExitStack

import ml_dtypes
import numpy as np

import concourse.bass as bass
from concourse import bacc
import concourse.mybir as mybir
import concourse.tile as tile
from concourse.masks import make_identity

F32 = mybir.dt.float32
F32R = mybir.dt.float32r
BF16 = mybir.dt.bfloat16
EXP = mybir.ActivationFunctionType.Exp

B, N, DIM = 2, 2048, 1024
HEADS, DH = 16, 64
INNER = HEADS * DH            # 1024
SCALE = DH ** -0.5
GROUP = 4                     # tensor-parallel group size (cores per batch)
IC = INNER // GROUP           # 256 inner dims per core (4 heads)
NQ = N // GROUP               # 512 output tokens per core
NEG = -1.0e30

P = 128
TB = 512                      # i-block (moving-dim max)
NT = N // P                   # 16 j-chunks
NJP = NT // 2                 # 8 j-chunk pairs (one psS/exp per pair)
ND = DIM // P                 # 8 contraction chunks
NTB = N // TB                 # 4 token blocks
LAGP = 2                      # AV trails exp by LAGP j-chunk-pairs

_CACHE = {}


def _mm(nc, out, lhsT, rhs, start=True, stop=True, tile_position=None):
    nc.tensor.matmul(
        out, lhsT, rhs, start=start, stop=stop, tile_position=tile_position
    )


def _build(mask_any: bool) -> bass.Bass:
    nc = bacc.Bacc()

    xt_in = nc.declare_dram_parameter("xt_b", [NTB, P, ND, TB], BF16, False)
    wq = nc.declare_dram_parameter("wq_s", [DIM, IC], BF16, False)
    wk = nc.declare_dram_parameter("wk_s", [DIM, IC], BF16, False)
    wv = nc.declare_dram_parameter("wv_s", [DIM, IC], BF16, False)
    wo = nc.declare_dram_parameter("wo", [INNER, DIM], BF16, False)
    bo = nc.declare_dram_parameter("bo", [1, DIM], F32R, False)
    # gate[:, g] = 1.0 iff this core handles batch g (replicated down the
    # partition axis); scales the transpose-back identity so cross-batch
    # A2A chunks carry zeros.
    gate = nc.declare_dram_parameter("gate", [P, 2], F32, False)
    if mask_any:
        mb = nc.declare_dram_parameter("mbias", [P, NT], F32, False)
    y = nc.declare_dram_parameter("y", [NQ, DIM], F32, True)

    with ExitStack() as ctx:
        tc = ctx.enter_context(tile.TileContext(nc))

        const = ctx.enter_context(tc.tile_pool(name="const", bufs=1))
        ident = const.tile([P, P], BF16, tag="ident")
        make_identity(nc, ident[:])
        gate_sb = const.tile([P, 2], F32, tag="gate_sb")
        nc.scalar.dma_start(gate_sb[:], gate.ap())
        ones_f = const.tile([P, P], F32, tag="ones_f")
        nc.vector.memset(ones_f[:], 1.0)
        ones_r = const.tile([P, P], F32R, tag="ones_r")
        nc.vector.tensor_copy(ones_r[:], ones_f[:])
        bo_sb = const.tile([1, DIM], F32R, tag="bo_sb")
        nc.scalar.dma_start(bo_sb[:], bo.ap())
        if mask_any:
            mb_sb = const.tile([P, NT], F32, tag="mb_sb")
            nc.scalar.dma_start(mb_sb[:], mb.ap())

        # ---- persistent SBUF ----
        big = ctx.enter_context(tc.tile_pool(name="big", bufs=1))
        wk_all = big.tile([P, ND, IC], BF16, tag="w", bufs=3, name="wk_all")
        wq_all = big.tile([P, ND, IC], BF16, tag="w", bufs=3, name="wq_all")
        wv_all = big.tile([P, ND, IC], BF16, tag="w", bufs=3, name="wv_all")
        wo_all = big.tile([P, ND, DIM], BF16, tag="woa", name="wo_all")
        wk_sb = [wk_all[:, c, :] for c in range(ND)]
        wq_sb = [wq_all[:, c, :] for c in range(ND)]
        wv_sb = [wv_all[:, c, :] for c in range(ND)]
        wo_sb = [wo_all[:, c, :] for c in range(ND)]
        xTb = [big.tile([P, ND, TB], BF16, tag="xT", bufs=4, name=f"xT{tb}")
               for tb in range(NTB)]

        def xT(c, lo, hi):
            # feature-chunk c, token range [lo, hi) (within one tb block)
            tb, off = divmod(lo, TB)
            return xTb[tb][:, c, off : off + (hi - lo)]
        q2 = [big.tile([P, N], BF16, tag="qk", bufs=4, name=f"q2_{hp}")
              for hp in range(2)]
        k2 = [big.tile([P, N], BF16, tag="qk", bufs=4, name=f"k2_{hp}")
              for hp in range(2)]
        v_all = big.tile([P, 4 * NT * (DH + 1)], BF16, tag="vx",
                         name="v_all")
        va = v_all[:].rearrange("p (h t c) -> p h t c", h=4, t=NT)
        aT = [big.tile([P, NQ], BF16, tag="aT", bufs=8, name=f"aT_{c}")
              for c in range(ND)]
        partial = [big.tile([P, TB], F32, tag="part", bufs=8, name=f"part{i}")
                   for i in range(8)]
        bo_rep = const.tile([P, DIM], F32, tag="bo_rep")

        # DMA issue order: one DMA per weight tensor on the scalar (ACT)
        # HWDGE queue - 4 configs total so neither the ACT sequencer nor the
        # queue's in-flight window ever backs up; x transposes on SP.
        nc.sync.dma_start(xTb[0][:], xt_in.ap()[0])
        nc.scalar.dma_start(wk_all[:], wk.ap().rearrange("(c p) f -> p c f",
                                                         c=ND))
        nc.sync.dma_start(xTb[1][:], xt_in.ap()[1])
        nc.scalar.dma_start(wq_all[:], wq.ap().rearrange("(c p) f -> p c f",
                                                         c=ND))
        nc.sync.dma_start(xTb[2][:], xt_in.ap()[2])
        nc.sync.dma_start(xTb[3][:], xt_in.ap()[3])
        nc.scalar.dma_start(wv_all[:], wv.ap().rearrange("(c p) f -> p c f",
                                                         c=ND))
        nc.scalar.dma_start(wo_all[:], wo.ap().rearrange("(c p) f -> p c f",
                                                         c=ND))

        nc.vector.memset(va[:, :, :, DH], 1.0)

        # A2A buffers (DRAM), one per head: chunk j (64 rows) -> core j;
        # chunk content = this head's [64 d, 512 tok] for quarter j%4,
        # zeros unless j//4 == my batch.
        dram = ctx.enter_context(tc.tile_pool(name="dram", bufs=1, space="DRAM"))
        a2a_in = [dram.tile([8 * DH, NQ], BF16, tag="a2a_in", bufs=4,
                            name=f"a2a_in{h}") for h in range(4)]
        a2a_out = [dram.tile([8 * DH, NQ], BF16, tag="a2a_out", bufs=4,
                             name=f"a2a_out{h}") for h in range(4)]

        ps_s = ctx.enter_context(tc.tile_pool(name="ps_s", bufs=2, space="PSUM"))
        ps_av = ctx.enter_context(tc.tile_pool(name="ps_av", bufs=2, space="PSUM"))
        ps_m = ctx.enter_context(tc.tile_pool(name="ps_m", bufs=2, space="PSUM"))

        sm = ctx.enter_context(tc.tile_pool(name="sm", bufs=1))

        # bias broadcast rows (uses the scores PSUM arena before attention)
        for nb in range(DIM // TB):
            psb = ps_s.tile([P, 2 * TB], F32, tag="s", name="psb")
            _mm(nc, psb[:, 0:TB], ones_r[0:1, :],
                bo_sb[:, nb * TB : (nb + 1) * TB])
            nc.vector.tensor_copy(bo_rep[:, nb * TB : (nb + 1) * TB],
                                  psb[:, 0:TB])

        # ---- projection / wo step closures (popped one per exp slot) ----
        def proj_step(kind, hp, tb):
            def run():
                pj = ps_m.tile([P, TB], F32, tag="m", name="pj")
                wsb = wq_sb if kind == "q" else wk_sb
                dest = q2[hp] if kind == "q" else k2[hp]
                for c in range(ND):
                    _mm(nc, pj[:], wsb[c][:, hp * P : (hp + 1) * P],
                        xT(c, tb * TB, (tb + 1) * TB),
                        start=(c == 0), stop=(c == ND - 1))
                nc.vector.tensor_copy(dest[:, tb * TB : (tb + 1) * TB], pj[:])
            return run

        def v_step(t):
            def run():
                pv = ps_m.tile([P, IC], F32, tag="m", name="pv")
                for c in range(ND):
                    _mm(nc, pv[:], xT(c, t * P, (t + 1) * P), wv_sb[c],
                        start=(c == 0), stop=(c == ND - 1))
                nc.vector.tensor_copy(
                    va[:, :, t, 0:DH],
                    pv[:].rearrange("p (h c) -> p h c", h=4))
            return run

        def wo_step(parity, t, nb, alt):
            def run():
                if alt:
                    pw_t = ps_s.tile([P, 2 * TB], F32, tag="s", name="pw")
                    pw = pw_t[:, 0:TB]
                else:
                    pw = ps_m.tile([P, TB], F32, tag="m", name="pw")[:]
                for m in range(4):
                    _mm(nc, pw,
                        aT[2 * m + parity][:, t * P : (t + 1) * P],
                        wo_sb[2 * m + parity][:, nb * TB : (nb + 1) * TB],
                        start=(m == 0), stop=(m == 3))
                if parity == 0:
                    nc.vector.tensor_add(
                        partial[t * 2 + nb][:], pw,
                        bo_rep[:, nb * TB : (nb + 1) * TB])
                else:
                    fo = sm.tile([P, TB], F32, tag="fo", bufs=4, name="fo")
                    nc.vector.tensor_add(fo[:], pw, partial[t * 2 + nb][:])
                    nc.sync.dma_start(
                        y.ap()[t * P : (t + 1) * P, nb * TB : (nb + 1) * TB],
                        fo[:])
            return run

        # stage 1: only the projections that gate the first scores
        proj_step("k", 0, 0)()
        proj_step("k", 0, 1)()
        proj_step("q", 0, 0)()
        proj_step("q", 0, 1)()
        proj_step("k", 0, 2)()
        proj_step("k", 0, 3)()

        # pop queue: exp-slot index -> closure (16 strips x 8 slots = 128).
        # v chunk t is consumed by the (full-strip-lagged) AV at absolute
        # slot 8 + t//2; popping v(t) at slot t-1 stays ahead (same-engine
        # program order guarantees no deadlock even at zero slack).
        popq = {0: [v_step(0), v_step(1)]}
        for t in range(2, NT):
            popq.setdefault(t - 1, []).append(v_step(t))
        popq.setdefault(15, []).append(proj_step("q", 0, 2))
        popq.setdefault(17, []).append(proj_step("q", 0, 3))
        for i in range(NTB):
            popq.setdefault(18 + 4 * i, []).append(proj_step("k", 1, i))
            popq.setdefault(34 + 4 * i, []).append(proj_step("q", 1, i))

        # ---- attention strips: one head at a time; AV for strip s runs
        # during strip s+1 (one pair per exp slot) so the interleaved
        # v/k/q projection steps never sit behind a waiting AV matmul.
        def do_av(st, jt):
            et = st["es"][jt // 2]
            off = (jt % 2) * TB
            if st["av"] is None:
                st["av"] = ps_av.tile([P, 4 * (DH + 1)], F32, tag="av",
                                      name="av")
                # four accumulation groups share this bank: zero once and
                # accumulate with start=False so their start bits can't
                # clobber each other's partials
                nc.vector.memset(st["av"][:], 0.0)
            av = st["av"]
            for c in range(4):
                nc.tensor.matmul(
                    av[:, c * (DH + 1) : (c + 1) * (DH + 1)],
                    et[:, off + c * P : off + (c + 1) * P],
                    va[:, st["h"], jt, :],
                    start=False, stop=(jt == NT - 1),
                    skip_group_check=True)

        def norm_dve(st):
            """Normalize (DVE); returns deferred PE-transpose + staging."""
            av, h, ib = st["av"], st["h"], st["ib"]
            rcp = sm.tile([P, 4], F32, tag="rcp", bufs=3, name="rcp")
            with nc.allow_low_precision("softmax denom"):
                nc.vector.reciprocal(rcp[:], av[:, DH :: DH + 1])
            onrm = sm.tile([P, 4 * DH], BF16, tag="onrm", bufs=3, name="onrm")
            for c in range(4):
                nc.vector.tensor_scalar_mul(
                    onrm[:, c * DH : (c + 1) * DH],
                    av[:, c * (DH + 1) : c * (DH + 1) + DH],
                    rcp[:, c : c + 1])

            def part2():
                oT = ps_m.tile([DH, TB], BF16, tag="m", name="oT")
                for c in range(4):
                    nc.tensor.transpose(oT[:, c * P : (c + 1) * P],
                                        onrm[:, c * DH : (c + 1) * DH],
                                        ident[:])
                for g in range(2):
                    stg = sm.tile([DH, TB], BF16, tag="stg", bufs=4,
                                  name="stg")
                    nc.vector.tensor_scalar_mul(stg[:], oT[:],
                                                gate_sb[0:DH, g : g + 1])
                    row = (4 * g + ib) * DH
                    nc.sync.dma_start(a2a_in[h][row : row + DH, :], stg[:])
                if ib == NTB - 1:
                    fire_a2a(h)
            return part2

        def fire_a2a(h):
            nc.gpsimd.collective_compute(
                "AllToAll",
                mybir.AluOpType.bypass,
                replica_groups=[[0, 1, 2, 3, 4, 5, 6, 7]],
                ins=[a2a_in[h].opt()],
                outs=[a2a_out[h].opt()],
            )

        def recv_a2a(h):
            # receive: aT[2m+hp] rows [e*64, e*64+64) = chunk m + chunk 4+m
            # (exactly one is nonzero; SWDGE accumulate adds the other).
            # Emitted only after the LAST collective so the SP/Pool queues
            # never block a later collective's issue while waiting.
            hp, e = divmod(h, 2)
            for m in range(GROUP):
                dst = aT[2 * m + hp][e * DH : (e + 1) * DH, :]
                nc.sync.dma_start(dst, a2a_out[h][m * DH : (m + 1) * DH, :])
                nc.gpsimd.dma_start(
                    dst, a2a_out[h][(4 + m) * DH : (5 + m) * DH, :],
                    accum_op=mybir.AluOpType.add)

        # Strips 0-1 lag their AV by a full strip (the v projections are
        # still streaming in); strips 2+ run their own AV in-strip with a
        # 2-pair lag, so each head's A2A fires right at its last strip's
        # end and the four collectives never queue on each other.
        slot = 0
        prev = None
        pend2 = []
        for h in range(4):
            hp, e = divmod(h, 2)
            ksb = k2[hp][e * DH : (e + 1) * DH, :]
            qsb = q2[hp][e * DH : (e + 1) * DH, :]
            tp = (e * DH, 0)
            for ib in range(NTB):
                isl = slice(ib * TB, (ib + 1) * TB)
                s = 4 * h + ib
                in_strip = s >= 2
                cur = {"h": h, "ib": ib, "es": [None] * NJP, "av": None}
                for jp in range(NJP):
                    psS = ps_s.tile([P, 2 * TB], F32, tag="s", name="psS")
                    for sub in range(2):
                        jt = 2 * jp + sub
                        jsl = slice(jt * P, (jt + 1) * P)
                        _mm(nc, psS[:, sub * TB : (sub + 1) * TB],
                            ksb[:, jsl], qsb[:, isl], tile_position=tp)
                        if mask_any:
                            nc.vector.tensor_scalar_add(
                                psS[:, sub * TB : (sub + 1) * TB],
                                psS[:, sub * TB : (sub + 1) * TB],
                                mb_sb[:, jt : jt + 1])
                    e_t = sm.tile([P, 2 * TB], BF16, tag="e", bufs=16,
                                  name="e")
                    nc.scalar.activation(e_t[:], psS[:], EXP)
                    cur["es"][jp] = e_t
                    for fn in popq.pop(slot, ()):
                        fn()
                    if jp == 2 and pend2:
                        for fn in pend2:
                            fn()
                        pend2 = []
                    if prev is not None:
                        do_av(prev, 2 * jp)
                        do_av(prev, 2 * jp + 1)
                    if in_strip and jp >= 2:
                        do_av(cur, 2 * (jp - 2))
                        do_av(cur, 2 * (jp - 2) + 1)
                    slot += 1
                if in_strip:
                    if prev is not None:
                        pend2.append(norm_dve(prev))
                        prev = None
                    for jp in range(NJP - 2, NJP):
                        do_av(cur, 2 * jp)
                        do_av(cur, 2 * jp + 1)
                    pc = norm_dve(cur)
                    if ib == NTB - 1:
                        pc()
                    else:
                        pend2.append(pc)
                else:
                    if prev is not None:
                        pend2.append(norm_dve(prev))
                    prev = cur

        for h in range(4):
            recv_a2a(h)

        # wo even stage: runs inside the final collective's window
        wi = 0
        for t in range(NQ // P):
            for nb in range(DIM // TB):
                wo_step(0, t, nb, alt=(wi % 2 == 1))()
                wi += 1
        # drain any unfired pop steps (none expected)
        for i in sorted(popq):
            for fn in popq.pop(i):
                fn()

        # wo odd stage (alternate PSUM arenas to avoid WAR stalls)
        wi = 0
        for t in range(NQ // P):
            for nb in range(DIM // TB):
                wo_step(1, t, nb, alt=(wi % 2 == 1))()
                wi += 1

    nc.compile()
    return nc


def _get_nc(mask_any: bool) -> bass.Bass:
    if mask_any not in _CACHE:
        _CACHE[mask_any] = _build(mask_any)
    return _CACHE[mask_any]


def _in_maps(x, mask, Wq, Wkv, Wo, bo, mask_any):
    bf = ml_dtypes.bfloat16
    bo2 = np.ascontiguousarray(np.asarray(bo, np.float32).reshape(1, DIM))
    wo_bf = np.ascontiguousarray(np.asarray(Wo, np.float32).astype(bf))
    maps = []
    for c in range(8):
        g, r = divmod(c, GROUP)
        csl = slice(r * IC, (r + 1) * IC)
        m = {
            "xt_b": np.ascontiguousarray(
                x[g].reshape(NTB, TB, ND, P).transpose(0, 3, 2, 1).astype(bf)),
            "wq_s": np.ascontiguousarray(
                (Wq[:, csl] * np.float32(SCALE)).astype(bf)),
            "wk_s": np.ascontiguousarray(Wkv[:, csl].astype(bf)),
            "wv_s": np.ascontiguousarray(
                Wkv[:, INNER + r * IC : INNER + (r + 1) * IC].astype(bf)),
            "wo": wo_bf,
            "bo": bo2,
            "gate": np.ascontiguousarray(
                np.tile(np.array([[1.0 - g, float(g)]], np.float32), (P, 1))),
        }
        if mask_any:
            mvec = np.where(mask[g], np.float32(NEG), np.float32(0.0)).astype(
                np.float32)
            m["mbias"] = np.ascontiguousarray(mvec.reshape(NT, P).T)
        maps.append(m)
    return maps


_RUNNER = {}


def _get_runner(mask_any: bool):
    """Cached jax-jitted SPMD executor for the Bass module."""
    if mask_any in _RUNNER:
        return _RUNNER[mask_any]
    import jax
    from jax.sharding import Mesh, PartitionSpec
    from jax.experimental.shard_map import shard_map
    from concourse import bass2jax

    nc = _get_nc(mask_any)
    bass2jax.install_neuronx_cc_hook()

    partition_name = (
        nc.partition_id_tensor.name if nc.partition_id_tensor else None
    )
    in_names, out_names, out_avals = [], [], []
    for alloc in nc.m.functions[0].allocations:
        if not isinstance(alloc, mybir.MemoryLocationSet):
            continue
        name = alloc.memorylocations[0].name
        if alloc.kind == "ExternalInput":
            if name != partition_name:
                in_names.append(name)
        elif alloc.kind == "ExternalOutput":
            shape = tuple(alloc.tensor_shape)
            dtype = mybir.dt.np(alloc.dtype)
            out_names.append(name)
            out_avals.append(jax.core.ShapedArray(shape, dtype))
    n_params = len(in_names)
    n_outs = len(out_avals)
    all_names = list(in_names) + list(out_names)
    if partition_name is not None:
        all_names.append(partition_name)
    donate = tuple(range(n_params, n_params + n_outs))

    def _body(*args):
        operands = list(args)
        if partition_name is not None:
            operands.append(bass2jax.partition_id_tensor())
        outs = bass2jax._bass_exec_p.bind(
            *operands,
            out_avals=tuple(out_avals),
            in_names=tuple(all_names),
            out_names=tuple(out_names),
            lowering_input_output_aliases=(),
            sim_require_finite=True,
            sim_require_nnan=True,
            nc=nc,
        )
        return tuple(outs)

    devices = jax.devices()[:8]
    mesh = Mesh(np.asarray(devices), ("core",))
    in_specs = (PartitionSpec("core"),) * (n_params + n_outs)
    out_specs = (PartitionSpec("core"),) * n_outs
    sharded = jax.jit(
        shard_map(_body, mesh=mesh, in_specs=in_specs, out_specs=out_specs,
                  check_rep=False),
        donate_argnums=donate,
        keep_unused=True,
    )
    zero_shapes = [tuple(a.shape) for a in out_avals]
    zero_dtypes = [a.dtype for a in out_avals]

    def call(maps):
        concat_in = [
            np.concatenate([np.asarray(maps[c][nm]) for c in range(8)], axis=0)
            for nm in in_names
        ]
        concat_zeros = [
            np.zeros((8 * s[0], *s[1:]), d)
            for s, d in zip(zero_shapes, zero_dtypes)
        ]
        out_arrs = sharded(*concat_in, *concat_zeros)
        return [
            {
                nm: np.asarray(out_arrs[i]).reshape(8, *zero_shapes[i])[c]
                for i, nm in enumerate(out_names)
            }
            for c in range(8)
        ]

    _RUNNER[mask_any] = call
    return call


def run(x, mask, Wq, Wkv, Wo, bo, trace=False):
    x = np.asarray(x, np.float32)
    mask = np.asarray(mask, bool)
    Wq = np.asarray(Wq, np.float32)
    Wkv = np.asarray(Wkv, np.float32)
    Wo = np.asarray(Wo, np.float32)
    bo = np.asarray(bo, np.float32)
    mask_any = bool(mask.any())
    maps = _in_maps(x, mask, Wq, Wkv, Wo, bo, mask_any)
    results = _get_runner(mask_any)(maps)
    out = np.empty((B, N, DIM), np.float32)
    for c in range(8):
        g, r = divmod(c, GROUP)
        out[g, r * NQ : (r + 1) * NQ, :] = results[c]["y"]
    return out, results


def kernel(x, mask, Wq, Wkv, Wo, bo):
    out, _ = run(x, mask, Wq, Wkv, Wo, bo)
    return out


# revision 4
# speedup vs baseline: 1.0481x; 1.0037x over previous
"""Distributed multi-head attention kernel for one TRN2 chip (8 NeuronCores).

Problem: y = Attention(x) with b=2, n=2048, dim=1024, heads=16, dim_head=64.

Sharding (data + tensor parallel):
  core c: batch g = c // 4, head-group r = c % 4 (4 heads = 256 inner dims).

Design (v2):
  - Host pre-converts x (transposed to feature-major xT[p, c, t] =
    x[t, 128c+p]) and Wq/Wk/Wv/Wo to bf16, with SCALE folded into Wq, so
    no on-device transposes or conversions are needed.
  - Attention in single-head strips (h, ib): scores psS [128 j, 2*512 i]
    (two j-chunks per PSUM tile pair), exp on ACT -> e bf16 tiles. The
    ACT engine (exp throughput) paces the attention phase; q/k/v
    projections are interleaved into the PE stream one step per exp slot.
  - AV in transposed layout: out [i=128 tok, 65] per i-chunk with a
    ones-column in the v tile giving the softmax denominator in col 64.
    Full 128-partition outputs halve PE cost vs the [65, i] layout, and
    normalization becomes a cheap per-partition DVE scalar op. The four
    i-chunk accumulation groups share one PSUM bank, so the bank is
    zeroed once and all matmuls accumulate with start=False.
  - AV runs in-strip with a 2-pair lag (strips 0-1 lag a full strip while
    the v projections stream in); normalize+transpose-back+staging for
    strip s is deferred into strip s+1 so the PE never stalls on the DVE
    normalization chain at strip boundaries.
  - Transpose-back to [d, tok] via PE with a bf16 identity; two
    gate-scaled DVE copies stage it into the A2A buffer (the wrong-batch
    copy writes zeros).
  - Exchange: four 8-core AllToAll collectives (one per head, fired
    inline as each head's last strip closes) so only the smallest
    possible transfer sits in the tail; receives (plain + SWDGE
    accumulate, summing the two batch halves) are deferred until after
    the last collective so no queue ever blocks a later collective's
    issue.
  - Output projection in two stages: even aT chunks (head pairs 0/1)
    run inside the later collectives' windows; odd chunks in the tail,
    alternating PSUM arenas to avoid write-after-read stalls.
"""

import sys

if "/opt/trn_rl_repo" not in sys.path:
    sys.path.insert(0, "/opt/trn_rl_repo")

from contextlib import ExitStack

import ml_dtypes
import numpy as np

import concourse.bass as bass
from concourse import bacc
import concourse.mybir as mybir
import concourse.tile as tile
from concourse.masks import make_identity

F32 = mybir.dt.float32
F32R = mybir.dt.float32r
BF16 = mybir.dt.bfloat16
EXP = mybir.ActivationFunctionType.Exp

B, N, DIM = 2, 2048, 1024
HEADS, DH = 16, 64
INNER = HEADS * DH            # 1024
SCALE = DH ** -0.5
GROUP = 4                     # tensor-parallel group size (cores per batch)
IC = INNER // GROUP           # 256 inner dims per core (4 heads)
NQ = N // GROUP               # 512 output tokens per core
NEG = -1.0e30

P = 128
TB = 512                      # i-block (moving-dim max)
NT = N // P                   # 16 j-chunks
NJP = NT // 2                 # 8 j-chunk pairs (one psS/exp per pair)
ND = DIM // P                 # 8 contraction chunks
NTB = N // TB                 # 4 token blocks
LAGP = 2                      # AV trails exp by LAGP j-chunk-pairs

_CACHE = {}


def _mm(nc, out, lhsT, rhs, start=True, stop=True, tile_position=None):
    nc.tensor.matmul(
        out, lhsT, rhs, start=start, stop=stop, tile_position=tile_position
    )


def _build(mask_any: bool) -> bass.Bass:
    nc = bacc.Bacc()

    xt_in = nc.declare_dram_parameter("xt_b", [NTB, P, ND, TB], BF16, False)
    wq = nc.declare_dram_parameter("wq_s", [DIM, IC], BF16, False)
    wk = nc.declare_dram_parameter("wk_s", [DIM, IC], BF16, False)
    wv = nc.declare_dram_parameter("wv_s", [DIM, IC], BF16, False)
    wo = nc.declare_dram_parameter("wo", [INNER, DIM], BF16, False)
    bo = nc.declare_dram_parameter("bo", [1, DIM], F32R, False)
    # gate[:, g] = 1.0 iff this core handles batch g (replicated down the
    # partition axis); scales the transpose-back identity so cross-batch
    # A2A chunks carry zeros.
    gate = nc.declare_dram_parameter("gate", [P, 2], F32, False)
    if mask_any:
        mb = nc.declare_dram_parameter("mbias", [P, NT], F32, False)
    y = nc.declare_dram_parameter("y", [NQ, DIM], F32, True)

    with ExitStack() as ctx:
        tc = ctx.enter_context(tile.TileContext(nc))

        const = ctx.enter_context(tc.tile_pool(name="const", bufs=1))
        ident = const.tile([P, P], BF16, tag="ident")
        make_identity(nc, ident[:])
        gate_sb = const.tile([P, 2], F32, tag="gate_sb")
        nc.scalar.dma_start(gate_sb[:], gate.ap())
        ones_f = const.tile([P, P], F32, tag="ones_f")
        nc.vector.memset(ones_f[:], 1.0)
        ones_r = const.tile([P, P], F32R, tag="ones_r")
        nc.vector.tensor_copy(ones_r[:], ones_f[:])
        bo_sb = const.tile([1, DIM], F32R, tag="bo_sb")
        nc.scalar.dma_start(bo_sb[:], bo.ap())
        if mask_any:
            mb_sb = const.tile([P, NT], F32, tag="mb_sb")
            nc.scalar.dma_start(mb_sb[:], mb.ap())

        # ---- persistent SBUF ----
        big = ctx.enter_context(tc.tile_pool(name="big", bufs=1))
        wk_all = big.tile([P, ND, IC], BF16, tag="w", bufs=3, name="wk_all")
        wq_all = big.tile([P, ND, IC], BF16, tag="w", bufs=3, name="wq_all")
        wv_all = big.tile([P, ND, IC], BF16, tag="w", bufs=3, name="wv_all")
        wo_all = big.tile([P, ND, DIM], BF16, tag="woa", name="wo_all")
        wk_sb = [wk_all[:, c, :] for c in range(ND)]
        wq_sb = [wq_all[:, c, :] for c in range(ND)]
        wv_sb = [wv_all[:, c, :] for c in range(ND)]
        wo_sb = [wo_all[:, c, :] for c in range(ND)]
        xTb = [big.tile([P, ND, TB], BF16, tag="xT", bufs=4, name=f"xT{tb}")
               for tb in range(NTB)]

        def xT(c, lo, hi):
            # feature-chunk c, token range [lo, hi) (within one tb block)
            tb, off = divmod(lo, TB)
            return xTb[tb][:, c, off : off + (hi - lo)]
        q2 = [big.tile([P, N], BF16, tag="qk", bufs=4, name=f"q2_{hp}")
              for hp in range(2)]
        k2 = [big.tile([P, N], BF16, tag="qk", bufs=4, name=f"k2_{hp}")
              for hp in range(2)]
        v_all = big.tile([P, 4 * NT * (DH + 1)], BF16, tag="vx",
                         name="v_all")
        va = v_all[:].rearrange("p (h t c) -> p h t c", h=4, t=NT)
        aT = [big.tile([P, NQ], BF16, tag="aT", bufs=8, name=f"aT_{c}")
              for c in range(ND)]
        partial = [big.tile([P, TB], F32, tag="part", bufs=8, name=f"part{i}")
                   for i in range(8)]
        bo_rep = const.tile([P, DIM], F32, tag="bo_rep")

        # DMA issue order: one DMA per weight tensor on the scalar (ACT)
        # HWDGE queue - 4 configs total so neither the ACT sequencer nor the
        # queue's in-flight window ever backs up; x transposes on SP.
        nc.sync.dma_start(xTb[0][:], xt_in.ap()[0])
        nc.scalar.dma_start(wk_all[:], wk.ap().rearrange("(c p) f -> p c f",
                                                         c=ND))
        nc.sync.dma_start(xTb[1][:], xt_in.ap()[1])
        nc.scalar.dma_start(wq_all[:], wq.ap().rearrange("(c p) f -> p c f",
                                                         c=ND))
        nc.sync.dma_start(xTb[2][:], xt_in.ap()[2])
        nc.sync.dma_start(xTb[3][:], xt_in.ap()[3])
        nc.scalar.dma_start(wv_all[:], wv.ap().rearrange("(c p) f -> p c f",
                                                         c=ND))
        nc.scalar.dma_start(wo_all[:], wo.ap().rearrange("(c p) f -> p c f",
                                                         c=ND))

        nc.vector.memset(va[:, :, :, DH], 1.0)

        # A2A buffers (DRAM), one per head: chunk j (64 rows) -> core j;
        # chunk content = this head's [64 d, 512 tok] for quarter j%4,
        # zeros unless j//4 == my batch.
        dram = ctx.enter_context(tc.tile_pool(name="dram", bufs=1, space="DRAM"))
        a2a_in = [dram.tile([8 * DH, NQ], BF16, tag="a2a_in", bufs=4,
                            name=f"a2a_in{h}") for h in range(4)]
        a2a_out = [dram.tile([8 * DH, NQ], BF16, tag="a2a_out", bufs=4,
                             name=f"a2a_out{h}") for h in range(4)]

        ps_s = ctx.enter_context(tc.tile_pool(name="ps_s", bufs=2, space="PSUM"))
        ps_av = ctx.enter_context(tc.tile_pool(name="ps_av", bufs=2, space="PSUM"))
        ps_m = ctx.enter_context(tc.tile_pool(name="ps_m", bufs=2, space="PSUM"))

        sm = ctx.enter_context(tc.tile_pool(name="sm", bufs=1))

        # bias broadcast rows (uses the scores PSUM arena before attention)
        for nb in range(DIM // TB):
            psb = ps_s.tile([P, 2 * TB], F32, tag="s", name="psb")
            _mm(nc, psb[:, 0:TB], ones_r[0:1, :],
                bo_sb[:, nb * TB : (nb + 1) * TB])
            nc.vector.tensor_copy(bo_rep[:, nb * TB : (nb + 1) * TB],
                                  psb[:, 0:TB])

        # ---- projection / wo step closures (popped one per exp slot) ----
        def proj_step(kind, hp, tb):
            def run():
                pj = ps_m.tile([P, TB], F32, tag="m", name="pj")
                wsb = wq_sb if kind == "q" else wk_sb
                dest = q2[hp] if kind == "q" else k2[hp]
                for c in range(ND):
                    _mm(nc, pj[:], wsb[c][:, hp * P : (hp + 1) * P],
                        xT(c, tb * TB, (tb + 1) * TB),
                        start=(c == 0), stop=(c == ND - 1))
                nc.vector.tensor_copy(dest[:, tb * TB : (tb + 1) * TB], pj[:])
            return run

        def v_step(t):
            def run():
                pv = ps_m.tile([P, IC], F32, tag="m", name="pv")
                for c in range(ND):
                    _mm(nc, pv[:], xT(c, t * P, (t + 1) * P), wv_sb[c],
                        start=(c == 0), stop=(c == ND - 1))
                nc.vector.tensor_copy(
                    va[:, :, t, 0:DH],
                    pv[:].rearrange("p (h c) -> p h c", h=4))
            return run

        def wo_step(parity, t, nb, alt):
            def run():
                if alt:
                    pw_t = ps_s.tile([P, 2 * TB], F32, tag="s", name="pw")
                    pw = pw_t[:, 0:TB]
                else:
                    pw = ps_m.tile([P, TB], F32, tag="m", name="pw")[:]
                for m in range(4):
                    _mm(nc, pw,
                        aT[2 * m + parity][:, t * P : (t + 1) * P],
                        wo_sb[2 * m + parity][:, nb * TB : (nb + 1) * TB],
                        start=(m == 0), stop=(m == 3))
                if parity == 0:
                    nc.vector.tensor_add(
                        partial[t * 2 + nb][:], pw,
                        bo_rep[:, nb * TB : (nb + 1) * TB])
                else:
                    fo = sm.tile([P, TB], F32, tag="fo", bufs=6, name="fo")
                    nc.vector.tensor_add(fo[:], pw, partial[t * 2 + nb][:])
                    nc.sync.dma_start(
                        y.ap()[t * P : (t + 1) * P, nb * TB : (nb + 1) * TB],
                        fo[:])
            return run

        # stage 1: only the projections that gate the first scores
        proj_step("k", 0, 0)()
        proj_step("k", 0, 1)()
        proj_step("q", 0, 0)()
        proj_step("q", 0, 1)()
        proj_step("k", 0, 2)()
        proj_step("k", 0, 3)()

        # pop queue: exp-slot index -> closure (16 strips x 8 slots = 128).
        # v chunk t is consumed by the (full-strip-lagged) AV at absolute
        # slot 8 + t//2; popping v(t) at slot t-1 stays ahead (same-engine
        # program order guarantees no deadlock even at zero slack).
        popq = {0: [v_step(0), v_step(1)]}
        for t in range(2, NT):
            popq.setdefault(t - 1, []).append(v_step(t))
        popq.setdefault(15, []).append(proj_step("q", 0, 2))
        popq.setdefault(17, []).append(proj_step("q", 0, 3))
        for i in range(NTB):
            popq.setdefault(18 + 4 * i, []).append(proj_step("k", 1, i))
            popq.setdefault(34 + 4 * i, []).append(proj_step("q", 1, i))

        # ---- attention strips: one head at a time; AV for strip s runs
        # during strip s+1 (one pair per exp slot) so the interleaved
        # v/k/q projection steps never sit behind a waiting AV matmul.
        def do_av(st, jt):
            et = st["es"][jt // 2]
            off = (jt % 2) * TB
            if st["av"] is None:
                st["av"] = ps_av.tile([P, 4 * (DH + 1)], F32, tag="av",
                                      name="av")
                # four accumulation groups share this bank: zero once and
                # accumulate with start=False so their start bits can't
                # clobber each other's partials
                nc.vector.memset(st["av"][:], 0.0)
            av = st["av"]
            for c in range(4):
                nc.tensor.matmul(
                    av[:, c * (DH + 1) : (c + 1) * (DH + 1)],
                    et[:, off + c * P : off + (c + 1) * P],
                    va[:, st["h"], jt, :],
                    start=False, stop=(jt == NT - 1),
                    skip_group_check=True)

        def norm_dve(st):
            """Normalize (DVE); returns deferred PE-transpose + staging."""
            av, h, ib = st["av"], st["h"], st["ib"]
            rcp = sm.tile([P, 4], F32, tag="rcp", bufs=4, name="rcp")
            with nc.allow_low_precision("softmax denom"):
                nc.vector.reciprocal(rcp[:], av[:, DH :: DH + 1])
            onrm = sm.tile([P, 4 * DH], BF16, tag="onrm", bufs=4, name="onrm")
            for c in range(4):
                nc.vector.tensor_scalar_mul(
                    onrm[:, c * DH : (c + 1) * DH],
                    av[:, c * (DH + 1) : c * (DH + 1) + DH],
                    rcp[:, c : c + 1])

            def part2():
                oT = ps_m.tile([DH, TB], BF16, tag="m", name="oT")
                for c in range(4):
                    nc.tensor.transpose(oT[:, c * P : (c + 1) * P],
                                        onrm[:, c * DH : (c + 1) * DH],
                                        ident[:])
                for g in range(2):
                    stg = sm.tile([DH, TB], BF16, tag="stg", bufs=6,
                                  name="stg")
                    nc.vector.tensor_scalar_mul(stg[:], oT[:],
                                                gate_sb[0:DH, g : g + 1])
                    row = (4 * g + ib) * DH
                    nc.sync.dma_start(a2a_in[h][row : row + DH, :], stg[:])
                if ib == NTB - 1:
                    fire_a2a(h)
            return part2

        def fire_a2a(h):
            nc.gpsimd.collective_compute(
                "AllToAll",
                mybir.AluOpType.bypass,
                replica_groups=[[0, 1, 2, 3, 4, 5, 6, 7]],
                ins=[a2a_in[h].opt()],
                outs=[a2a_out[h].opt()],
            )

        def recv_a2a(h, use_dve=False):
            # receive: aT[2m+hp] rows [e*64, e*64+64) = chunk m + chunk 4+m
            # (exactly one is nonzero). SWDGE accumulate adds the second
            # half; the last head instead loads both halves and adds on DVE
            # (idle in the tail) to skip the serial SWDGE generation chain.
            # Emitted only after the LAST collective so the SP/Pool queues
            # never block a later collective's issue while waiting.
            hp, e = divmod(h, 2)
            for m in range(GROUP):
                dst = aT[2 * m + hp][e * DH : (e + 1) * DH, :]
                nc.sync.dma_start(dst, a2a_out[h][m * DH : (m + 1) * DH, :])
                if use_dve:
                    tmp = sm.tile([DH, NQ], BF16, tag="rtmp", bufs=4,
                                  name="rtmp")
                    nc.sync.dma_start(
                        tmp[:], a2a_out[h][(4 + m) * DH : (5 + m) * DH, :])
                    nc.vector.tensor_add(dst, dst, tmp[:])
                else:
                    nc.gpsimd.dma_start(
                        dst, a2a_out[h][(4 + m) * DH : (5 + m) * DH, :],
                        accum_op=mybir.AluOpType.add)

        # Strips 0-1 lag their AV by a full strip (the v projections are
        # still streaming in); strips 2+ run their own AV in-strip with a
        # 2-pair lag, so each head's A2A fires right at its last strip's
        # end and the four collectives never queue on each other.
        slot = 0
        prev = None
        pend2 = []
        for h in range(4):
            hp, e = divmod(h, 2)
            ksb = k2[hp][e * DH : (e + 1) * DH, :]
            qsb = q2[hp][e * DH : (e + 1) * DH, :]
            tp = (e * DH, 0)
            for ib in range(NTB):
                isl = slice(ib * TB, (ib + 1) * TB)
                s = 4 * h + ib
                in_strip = s >= 2
                cur = {"h": h, "ib": ib, "es": [None] * NJP, "av": None}
                for jp in range(NJP):
                    psS = ps_s.tile([P, 2 * TB], F32, tag="s", name="psS")
                    for sub in range(2):
                        jt = 2 * jp + sub
                        jsl = slice(jt * P, (jt + 1) * P)
                        _mm(nc, psS[:, sub * TB : (sub + 1) * TB],
                            ksb[:, jsl], qsb[:, isl], tile_position=tp)
                        if mask_any:
                            nc.vector.tensor_scalar_add(
                                psS[:, sub * TB : (sub + 1) * TB],
                                psS[:, sub * TB : (sub + 1) * TB],
                                mb_sb[:, jt : jt + 1])
                    e_t = sm.tile([P, 2 * TB], BF16, tag="e", bufs=20,
                                  name="e")
                    nc.scalar.activation(e_t[:], psS[:], EXP)
                    cur["es"][jp] = e_t
                    for fn in popq.pop(slot, ()):
                        fn()
                    if jp == 2 and pend2:
                        for fn in pend2:
                            fn()
                        pend2 = []
                    if prev is not None:
                        do_av(prev, 2 * jp)
                        do_av(prev, 2 * jp + 1)
                    if in_strip and jp >= 3:
                        do_av(cur, 2 * (jp - 3))
                        do_av(cur, 2 * (jp - 3) + 1)
                    slot += 1
                if in_strip:
                    if prev is not None:
                        pend2.append(norm_dve(prev))
                        prev = None
                    for jp in range(NJP - 3, NJP):
                        do_av(cur, 2 * jp)
                        do_av(cur, 2 * jp + 1)
                    pc = norm_dve(cur)
                    if ib == NTB - 1:
                        pc()
                    else:
                        pend2.append(pc)
                else:
                    if prev is not None:
                        pend2.append(norm_dve(prev))
                    prev = cur

        for h in range(4):
            recv_a2a(h)

        # wo even stage: runs inside the final collective's window
        wi = 0
        for t in range(NQ // P):
            for nb in range(DIM // TB):
                wo_step(0, t, nb, alt=(wi % 2 == 1))()
                wi += 1
        # drain any unfired pop steps (none expected)
        for i in sorted(popq):
            for fn in popq.pop(i):
                fn()

        # wo odd stage (alternate PSUM arenas to avoid WAR stalls)
        wi = 0
        for t in range(NQ // P):
            for nb in range(DIM // TB):
                wo_step(1, t, nb, alt=(wi % 2 == 1))()
                wi += 1

    nc.compile()
    return nc


def _get_nc(mask_any: bool) -> bass.Bass:
    if mask_any not in _CACHE:
        _CACHE[mask_any] = _build(mask_any)
    return _CACHE[mask_any]


def _in_maps(x, mask, Wq, Wkv, Wo, bo, mask_any):
    bf = ml_dtypes.bfloat16
    bo2 = np.ascontiguousarray(np.asarray(bo, np.float32).reshape(1, DIM))
    wo_bf = np.ascontiguousarray(np.asarray(Wo, np.float32).astype(bf))
    maps = []
    for c in range(8):
        g, r = divmod(c, GROUP)
        csl = slice(r * IC, (r + 1) * IC)
        m = {
            "xt_b": np.ascontiguousarray(
                x[g].reshape(NTB, TB, ND, P).transpose(0, 3, 2, 1).astype(bf)),
            "wq_s": np.ascontiguousarray(
                (Wq[:, csl] * np.float32(SCALE)).astype(bf)),
            "wk_s": np.ascontiguousarray(Wkv[:, csl].astype(bf)),
            "wv_s": np.ascontiguousarray(
                Wkv[:, INNER + r * IC : INNER + (r + 1) * IC].astype(bf)),
            "wo": wo_bf,
            "bo": bo2,
            "gate": np.ascontiguousarray(
                np.tile(np.array([[1.0 - g, float(g)]], np.float32), (P, 1))),
        }
        if mask_any:
            mvec = np.where(mask[g], np.float32(NEG), np.float32(0.0)).astype(
                np.float32)
            m["mbias"] = np.ascontiguousarray(mvec.reshape(NT, P).T)
        maps.append(m)
    return maps


_RUNNER = {}


def _get_runner(mask_any: bool):
    """Cached jax-jitted SPMD executor for the Bass module."""
    if mask_any in _RUNNER:
        return _RUNNER[mask_any]
    import jax
    from jax.sharding import Mesh, PartitionSpec
    from jax.experimental.shard_map import shard_map
    from concourse import bass2jax

    nc = _get_nc(mask_any)
    bass2jax.install_neuronx_cc_hook()

    partition_name = (
        nc.partition_id_tensor.name if nc.partition_id_tensor else None
    )
    in_names, out_names, out_avals = [], [], []
    for alloc in nc.m.functions[0].allocations:
        if not isinstance(alloc, mybir.MemoryLocationSet):
            continue
        name = alloc.memorylocations[0].name
        if alloc.kind == "ExternalInput":
            if name != partition_name:
                in_names.append(name)
        elif alloc.kind == "ExternalOutput":
            shape = tuple(alloc.tensor_shape)
            dtype = mybir.dt.np(alloc.dtype)
            out_names.append(name)
            out_avals.append(jax.core.ShapedArray(shape, dtype))
    n_params = len(in_names)
    n_outs = len(out_avals)
    all_names = list(in_names) + list(out_names)
    if partition_name is not None:
        all_names.append(partition_name)
    donate = tuple(range(n_params, n_params + n_outs))

    def _body(*args):
        operands = list(args)
        if partition_name is not None:
            operands.append(bass2jax.partition_id_tensor())
        outs = bass2jax._bass_exec_p.bind(
            *operands,
            out_avals=tuple(out_avals),
            in_names=tuple(all_names),
            out_names=tuple(out_names),
            lowering_input_output_aliases=(),
            sim_require_finite=True,
            sim_require_nnan=True,
            nc=nc,
        )
        return tuple(outs)

    devices = jax.devices()[:8]
    mesh = Mesh(np.asarray(devices), ("core",))
    in_specs = (PartitionSpec("core"),) * (n_params + n_outs)
    out_specs = (PartitionSpec("core"),) * n_outs
    sharded = jax.jit(
        shard_map(_body, mesh=mesh, in_specs=in_specs, out_specs=out_specs,
                  check_rep=False),
        donate_argnums=donate,
        keep_unused=True,
    )
    zero_shapes = [tuple(a.shape) for a in out_avals]
    zero_dtypes = [a.dtype for a in out_avals]

    def call(maps):
        concat_in = [
            np.concatenate([np.asarray(maps[c][nm]) for c in range(8)], axis=0)
            for nm in in_names
        ]
        concat_zeros = [
            np.zeros((8 * s[0], *s[1:]), d)
            for s, d in zip(zero_shapes, zero_dtypes)
        ]
        out_arrs = sharded(*concat_in, *concat_zeros)
        return [
            {
                nm: np.asarray(out_arrs[i]).reshape(8, *zero_shapes[i])[c]
                for i, nm in enumerate(out_names)
            }
            for c in range(8)
        ]

    _RUNNER[mask_any] = call
    return call


def run(x, mask, Wq, Wkv, Wo, bo, trace=False):
    x = np.asarray(x, np.float32)
    mask = np.asarray(mask, bool)
    Wq = np.asarray(Wq, np.float32)
    Wkv = np.asarray(Wkv, np.float32)
    Wo = np.asarray(Wo, np.float32)
    bo = np.asarray(bo, np.float32)
    mask_any = bool(mask.any())
    maps = _in_maps(x, mask, Wq, Wkv, Wo, bo, mask_any)
    results = _get_runner(mask_any)(maps)
    out = np.empty((B, N, DIM), np.float32)
    for c in range(8):
        g, r = divmod(c, GROUP)
        out[g, r * NQ : (r + 1) * NQ, :] = results[c]["y"]
    return out, results


def kernel(x, mask, Wq, Wkv, Wo, bo):
    out, _ = run(x, mask, Wq, Wkv, Wo, bo)
    return out


# revision 5
# speedup vs baseline: 1.0505x; 1.0023x over previous
"""Distributed multi-head attention kernel for one TRN2 chip (8 NeuronCores).

Problem: y = Attention(x) with b=2, n=2048, dim=1024, heads=16, dim_head=64.

Sharding (data + tensor parallel):
  core c: batch g = c // 4, head-group r = c % 4 (4 heads = 256 inner dims).

Design (v2):
  - Host pre-converts x (transposed to feature-major xT[p, c, t] =
    x[t, 128c+p]) and Wq/Wk/Wv/Wo to bf16, with SCALE folded into Wq, so
    no on-device transposes or conversions are needed.
  - Attention in single-head strips (h, ib): scores psS [128 j, 2*512 i]
    (two j-chunks per PSUM tile pair), exp on ACT -> e bf16 tiles. The
    ACT engine (exp throughput) paces the attention phase; q/k/v
    projections are interleaved into the PE stream one step per exp slot.
  - AV in transposed layout: out [i=128 tok, 65] per i-chunk with a
    ones-column in the v tile giving the softmax denominator in col 64.
    Full 128-partition outputs halve PE cost vs the [65, i] layout, and
    normalization becomes a cheap per-partition DVE scalar op. The four
    i-chunk accumulation groups share one PSUM bank, so the bank is
    zeroed once and all matmuls accumulate with start=False.
  - AV runs in-strip with a 2-pair lag (strips 0-1 lag a full strip while
    the v projections stream in); normalize+transpose-back+staging for
    strip s is deferred into strip s+1 so the PE never stalls on the DVE
    normalization chain at strip boundaries.
  - Transpose-back to [d, tok] via PE with a bf16 identity; two
    gate-scaled DVE copies stage it into the A2A buffer (the wrong-batch
    copy writes zeros).
  - Exchange: four 8-core AllToAll collectives (one per head, fired
    inline as each head's last strip closes) so only the smallest
    possible transfer sits in the tail; receives (plain + SWDGE
    accumulate, summing the two batch halves) are deferred until after
    the last collective so no queue ever blocks a later collective's
    issue.
  - Output projection in two stages: even aT chunks (head pairs 0/1)
    run inside the later collectives' windows; odd chunks in the tail,
    alternating PSUM arenas to avoid write-after-read stalls.
"""

import sys

if "/opt/trn_rl_repo" not in sys.path:
    sys.path.insert(0, "/opt/trn_rl_repo")

from contextlib import ExitStack

import ml_dtypes
import numpy as np

import concourse.bass as bass
from concourse import bacc
import concourse.mybir as mybir
import concourse.tile as tile
from concourse.masks import make_identity

F32 = mybir.dt.float32
F32R = mybir.dt.float32r
BF16 = mybir.dt.bfloat16
EXP = mybir.ActivationFunctionType.Exp

B, N, DIM = 2, 2048, 1024
HEADS, DH = 16, 64
INNER = HEADS * DH            # 1024
SCALE = DH ** -0.5
GROUP = 4                     # tensor-parallel group size (cores per batch)
IC = INNER // GROUP           # 256 inner dims per core (4 heads)
NQ = N // GROUP               # 512 output tokens per core
NEG = -1.0e30

P = 128
TB = 512                      # i-block (moving-dim max)
NT = N // P                   # 16 j-chunks
NJP = NT // 2                 # 8 j-chunk pairs (one psS/exp per pair)
ND = DIM // P                 # 8 contraction chunks
NTB = N // TB                 # 4 token blocks
LAGP = 2                      # AV trails exp by LAGP j-chunk-pairs

_CACHE = {}


def _mm(nc, out, lhsT, rhs, start=True, stop=True, tile_position=None):
    nc.tensor.matmul(
        out, lhsT, rhs, start=start, stop=stop, tile_position=tile_position
    )


def _build(mask_any: bool) -> bass.Bass:
    nc = bacc.Bacc()

    xt_in = nc.declare_dram_parameter("xt_b", [NTB, P, ND, TB], BF16, False)
    wq = nc.declare_dram_parameter("wq_s", [DIM, IC], BF16, False)
    wk = nc.declare_dram_parameter("wk_s", [DIM, IC], BF16, False)
    wv = nc.declare_dram_parameter("wv_s", [DIM, IC], BF16, False)
    wo = nc.declare_dram_parameter("wo", [INNER, DIM], BF16, False)
    bo = nc.declare_dram_parameter("bo", [1, DIM], F32R, False)
    # gate[:, g] = 1.0 iff this core handles batch g (replicated down the
    # partition axis); scales the transpose-back identity so cross-batch
    # A2A chunks carry zeros.
    gate = nc.declare_dram_parameter("gate", [P, 2], F32, False)
    if mask_any:
        mb = nc.declare_dram_parameter("mbias", [P, NT], F32, False)
    y = nc.declare_dram_parameter("y", [NQ, DIM], F32, True)

    with ExitStack() as ctx:
        tc = ctx.enter_context(tile.TileContext(nc))

        const = ctx.enter_context(tc.tile_pool(name="const", bufs=1))
        ident = const.tile([P, P], BF16, tag="ident")
        make_identity(nc, ident[:])
        gate_sb = const.tile([P, 2], F32, tag="gate_sb")
        nc.scalar.dma_start(gate_sb[:], gate.ap())
        ones_f = const.tile([P, P], F32, tag="ones_f")
        nc.vector.memset(ones_f[:], 1.0)
        ones_r = const.tile([P, P], F32R, tag="ones_r")
        nc.vector.tensor_copy(ones_r[:], ones_f[:])
        bo_sb = const.tile([1, DIM], F32R, tag="bo_sb")
        nc.scalar.dma_start(bo_sb[:], bo.ap())
        if mask_any:
            mb_sb = const.tile([P, NT], F32, tag="mb_sb")
            nc.scalar.dma_start(mb_sb[:], mb.ap())

        # ---- persistent SBUF ----
        big = ctx.enter_context(tc.tile_pool(name="big", bufs=1))
        wk_all = big.tile([P, ND, IC], BF16, tag="w", bufs=3, name="wk_all")
        wq_all = big.tile([P, ND, IC], BF16, tag="w", bufs=3, name="wq_all")
        wv_all = big.tile([P, ND, IC], BF16, tag="w", bufs=3, name="wv_all")
        wo_all = big.tile([P, ND, DIM], BF16, tag="woa", name="wo_all")
        wk_sb = [wk_all[:, c, :] for c in range(ND)]
        wq_sb = [wq_all[:, c, :] for c in range(ND)]
        wv_sb = [wv_all[:, c, :] for c in range(ND)]
        wo_sb = [wo_all[:, c, :] for c in range(ND)]
        xTb = [big.tile([P, ND, TB], BF16, tag="xT", bufs=4, name=f"xT{tb}")
               for tb in range(NTB)]

        def xT(c, lo, hi):
            # feature-chunk c, token range [lo, hi) (within one tb block)
            tb, off = divmod(lo, TB)
            return xTb[tb][:, c, off : off + (hi - lo)]
        q2 = [big.tile([P, N], BF16, tag="qk", bufs=4, name=f"q2_{hp}")
              for hp in range(2)]
        k2 = [big.tile([P, N], BF16, tag="qk", bufs=4, name=f"k2_{hp}")
              for hp in range(2)]
        v_all = big.tile([P, 4 * NT * (DH + 1)], BF16, tag="vx",
                         name="v_all")
        va = v_all[:].rearrange("p (h t c) -> p h t c", h=4, t=NT)
        aT = [big.tile([P, NQ], BF16, tag="aT", bufs=8, name=f"aT_{c}")
              for c in range(ND)]
        partial = [big.tile([P, TB], F32, tag="part", bufs=8, name=f"part{i}")
                   for i in range(8)]
        bo_rep = const.tile([P, DIM], F32, tag="bo_rep")

        # DMA issue order: one DMA per weight tensor on the scalar (ACT)
        # HWDGE queue - 4 configs total so neither the ACT sequencer nor the
        # queue's in-flight window ever backs up; x transposes on SP.
        nc.sync.dma_start(xTb[0][:], xt_in.ap()[0])
        nc.scalar.dma_start(wk_all[:], wk.ap().rearrange("(c p) f -> p c f",
                                                         c=ND))
        nc.sync.dma_start(xTb[1][:], xt_in.ap()[1])
        nc.scalar.dma_start(wq_all[:], wq.ap().rearrange("(c p) f -> p c f",
                                                         c=ND))
        nc.sync.dma_start(xTb[2][:], xt_in.ap()[2])
        nc.sync.dma_start(xTb[3][:], xt_in.ap()[3])
        nc.scalar.dma_start(wv_all[:], wv.ap().rearrange("(c p) f -> p c f",
                                                         c=ND))
        nc.scalar.dma_start(wo_all[:], wo.ap().rearrange("(c p) f -> p c f",
                                                         c=ND))

        nc.vector.memset(va[:, :, :, DH], 1.0)

        # A2A buffers (DRAM), one per head: chunk j (64 rows) -> core j;
        # chunk content = this head's [64 d, 512 tok] for quarter j%4,
        # zeros unless j//4 == my batch.
        dram = ctx.enter_context(tc.tile_pool(name="dram", bufs=1, space="DRAM"))
        a2a_in = [dram.tile([8 * DH, NQ], BF16, tag="a2a_in", bufs=4,
                            name=f"a2a_in{h}") for h in range(4)]
        a2a_out = [dram.tile([8 * DH, NQ], BF16, tag="a2a_out", bufs=4,
                             name=f"a2a_out{h}") for h in range(4)]

        ps_s = ctx.enter_context(tc.tile_pool(name="ps_s", bufs=2, space="PSUM"))
        ps_av = ctx.enter_context(tc.tile_pool(name="ps_av", bufs=2, space="PSUM"))
        ps_m = ctx.enter_context(tc.tile_pool(name="ps_m", bufs=2, space="PSUM"))

        sm = ctx.enter_context(tc.tile_pool(name="sm", bufs=1))

        # bias broadcast rows (uses the scores PSUM arena before attention)
        for nb in range(DIM // TB):
            psb = ps_s.tile([P, 2 * TB], F32, tag="s", name="psb")
            _mm(nc, psb[:, 0:TB], ones_r[0:1, :],
                bo_sb[:, nb * TB : (nb + 1) * TB])
            nc.vector.tensor_copy(bo_rep[:, nb * TB : (nb + 1) * TB],
                                  psb[:, 0:TB])

        # ---- projection / wo step closures (popped one per exp slot) ----
        def proj_step(kind, hp, tb):
            def run():
                pj = ps_m.tile([P, TB], F32, tag="m", name="pj")
                wsb = wq_sb if kind == "q" else wk_sb
                dest = q2[hp] if kind == "q" else k2[hp]
                for c in range(ND):
                    _mm(nc, pj[:], wsb[c][:, hp * P : (hp + 1) * P],
                        xT(c, tb * TB, (tb + 1) * TB),
                        start=(c == 0), stop=(c == ND - 1))
                nc.vector.tensor_copy(dest[:, tb * TB : (tb + 1) * TB], pj[:])
            return run

        def proj_half(kind, hp, j):
            # half-range projection: tokens [256j, 256j+256)
            def run():
                pj = ps_m.tile([P, TB], F32, tag="m", name="pjh")
                wsb = wq_sb if kind == "q" else wk_sb
                dest = q2[hp] if kind == "q" else k2[hp]
                lo = j * 256
                for c in range(ND):
                    _mm(nc, pj[:, 0:256], wsb[c][:, hp * P : (hp + 1) * P],
                        xT(c, lo, lo + 256),
                        start=(c == 0), stop=(c == ND - 1))
                nc.vector.tensor_copy(dest[:, lo : lo + 256], pj[:, 0:256])
            return run

        def v_step(t):
            def run():
                pv = ps_m.tile([P, IC], F32, tag="m", name="pv")
                for c in range(ND):
                    _mm(nc, pv[:], xT(c, t * P, (t + 1) * P), wv_sb[c],
                        start=(c == 0), stop=(c == ND - 1))
                nc.vector.tensor_copy(
                    va[:, :, t, 0:DH],
                    pv[:].rearrange("p (h c) -> p h c", h=4))
            return run

        def wo_step(parity, t, nb, alt):
            def run():
                if alt:
                    pw_t = ps_s.tile([P, 2 * TB], F32, tag="s", name="pw")
                    pw = pw_t[:, 0:TB]
                else:
                    pw = ps_m.tile([P, TB], F32, tag="m", name="pw")[:]
                for m in range(4):
                    _mm(nc, pw,
                        aT[2 * m + parity][:, t * P : (t + 1) * P],
                        wo_sb[2 * m + parity][:, nb * TB : (nb + 1) * TB],
                        start=(m == 0), stop=(m == 3))
                if parity == 0:
                    nc.vector.tensor_add(
                        partial[t * 2 + nb][:], pw,
                        bo_rep[:, nb * TB : (nb + 1) * TB])
                else:
                    fo = sm.tile([P, TB], F32, tag="fo", bufs=6, name="fo")
                    nc.vector.tensor_add(fo[:], pw, partial[t * 2 + nb][:])
                    nc.sync.dma_start(
                        y.ap()[t * P : (t + 1) * P, nb * TB : (nb + 1) * TB],
                        fo[:])
            return run

        # stage 1: only the projections that gate the first scores
        proj_step("k", 0, 0)()
        proj_step("k", 0, 1)()
        proj_step("q", 0, 0)()
        proj_step("q", 0, 1)()
        proj_step("k", 0, 2)()
        proj_step("k", 0, 3)()

        # pop queue: exp-slot index -> closure (16 strips x 8 slots = 128).
        # v chunk t is consumed by the (full-strip-lagged) AV at absolute
        # slot 8 + t//2; popping v(t) at slot t-1 stays ahead (same-engine
        # program order guarantees no deadlock even at zero slack).
        popq = {0: [v_step(0), v_step(1)]}
        for t in range(2, NT):
            popq.setdefault(t - 1, []).append(v_step(t))
        popq.setdefault(15, []).append(proj_step("q", 0, 2))
        popq.setdefault(17, []).append(proj_step("q", 0, 3))
        for j in range(8):
            popq.setdefault(18 + 2 * j, []).append(proj_half("k", 1, j))
            popq.setdefault(35 + 2 * j, []).append(proj_half("q", 1, j))

        # ---- attention strips: one head at a time; AV for strip s runs
        # during strip s+1 (one pair per exp slot) so the interleaved
        # v/k/q projection steps never sit behind a waiting AV matmul.
        def do_av(st, jt):
            et = st["es"][jt // 2]
            off = (jt % 2) * TB
            # four accumulation groups share this bank: it was zeroed at
            # strip start and all matmuls accumulate with start=False so
            # their start bits can't clobber each other's partials
            av = st["av"]
            for c in range(4):
                nc.tensor.matmul(
                    av[:, c * (DH + 1) : (c + 1) * (DH + 1)],
                    et[:, off + c * P : off + (c + 1) * P],
                    va[:, st["h"], jt, :],
                    start=False, stop=(jt == NT - 1),
                    skip_group_check=True)

        def norm_dve(st):
            """Normalize (DVE); returns deferred PE-transpose + staging."""
            av, h, ib = st["av"], st["h"], st["ib"]
            rcp = sm.tile([P, 4], F32, tag="rcp", bufs=4, name="rcp")
            with nc.allow_low_precision("softmax denom"):
                nc.vector.reciprocal(rcp[:], av[:, DH :: DH + 1])
            onrm = sm.tile([P, 4 * DH], BF16, tag="onrm", bufs=4, name="onrm")
            for c in range(4):
                nc.vector.tensor_scalar_mul(
                    onrm[:, c * DH : (c + 1) * DH],
                    av[:, c * (DH + 1) : c * (DH + 1) + DH],
                    rcp[:, c : c + 1])

            def part2():
                oT = ps_m.tile([DH, TB], BF16, tag="m", name="oT")
                for c in range(4):
                    nc.tensor.transpose(oT[:, c * P : (c + 1) * P],
                                        onrm[:, c * DH : (c + 1) * DH],
                                        ident[:])
                for g in range(2):
                    stg = sm.tile([DH, TB], BF16, tag="stg", bufs=6,
                                  name="stg")
                    nc.vector.tensor_scalar_mul(stg[:], oT[:],
                                                gate_sb[0:DH, g : g + 1])
                    row = (4 * g + ib) * DH
                    nc.sync.dma_start(a2a_in[h][row : row + DH, :], stg[:])
                if ib == NTB - 1:
                    fire_a2a(h)
            return part2

        def fire_a2a(h):
            nc.gpsimd.collective_compute(
                "AllToAll",
                mybir.AluOpType.bypass,
                replica_groups=[[0, 1, 2, 3, 4, 5, 6, 7]],
                ins=[a2a_in[h].opt()],
                outs=[a2a_out[h].opt()],
            )

        def recv_a2a(h, use_dve=False):
            # receive: aT[2m+hp] rows [e*64, e*64+64) = chunk m + chunk 4+m
            # (exactly one is nonzero). SWDGE accumulate adds the second
            # half; the last head instead loads both halves and adds on DVE
            # (idle in the tail) to skip the serial SWDGE generation chain.
            # Emitted only after the LAST collective so the SP/Pool queues
            # never block a later collective's issue while waiting.
            hp, e = divmod(h, 2)
            for m in range(GROUP):
                dst = aT[2 * m + hp][e * DH : (e + 1) * DH, :]
                nc.sync.dma_start(dst, a2a_out[h][m * DH : (m + 1) * DH, :])
                if use_dve:
                    tmp = sm.tile([DH, NQ], BF16, tag="rtmp", bufs=4,
                                  name="rtmp")
                    nc.sync.dma_start(
                        tmp[:], a2a_out[h][(4 + m) * DH : (5 + m) * DH, :])
                    nc.vector.tensor_add(dst, dst, tmp[:])
                else:
                    nc.gpsimd.dma_start(
                        dst, a2a_out[h][(4 + m) * DH : (5 + m) * DH, :],
                        accum_op=mybir.AluOpType.add)

        # Strips 0-1 lag their AV by a full strip (the v projections are
        # still streaming in); strips 2+ run their own AV in-strip with a
        # 2-pair lag, so each head's A2A fires right at its last strip's
        # end and the four collectives never queue on each other.
        slot = 0
        prev = None
        pend2 = []
        for h in range(4):
            hp, e = divmod(h, 2)
            ksb = k2[hp][e * DH : (e + 1) * DH, :]
            qsb = q2[hp][e * DH : (e + 1) * DH, :]
            tp = (e * DH, 0)
            for ib in range(NTB):
                isl = slice(ib * TB, (ib + 1) * TB)
                s = 4 * h + ib
                in_strip = s >= 2
                cur = {"h": h, "ib": ib, "es": [None] * NJP, "av": None}
                cur["av"] = ps_av.tile([P, 4 * (DH + 1)], F32, tag="av",
                                       name="av")
                nc.vector.memset(cur["av"][:], 0.0)
                for jp in range(NJP):
                    psS = ps_s.tile([P, 2 * TB], F32, tag="s", name="psS")
                    for sub in range(2):
                        jt = 2 * jp + sub
                        jsl = slice(jt * P, (jt + 1) * P)
                        _mm(nc, psS[:, sub * TB : (sub + 1) * TB],
                            ksb[:, jsl], qsb[:, isl], tile_position=tp)
                        if mask_any:
                            nc.vector.tensor_scalar_add(
                                psS[:, sub * TB : (sub + 1) * TB],
                                psS[:, sub * TB : (sub + 1) * TB],
                                mb_sb[:, jt : jt + 1])
                    e_t = sm.tile([P, 2 * TB], BF16, tag="e", bufs=20,
                                  name="e")
                    nc.scalar.activation(e_t[:], psS[:], EXP)
                    cur["es"][jp] = e_t
                    for fn in popq.pop(slot, ()):
                        fn()
                    if jp == 2 and pend2:
                        for fn in pend2:
                            fn()
                        pend2 = []
                    if prev is not None:
                        do_av(prev, 2 * jp)
                        do_av(prev, 2 * jp + 1)
                    if in_strip and jp >= 3:
                        do_av(cur, 2 * (jp - 3))
                        do_av(cur, 2 * (jp - 3) + 1)
                    slot += 1
                if in_strip:
                    if prev is not None:
                        pend2.append(norm_dve(prev))
                        prev = None
                    for jp in range(NJP - 3, NJP):
                        do_av(cur, 2 * jp)
                        do_av(cur, 2 * jp + 1)
                    pc = norm_dve(cur)
                    if ib == NTB - 1:
                        pc()
                    else:
                        pend2.append(pc)
                else:
                    if prev is not None:
                        pend2.append(norm_dve(prev))
                    prev = cur

        for h in range(4):
            recv_a2a(h)

        # wo even stage: runs inside the final collective's window
        wi = 0
        for t in range(NQ // P):
            for nb in range(DIM // TB):
                wo_step(0, t, nb, alt=(wi % 2 == 1))()
                wi += 1
        # drain any unfired pop steps (none expected)
        for i in sorted(popq):
            for fn in popq.pop(i):
                fn()

        # wo odd stage (alternate PSUM arenas to avoid WAR stalls)
        wi = 0
        for t in range(NQ // P):
            for nb in range(DIM // TB):
                wo_step(1, t, nb, alt=(wi % 2 == 1))()
                wi += 1

    nc.compile()
    return nc


def _get_nc(mask_any: bool) -> bass.Bass:
    if mask_any not in _CACHE:
        _CACHE[mask_any] = _build(mask_any)
    return _CACHE[mask_any]


def _in_maps(x, mask, Wq, Wkv, Wo, bo, mask_any):
    bf = ml_dtypes.bfloat16
    bo2 = np.ascontiguousarray(np.asarray(bo, np.float32).reshape(1, DIM))
    wo_bf = np.ascontiguousarray(np.asarray(Wo, np.float32).astype(bf))
    maps = []
    for c in range(8):
        g, r = divmod(c, GROUP)
        csl = slice(r * IC, (r + 1) * IC)
        m = {
            "xt_b": np.ascontiguousarray(
                x[g].reshape(NTB, TB, ND, P).transpose(0, 3, 2, 1).astype(bf)),
            "wq_s": np.ascontiguousarray(
                (Wq[:, csl] * np.float32(SCALE)).astype(bf)),
            "wk_s": np.ascontiguousarray(Wkv[:, csl].astype(bf)),
            "wv_s": np.ascontiguousarray(
                Wkv[:, INNER + r * IC : INNER + (r + 1) * IC].astype(bf)),
            "wo": wo_bf,
            "bo": bo2,
            "gate": np.ascontiguousarray(
                np.tile(np.array([[1.0 - g, float(g)]], np.float32), (P, 1))),
        }
        if mask_any:
            mvec = np.where(mask[g], np.float32(NEG), np.float32(0.0)).astype(
                np.float32)
            m["mbias"] = np.ascontiguousarray(mvec.reshape(NT, P).T)
        maps.append(m)
    return maps


_RUNNER = {}


def _get_runner(mask_any: bool):
    """Cached jax-jitted SPMD executor for the Bass module."""
    if mask_any in _RUNNER:
        return _RUNNER[mask_any]
    import jax
    from jax.sharding import Mesh, PartitionSpec
    from jax.experimental.shard_map import shard_map
    from concourse import bass2jax

    nc = _get_nc(mask_any)
    bass2jax.install_neuronx_cc_hook()

    partition_name = (
        nc.partition_id_tensor.name if nc.partition_id_tensor else None
    )
    in_names, out_names, out_avals = [], [], []
    for alloc in nc.m.functions[0].allocations:
        if not isinstance(alloc, mybir.MemoryLocationSet):
            continue
        name = alloc.memorylocations[0].name
        if alloc.kind == "ExternalInput":
            if name != partition_name:
                in_names.append(name)
        elif alloc.kind == "ExternalOutput":
            shape = tuple(alloc.tensor_shape)
            dtype = mybir.dt.np(alloc.dtype)
            out_names.append(name)
            out_avals.append(jax.core.ShapedArray(shape, dtype))
    n_params = len(in_names)
    n_outs = len(out_avals)
    all_names = list(in_names) + list(out_names)
    if partition_name is not None:
        all_names.append(partition_name)
    donate = tuple(range(n_params, n_params + n_outs))

    def _body(*args):
        operands = list(args)
        if partition_name is not None:
            operands.append(bass2jax.partition_id_tensor())
        outs = bass2jax._bass_exec_p.bind(
            *operands,
            out_avals=tuple(out_avals),
            in_names=tuple(all_names),
            out_names=tuple(out_names),
            lowering_input_output_aliases=(),
            sim_require_finite=True,
            sim_require_nnan=True,
            nc=nc,
        )
        return tuple(outs)

    devices = jax.devices()[:8]
    mesh = Mesh(np.asarray(devices), ("core",))
    in_specs = (PartitionSpec("core"),) * (n_params + n_outs)
    out_specs = (PartitionSpec("core"),) * n_outs
    sharded = jax.jit(
        shard_map(_body, mesh=mesh, in_specs=in_specs, out_specs=out_specs,
                  check_rep=False),
        donate_argnums=donate,
        keep_unused=True,
    )
    zero_shapes = [tuple(a.shape) for a in out_avals]
    zero_dtypes = [a.dtype for a in out_avals]

    def call(maps):
        concat_in = [
            np.concatenate([np.asarray(maps[c][nm]) for c in range(8)], axis=0)
            for nm in in_names
        ]
        concat_zeros = [
            np.zeros((8 * s[0], *s[1:]), d)
            for s, d in zip(zero_shapes, zero_dtypes)
        ]
        out_arrs = sharded(*concat_in, *concat_zeros)
        return [
            {
                nm: np.asarray(out_arrs[i]).reshape(8, *zero_shapes[i])[c]
                for i, nm in enumerate(out_names)
            }
            for c in range(8)
        ]

    _RUNNER[mask_any] = call
    return call


def run(x, mask, Wq, Wkv, Wo, bo, trace=False):
    x = np.asarray(x, np.float32)
    mask = np.asarray(mask, bool)
    Wq = np.asarray(Wq, np.float32)
    Wkv = np.asarray(Wkv, np.float32)
    Wo = np.asarray(Wo, np.float32)
    bo = np.asarray(bo, np.float32)
    mask_any = bool(mask.any())
    maps = _in_maps(x, mask, Wq, Wkv, Wo, bo, mask_any)
    results = _get_runner(mask_any)(maps)
    out = np.empty((B, N, DIM), np.float32)
    for c in range(8):
        g, r = divmod(c, GROUP)
        out[g, r * NQ : (r + 1) * NQ, :] = results[c]["y"]
    return out, results


def kernel(x, mask, Wq, Wkv, Wo, bo):
    out, _ = run(x, mask, Wq, Wkv, Wo, bo)
    return out


# revision 6
# speedup vs baseline: 1.0607x; 1.0097x over previous
"""Distributed multi-head attention kernel for one TRN2 chip (8 NeuronCores).

Problem: y = Attention(x) with b=2, n=2048, dim=1024, heads=16, dim_head=64.

Sharding (data + tensor parallel):
  core c: batch g = c // 4, head-group r = c % 4 (4 heads = 256 inner dims).

Design (v2):
  - Host pre-converts x (transposed to feature-major xT[p, c, t] =
    x[t, 128c+p]) and Wq/Wk/Wv/Wo to bf16, with SCALE folded into Wq, so
    no on-device transposes or conversions are needed.
  - Attention in single-head strips (h, ib): scores psS [128 j, 2*512 i]
    (two j-chunks per PSUM tile pair), exp on ACT -> e bf16 tiles. The
    ACT engine (exp throughput) paces the attention phase; q/k/v
    projections are interleaved into the PE stream one step per exp slot.
  - AV in transposed layout: out [i=128 tok, 65] per i-chunk with a
    ones-column in the v tile giving the softmax denominator in col 64.
    Full 128-partition outputs halve PE cost vs the [65, i] layout, and
    normalization becomes a cheap per-partition DVE scalar op. The four
    i-chunk accumulation groups share one PSUM bank, so the bank is
    zeroed once and all matmuls accumulate with start=False.
  - AV runs in-strip with a 2-pair lag (strips 0-1 lag a full strip while
    the v projections stream in); normalize+transpose-back+staging for
    strip s is deferred into strip s+1 so the PE never stalls on the DVE
    normalization chain at strip boundaries.
  - Transpose-back to [d, tok] via PE with a bf16 identity; two
    gate-scaled DVE copies stage it into the A2A buffer (the wrong-batch
    copy writes zeros).
  - Exchange: four 8-core AllToAll collectives (one per head, fired
    inline as each head's last strip closes) so only the smallest
    possible transfer sits in the tail; receives (plain + SWDGE
    accumulate, summing the two batch halves) are deferred until after
    the last collective so no queue ever blocks a later collective's
    issue.
  - Output projection in two stages: even aT chunks (head pairs 0/1)
    run inside the later collectives' windows; odd chunks in the tail,
    alternating PSUM arenas to avoid write-after-read stalls.
"""

import sys

if "/opt/trn_rl_repo" not in sys.path:
    sys.path.insert(0, "/opt/trn_rl_repo")

from contextlib import ExitStack

import ml_dtypes
import numpy as np

import concourse.bass as bass
from concourse import bacc
import concourse.mybir as mybir
import concourse.tile as tile
from concourse.masks import make_identity

F32 = mybir.dt.float32
F32R = mybir.dt.float32r
BF16 = mybir.dt.bfloat16
EXP = mybir.ActivationFunctionType.Exp

B, N, DIM = 2, 2048, 1024
HEADS, DH = 16, 64
INNER = HEADS * DH            # 1024
SCALE = DH ** -0.5
GROUP = 4                     # tensor-parallel group size (cores per batch)
IC = INNER // GROUP           # 256 inner dims per core (4 heads)
NQ = N // GROUP               # 512 output tokens per core
NEG = -1.0e30

P = 128
TB = 512                      # i-block (moving-dim max)
NT = N // P                   # 16 j-chunks
NJP = NT // 2                 # 8 j-chunk pairs (one psS/exp per pair)
ND = DIM // P                 # 8 contraction chunks
NTB = N // TB                 # 4 token blocks
LAGP = 2                      # AV trails exp by LAGP j-chunk-pairs

_CACHE = {}


def _mm(nc, out, lhsT, rhs, start=True, stop=True, tile_position=None):
    nc.tensor.matmul(
        out, lhsT, rhs, start=start, stop=stop, tile_position=tile_position
    )


def _build(mask_any: bool) -> bass.Bass:
    nc = bacc.Bacc()

    xt_in = nc.declare_dram_parameter("xt_b", [NTB, P, ND, TB], BF16, False)
    wq = nc.declare_dram_parameter("wq_s", [DIM, IC], BF16, False)
    wk = nc.declare_dram_parameter("wk_s", [DIM, IC], BF16, False)
    wv = nc.declare_dram_parameter("wv_s", [DIM, IC], BF16, False)
    wo = nc.declare_dram_parameter("wo", [INNER, DIM], BF16, False)
    bo = nc.declare_dram_parameter("bo", [1, DIM], F32R, False)
    # gate[:, g] = 1.0 iff this core handles batch g (replicated down the
    # partition axis); scales the transpose-back identity so cross-batch
    # A2A chunks carry zeros.
    gate = nc.declare_dram_parameter("gate", [P, 2], F32, False)
    if mask_any:
        mb = nc.declare_dram_parameter("mbias", [P, NT], F32, False)
    y = nc.declare_dram_parameter("y", [NQ, DIM], F32, True)

    with ExitStack() as ctx:
        tc = ctx.enter_context(tile.TileContext(nc))

        const = ctx.enter_context(tc.tile_pool(name="const", bufs=1))
        ident = const.tile([P, P], BF16, tag="ident")
        make_identity(nc, ident[:])
        gate_sb = const.tile([P, 2], F32, tag="gate_sb")
        nc.scalar.dma_start(gate_sb[:], gate.ap())
        ones_f = const.tile([P, P], F32, tag="ones_f")
        nc.vector.memset(ones_f[:], 1.0)
        ones_r = const.tile([P, P], F32R, tag="ones_r")
        nc.vector.tensor_copy(ones_r[:], ones_f[:])
        bo_sb = const.tile([1, DIM], F32R, tag="bo_sb")
        nc.scalar.dma_start(bo_sb[:], bo.ap())
        if mask_any:
            mb_sb = const.tile([P, NT], F32, tag="mb_sb")
            nc.scalar.dma_start(mb_sb[:], mb.ap())

        # ---- persistent SBUF ----
        big = ctx.enter_context(tc.tile_pool(name="big", bufs=1))
        wk_all = big.tile([P, ND, IC], BF16, tag="w", bufs=3, name="wk_all")
        wq_all = big.tile([P, ND, IC], BF16, tag="w", bufs=3, name="wq_all")
        wv_all = big.tile([P, ND, IC], BF16, tag="w", bufs=3, name="wv_all")
        wo_all = big.tile([P, ND, DIM], BF16, tag="woa", name="wo_all")
        wk_sb = [wk_all[:, c, :] for c in range(ND)]
        wq_sb = [wq_all[:, c, :] for c in range(ND)]
        wv_sb = [wv_all[:, c, :] for c in range(ND)]
        wo_sb = [wo_all[:, c, :] for c in range(ND)]
        xTb = [big.tile([P, ND, TB], BF16, tag="xT", bufs=4, name=f"xT{tb}")
               for tb in range(NTB)]

        def xT(c, lo, hi):
            # feature-chunk c, token range [lo, hi) (within one tb block)
            tb, off = divmod(lo, TB)
            return xTb[tb][:, c, off : off + (hi - lo)]
        q2 = [big.tile([P, N], BF16, tag="qk", bufs=4, name=f"q2_{hp}")
              for hp in range(2)]
        k2 = [big.tile([P, N], BF16, tag="qk", bufs=4, name=f"k2_{hp}")
              for hp in range(2)]
        v_all = big.tile([P, 4 * NT * (DH + 1)], BF16, tag="vx",
                         name="v_all")
        va = v_all[:].rearrange("p (h t c) -> p h t c", h=4, t=NT)
        aT = [big.tile([P, NQ], BF16, tag="aT", bufs=8, name=f"aT_{c}")
              for c in range(ND)]
        partial = [big.tile([P, TB], F32, tag="part", bufs=8, name=f"part{i}")
                   for i in range(8)]
        bo_rep = const.tile([P, DIM], F32, tag="bo_rep")

        # DMA issue order: one DMA per weight tensor on the scalar (ACT)
        # HWDGE queue - 4 configs total so neither the ACT sequencer nor the
        # queue's in-flight window ever backs up; x transposes on SP.
        nc.sync.dma_start(xTb[0][:], xt_in.ap()[0])
        nc.scalar.dma_start(wk_all[:], wk.ap().rearrange("(c p) f -> p c f",
                                                         c=ND))
        nc.sync.dma_start(xTb[1][:], xt_in.ap()[1])
        nc.scalar.dma_start(wq_all[:], wq.ap().rearrange("(c p) f -> p c f",
                                                         c=ND))
        nc.sync.dma_start(xTb[2][:], xt_in.ap()[2])
        nc.sync.dma_start(xTb[3][:], xt_in.ap()[3])
        nc.scalar.dma_start(wv_all[:], wv.ap().rearrange("(c p) f -> p c f",
                                                         c=ND))
        nc.scalar.dma_start(wo_all[:], wo.ap().rearrange("(c p) f -> p c f",
                                                         c=ND))

        nc.vector.memset(va[:, :, :, DH], 1.0)

        # A2A buffers (DRAM), one per head: chunk j (64 rows) -> core j;
        # chunk content = this head's [64 d, 512 tok] for quarter j%4,
        # zeros unless j//4 == my batch.
        dram = ctx.enter_context(tc.tile_pool(name="dram", bufs=1, space="DRAM"))
        a2a_in = [dram.tile([8 * DH, NQ], BF16, tag="a2a_in", bufs=4,
                            name=f"a2a_in{h}") for h in range(4)]
        a2a_out = [dram.tile([8 * DH, NQ], BF16, tag="a2a_out", bufs=4,
                             name=f"a2a_out{h}") for h in range(4)]

        ps_s = ctx.enter_context(tc.tile_pool(name="ps_s", bufs=2, space="PSUM"))
        ps_av = ctx.enter_context(tc.tile_pool(name="ps_av", bufs=2, space="PSUM"))
        ps_m = ctx.enter_context(tc.tile_pool(name="ps_m", bufs=2, space="PSUM"))

        sm = ctx.enter_context(tc.tile_pool(name="sm", bufs=1))

        # bias broadcast rows (uses the scores PSUM arena before attention)
        for nb in range(DIM // TB):
            psb = ps_s.tile([P, 2 * TB], F32, tag="s", name="psb")
            _mm(nc, psb[:, 0:TB], ones_r[0:1, :],
                bo_sb[:, nb * TB : (nb + 1) * TB])
            nc.vector.tensor_copy(bo_rep[:, nb * TB : (nb + 1) * TB],
                                  psb[:, 0:TB])

        # ---- projection / wo step closures (popped one per exp slot) ----
        def proj_step(kind, hp, tb):
            def run():
                pj = ps_m.tile([P, TB], F32, tag="m", name="pj")
                wsb = wq_sb if kind == "q" else wk_sb
                dest = q2[hp] if kind == "q" else k2[hp]
                for c in range(ND):
                    _mm(nc, pj[:], wsb[c][:, hp * P : (hp + 1) * P],
                        xT(c, tb * TB, (tb + 1) * TB),
                        start=(c == 0), stop=(c == ND - 1))
                nc.vector.tensor_copy(dest[:, tb * TB : (tb + 1) * TB], pj[:])
            return run

        def proj_half(kind, hp, j):
            # half-range projection: tokens [256j, 256j+256)
            def run():
                pj = ps_m.tile([P, TB], F32, tag="m", name="pjh")
                wsb = wq_sb if kind == "q" else wk_sb
                dest = q2[hp] if kind == "q" else k2[hp]
                lo = j * 256
                for c in range(ND):
                    _mm(nc, pj[:, 0:256], wsb[c][:, hp * P : (hp + 1) * P],
                        xT(c, lo, lo + 256),
                        start=(c == 0), stop=(c == ND - 1))
                nc.vector.tensor_copy(dest[:, lo : lo + 256], pj[:, 0:256])
            return run

        def v_step(t):
            def run():
                pv = ps_m.tile([P, IC], F32, tag="m", name="pv")
                for c in range(ND):
                    _mm(nc, pv[:], xT(c, t * P, (t + 1) * P), wv_sb[c],
                        start=(c == 0), stop=(c == ND - 1))
                nc.vector.tensor_copy(
                    va[:, :, t, 0:DH],
                    pv[:].rearrange("p (h c) -> p h c", h=4))
            return run

        def wo_step(parity, t, nb, alt):
            def run():
                if alt:
                    pw_t = ps_s.tile([P, 2 * TB], F32, tag="s", name="pw")
                    pw = pw_t[:, 0:TB]
                else:
                    pw = ps_m.tile([P, TB], F32, tag="m", name="pw")[:]
                for m in range(4):
                    _mm(nc, pw,
                        aT[2 * m + parity][:, t * P : (t + 1) * P],
                        wo_sb[2 * m + parity][:, nb * TB : (nb + 1) * TB],
                        start=(m == 0), stop=(m == 3))
                if parity == 0:
                    nc.vector.tensor_add(
                        partial[t * 2 + nb][:], pw,
                        bo_rep[:, nb * TB : (nb + 1) * TB])
                else:
                    fo = sm.tile([P, TB], F32, tag="fo", bufs=6, name="fo")
                    nc.vector.tensor_add(fo[:], pw, partial[t * 2 + nb][:])
                    nc.sync.dma_start(
                        y.ap()[t * P : (t + 1) * P, nb * TB : (nb + 1) * TB],
                        fo[:])
            return run

        # stage 1: only the projections that gate the first scores
        proj_step("k", 0, 0)()
        proj_step("k", 0, 1)()
        proj_step("q", 0, 0)()
        proj_step("q", 0, 1)()
        proj_step("k", 0, 2)()
        proj_step("k", 0, 3)()

        # pop queue: exp-slot index -> closure (16 strips x 8 slots = 128).
        # v chunk t is consumed by the (full-strip-lagged) AV at absolute
        # slot 8 + t//2; popping v(t) at slot t-1 stays ahead (same-engine
        # program order guarantees no deadlock even at zero slack).
        popq = {0: [v_step(0), v_step(1)]}
        for t in range(2, NT):
            popq.setdefault(t - 1, []).append(v_step(t))
        popq.setdefault(15, []).append(proj_step("q", 0, 2))
        popq.setdefault(17, []).append(proj_step("q", 0, 3))
        for j in range(8):
            popq.setdefault(32 + 2 * j, []).append(proj_half("k", 1, j))
            popq.setdefault(48 + 2 * j, []).append(proj_half("q", 1, j))

        # ---- attention strips: one head at a time; AV for strip s runs
        # during strip s+1 (one pair per exp slot) so the interleaved
        # v/k/q projection steps never sit behind a waiting AV matmul.
        def do_av(st, jt):
            et = st["es"][jt // 2]
            off = (jt % 2) * TB
            # four accumulation groups share this bank: it was zeroed at
            # strip start and all matmuls accumulate with start=False so
            # their start bits can't clobber each other's partials
            av = st["av"]
            for c in range(4):
                nc.tensor.matmul(
                    av[:, c * (DH + 1) : (c + 1) * (DH + 1)],
                    et[:, off + c * P : off + (c + 1) * P],
                    va[:, st["h"], jt, :],
                    start=False, stop=(jt == NT - 1),
                    skip_group_check=True)

        def norm_dve(st):
            """Normalize (DVE); returns deferred PE-transpose + staging."""
            av, h, ib = st["av"], st["h"], st["ib"]
            rcp = sm.tile([P, 4], F32, tag="rcp", bufs=4, name="rcp")
            with nc.allow_low_precision("softmax denom"):
                nc.vector.reciprocal(rcp[:], av[:, DH :: DH + 1])
            onrm = sm.tile([P, 4 * DH], BF16, tag="onrm", bufs=4, name="onrm")
            for c in range(4):
                nc.vector.tensor_scalar_mul(
                    onrm[:, c * DH : (c + 1) * DH],
                    av[:, c * (DH + 1) : c * (DH + 1) + DH],
                    rcp[:, c : c + 1])

            def part2():
                oT = ps_m.tile([DH, TB], BF16, tag="m", name="oT")
                for c in range(4):
                    nc.tensor.transpose(oT[:, c * P : (c + 1) * P],
                                        onrm[:, c * DH : (c + 1) * DH],
                                        ident[:])
                for g in range(2):
                    stg = sm.tile([DH, TB], BF16, tag="stg", bufs=6,
                                  name="stg")
                    nc.vector.tensor_scalar_mul(stg[:], oT[:],
                                                gate_sb[0:DH, g : g + 1])
                    row = (4 * g + ib) * DH
                    nc.sync.dma_start(a2a_in[h][row : row + DH, :], stg[:])
                if ib == NTB - 1:
                    fire_a2a(h)
            return part2

        def fire_a2a(h):
            nc.gpsimd.collective_compute(
                "AllToAll",
                mybir.AluOpType.bypass,
                replica_groups=[[0, 1, 2, 3, 4, 5, 6, 7]],
                ins=[a2a_in[h].opt()],
                outs=[a2a_out[h].opt()],
            )

        def recv_a2a(h, use_dve=False):
            # receive: aT[2m+hp] rows [e*64, e*64+64) = chunk m + chunk 4+m
            # (exactly one is nonzero). SWDGE accumulate adds the second
            # half; the last head instead loads both halves and adds on DVE
            # (idle in the tail) to skip the serial SWDGE generation chain.
            # Emitted only after the LAST collective so the SP/Pool queues
            # never block a later collective's issue while waiting.
            hp, e = divmod(h, 2)
            for m in range(GROUP):
                dst = aT[2 * m + hp][e * DH : (e + 1) * DH, :]
                nc.sync.dma_start(dst, a2a_out[h][m * DH : (m + 1) * DH, :])
                if use_dve:
                    tmp = sm.tile([DH, NQ], BF16, tag="rtmp", bufs=4,
                                  name="rtmp")
                    nc.sync.dma_start(
                        tmp[:], a2a_out[h][(4 + m) * DH : (5 + m) * DH, :])
                    nc.vector.tensor_add(dst, dst, tmp[:])
                else:
                    nc.gpsimd.dma_start(
                        dst, a2a_out[h][(4 + m) * DH : (5 + m) * DH, :],
                        accum_op=mybir.AluOpType.add)

        # Strips 0-1 lag their AV by a full strip (the v projections are
        # still streaming in); strips 2+ run their own AV in-strip with a
        # 2-pair lag, so each head's A2A fires right at its last strip's
        # end and the four collectives never queue on each other.
        slot = 0
        prev = None
        pend2 = []
        for h in range(4):
            hp, e = divmod(h, 2)
            ksb = k2[hp][e * DH : (e + 1) * DH, :]
            qsb = q2[hp][e * DH : (e + 1) * DH, :]
            tp = (e * DH, 0)
            for ib in range(NTB):
                isl = slice(ib * TB, (ib + 1) * TB)
                s = 4 * h + ib
                in_strip = s >= 2
                cur = {"h": h, "ib": ib, "es": [None] * NJP, "av": None}
                cur["av"] = ps_av.tile([P, 4 * (DH + 1)], F32, tag="av",
                                       name="av")
                nc.vector.memset(cur["av"][:], 0.0)
                for jp in range(NJP):
                    psS = ps_s.tile([P, 2 * TB], F32, tag="s", name="psS")
                    for sub in range(2):
                        jt = 2 * jp + sub
                        jsl = slice(jt * P, (jt + 1) * P)
                        _mm(nc, psS[:, sub * TB : (sub + 1) * TB],
                            ksb[:, jsl], qsb[:, isl], tile_position=tp)
                        if mask_any:
                            nc.vector.tensor_scalar_add(
                                psS[:, sub * TB : (sub + 1) * TB],
                                psS[:, sub * TB : (sub + 1) * TB],
                                mb_sb[:, jt : jt + 1])
                    e_t = sm.tile([P, 2 * TB], BF16, tag="e", bufs=20,
                                  name="e")
                    nc.scalar.activation(e_t[:], psS[:], EXP)
                    cur["es"][jp] = e_t
                    for fn in popq.pop(slot, ()):
                        fn()
                    if jp == 2 and pend2:
                        for fn in pend2:
                            fn()
                        pend2 = []
                    if prev is not None:
                        do_av(prev, 2 * jp)
                        do_av(prev, 2 * jp + 1)
                    if in_strip and jp >= 3:
                        do_av(cur, 2 * (jp - 3))
                        do_av(cur, 2 * (jp - 3) + 1)
                    slot += 1
                if in_strip:
                    if prev is not None:
                        pend2.append(norm_dve(prev))
                        prev = None
                    for jp in range(NJP - 3, NJP):
                        do_av(cur, 2 * jp)
                        do_av(cur, 2 * jp + 1)
                    pc = norm_dve(cur)
                    if ib == NTB - 1:
                        pc()
                    else:
                        pend2.append(pc)
                else:
                    if prev is not None:
                        pend2.append(norm_dve(prev))
                    prev = cur

        for h in range(4):
            recv_a2a(h)

        # wo even stage: runs inside the final collective's window
        wi = 0
        for t in range(NQ // P):
            for nb in range(DIM // TB):
                wo_step(0, t, nb, alt=(wi % 2 == 1))()
                wi += 1
        # drain any unfired pop steps (none expected)
        for i in sorted(popq):
            for fn in popq.pop(i):
                fn()

        # wo odd stage (alternate PSUM arenas to avoid WAR stalls)
        wi = 0
        for t in range(NQ // P):
            for nb in range(DIM // TB):
                wo_step(1, t, nb, alt=(wi % 2 == 1))()
                wi += 1

    nc.compile()
    return nc


def _get_nc(mask_any: bool) -> bass.Bass:
    if mask_any not in _CACHE:
        _CACHE[mask_any] = _build(mask_any)
    return _CACHE[mask_any]


def _in_maps(x, mask, Wq, Wkv, Wo, bo, mask_any):
    bf = ml_dtypes.bfloat16
    bo2 = np.ascontiguousarray(np.asarray(bo, np.float32).reshape(1, DIM))
    wo_bf = np.ascontiguousarray(np.asarray(Wo, np.float32).astype(bf))
    maps = []
    for c in range(8):
        g, r = divmod(c, GROUP)
        csl = slice(r * IC, (r + 1) * IC)
        m = {
            "xt_b": np.ascontiguousarray(
                x[g].reshape(NTB, TB, ND, P).transpose(0, 3, 2, 1).astype(bf)),
            "wq_s": np.ascontiguousarray(
                (Wq[:, csl] * np.float32(SCALE)).astype(bf)),
            "wk_s": np.ascontiguousarray(Wkv[:, csl].astype(bf)),
            "wv_s": np.ascontiguousarray(
                Wkv[:, INNER + r * IC : INNER + (r + 1) * IC].astype(bf)),
            "wo": wo_bf,
            "bo": bo2,
            "gate": np.ascontiguousarray(
                np.tile(np.array([[1.0 - g, float(g)]], np.float32), (P, 1))),
        }
        if mask_any:
            mvec = np.where(mask[g], np.float32(NEG), np.float32(0.0)).astype(
                np.float32)
            m["mbias"] = np.ascontiguousarray(mvec.reshape(NT, P).T)
        maps.append(m)
    return maps


_RUNNER = {}


def _get_runner(mask_any: bool):
    """Cached jax-jitted SPMD executor for the Bass module."""
    if mask_any in _RUNNER:
        return _RUNNER[mask_any]
    import jax
    from jax.sharding import Mesh, PartitionSpec
    from jax.experimental.shard_map import shard_map
    from concourse import bass2jax

    nc = _get_nc(mask_any)
    bass2jax.install_neuronx_cc_hook()

    partition_name = (
        nc.partition_id_tensor.name if nc.partition_id_tensor else None
    )
    in_names, out_names, out_avals = [], [], []
    for alloc in nc.m.functions[0].allocations:
        if not isinstance(alloc, mybir.MemoryLocationSet):
            continue
        name = alloc.memorylocations[0].name
        if alloc.kind == "ExternalInput":
            if name != partition_name:
                in_names.append(name)
        elif alloc.kind == "ExternalOutput":
            shape = tuple(alloc.tensor_shape)
            dtype = mybir.dt.np(alloc.dtype)
            out_names.append(name)
            out_avals.append(jax.core.ShapedArray(shape, dtype))
    n_params = len(in_names)
    n_outs = len(out_avals)
    all_names = list(in_names) + list(out_names)
    if partition_name is not None:
        all_names.append(partition_name)
    donate = tuple(range(n_params, n_params + n_outs))

    def _body(*args):
        operands = list(args)
        if partition_name is not None:
            operands.append(bass2jax.partition_id_tensor())
        outs = bass2jax._bass_exec_p.bind(
            *operands,
            out_avals=tuple(out_avals),
            in_names=tuple(all_names),
            out_names=tuple(out_names),
            lowering_input_output_aliases=(),
            sim_require_finite=True,
            sim_require_nnan=True,
            nc=nc,
        )
        return tuple(outs)

    devices = jax.devices()[:8]
    mesh = Mesh(np.asarray(devices), ("core",))
    in_specs = (PartitionSpec("core"),) * (n_params + n_outs)
    out_specs = (PartitionSpec("core"),) * n_outs
    sharded = jax.jit(
        shard_map(_body, mesh=mesh, in_specs=in_specs, out_specs=out_specs,
                  check_rep=False),
        donate_argnums=donate,
        keep_unused=True,
    )
    zero_shapes = [tuple(a.shape) for a in out_avals]
    zero_dtypes = [a.dtype for a in out_avals]

    def call(maps):
        concat_in = [
            np.concatenate([np.asarray(maps[c][nm]) for c in range(8)], axis=0)
            for nm in in_names
        ]
        concat_zeros = [
            np.zeros((8 * s[0], *s[1:]), d)
            for s, d in zip(zero_shapes, zero_dtypes)
        ]
        out_arrs = sharded(*concat_in, *concat_zeros)
        return [
            {
                nm: np.asarray(out_arrs[i]).reshape(8, *zero_shapes[i])[c]
                for i, nm in enumerate(out_names)
            }
            for c in range(8)
        ]

    _RUNNER[mask_any] = call
    return call


def run(x, mask, Wq, Wkv, Wo, bo, trace=False):
    x = np.asarray(x, np.float32)
    mask = np.asarray(mask, bool)
    Wq = np.asarray(Wq, np.float32)
    Wkv = np.asarray(Wkv, np.float32)
    Wo = np.asarray(Wo, np.float32)
    bo = np.asarray(bo, np.float32)
    mask_any = bool(mask.any())
    maps = _in_maps(x, mask, Wq, Wkv, Wo, bo, mask_any)
    results = _get_runner(mask_any)(maps)
    out = np.empty((B, N, DIM), np.float32)
    for c in range(8):
        g, r = divmod(c, GROUP)
        out[g, r * NQ : (r + 1) * NQ, :] = results[c]["y"]
    return out, results


def kernel(x, mask, Wq, Wkv, Wo, bo):
    out, _ = run(x, mask, Wq, Wkv, Wo, bo)
    return out


# revision 7
# speedup vs baseline: 1.0625x; 1.0017x over previous
"""Distributed multi-head attention kernel for one TRN2 chip (8 NeuronCores).

Problem: y = Attention(x) with b=2, n=2048, dim=1024, heads=16, dim_head=64.

Sharding (data + tensor parallel):
  core c: batch g = c // 4, head-group r = c % 4 (4 heads = 256 inner dims).

Design (v2):
  - Host pre-converts x (transposed to feature-major xT[p, c, t] =
    x[t, 128c+p]) and Wq/Wk/Wv/Wo to bf16, with SCALE folded into Wq, so
    no on-device transposes or conversions are needed.
  - Attention in single-head strips (h, ib): scores psS [128 j, 2*512 i]
    (two j-chunks per PSUM tile pair), exp on ACT -> e bf16 tiles. The
    ACT engine (exp throughput) paces the attention phase; q/k/v
    projections are interleaved into the PE stream one step per exp slot.
  - AV in transposed layout: out [i=128 tok, 65] per i-chunk with a
    ones-column in the v tile giving the softmax denominator in col 64.
    Full 128-partition outputs halve PE cost vs the [65, i] layout, and
    normalization becomes a cheap per-partition DVE scalar op. The four
    i-chunk accumulation groups share one PSUM bank, so the bank is
    zeroed once and all matmuls accumulate with start=False.
  - AV runs in-strip with a 2-pair lag (strips 0-1 lag a full strip while
    the v projections stream in); normalize+transpose-back+staging for
    strip s is deferred into strip s+1 so the PE never stalls on the DVE
    normalization chain at strip boundaries.
  - Transpose-back to [d, tok] via PE with a bf16 identity; two
    gate-scaled DVE copies stage it into the A2A buffer (the wrong-batch
    copy writes zeros).
  - Exchange: four 8-core AllToAll collectives (one per head, fired
    inline as each head's last strip closes) so only the smallest
    possible transfer sits in the tail; receives (plain + SWDGE
    accumulate, summing the two batch halves) are deferred until after
    the last collective so no queue ever blocks a later collective's
    issue.
  - Output projection in two stages: even aT chunks (head pairs 0/1)
    run inside the later collectives' windows; odd chunks in the tail,
    alternating PSUM arenas to avoid write-after-read stalls.
"""

import sys

if "/opt/trn_rl_repo" not in sys.path:
    sys.path.insert(0, "/opt/trn_rl_repo")

from contextlib import ExitStack

import ml_dtypes
import numpy as np

import concourse.bass as bass
from concourse import bacc
import concourse.mybir as mybir
import concourse.tile as tile
from concourse.masks import make_identity

F32 = mybir.dt.float32
F32R = mybir.dt.float32r
BF16 = mybir.dt.bfloat16
EXP = mybir.ActivationFunctionType.Exp

B, N, DIM = 2, 2048, 1024
HEADS, DH = 16, 64
INNER = HEADS * DH            # 1024
SCALE = DH ** -0.5
GROUP = 4                     # tensor-parallel group size (cores per batch)
IC = INNER // GROUP           # 256 inner dims per core (4 heads)
NQ = N // GROUP               # 512 output tokens per core
NEG = -1.0e30

P = 128
TB = 512                      # i-block (moving-dim max)
NT = N // P                   # 16 j-chunks
NJP = NT // 2                 # 8 j-chunk pairs (one psS/exp per pair)
ND = DIM // P                 # 8 contraction chunks
NTB = N // TB                 # 4 token blocks
LAGP = 2                      # AV trails exp by LAGP j-chunk-pairs

_CACHE = {}


def _mm(nc, out, lhsT, rhs, start=True, stop=True, tile_position=None):
    nc.tensor.matmul(
        out, lhsT, rhs, start=start, stop=stop, tile_position=tile_position
    )


def _build(mask_any: bool) -> bass.Bass:
    nc = bacc.Bacc()

    xt_in = nc.declare_dram_parameter("xt_b", [NTB, P, ND, TB], BF16, False)
    wq = nc.declare_dram_parameter("wq_s", [DIM, IC], BF16, False)
    wk = nc.declare_dram_parameter("wk_s", [DIM, IC], BF16, False)
    wv = nc.declare_dram_parameter("wv_s", [DIM, IC], BF16, False)
    wo = nc.declare_dram_parameter("wo", [INNER, DIM], BF16, False)
    bo = nc.declare_dram_parameter("bo", [1, DIM], F32R, False)
    # gate[:, g] = 1.0 iff this core handles batch g (replicated down the
    # partition axis); scales the transpose-back identity so cross-batch
    # A2A chunks carry zeros.
    gate = nc.declare_dram_parameter("gate", [P, 2], F32, False)
    if mask_any:
        mb = nc.declare_dram_parameter("mbias", [P, NT], F32, False)
    y = nc.declare_dram_parameter("y", [NQ, DIM], F32, True)

    with ExitStack() as ctx:
        tc = ctx.enter_context(tile.TileContext(nc))

        const = ctx.enter_context(tc.tile_pool(name="const", bufs=1))
        ident = const.tile([P, P], BF16, tag="ident")
        make_identity(nc, ident[:])
        gate_sb = const.tile([P, 2], F32, tag="gate_sb")
        nc.scalar.dma_start(gate_sb[:], gate.ap())
        ones_f = const.tile([P, P], F32, tag="ones_f")
        nc.vector.memset(ones_f[:], 1.0)
        ones_r = const.tile([P, P], F32R, tag="ones_r")
        nc.vector.tensor_copy(ones_r[:], ones_f[:])
        bo_sb = const.tile([1, DIM], F32R, tag="bo_sb")
        nc.scalar.dma_start(bo_sb[:], bo.ap())
        if mask_any:
            mb_sb = const.tile([P, NT], F32, tag="mb_sb")
            nc.scalar.dma_start(mb_sb[:], mb.ap())

        # ---- persistent SBUF ----
        big = ctx.enter_context(tc.tile_pool(name="big", bufs=1))
        wk_all = big.tile([P, ND, IC], BF16, tag="w", bufs=3, name="wk_all")
        wq_all = big.tile([P, ND, IC], BF16, tag="w", bufs=3, name="wq_all")
        wv_all = big.tile([P, ND, IC], BF16, tag="w", bufs=3, name="wv_all")
        wo_all = big.tile([P, ND, DIM], BF16, tag="woa", name="wo_all")
        wk_sb = [wk_all[:, c, :] for c in range(ND)]
        wq_sb = [wq_all[:, c, :] for c in range(ND)]
        wv_sb = [wv_all[:, c, :] for c in range(ND)]
        wo_sb = [wo_all[:, c, :] for c in range(ND)]
        xTb = [big.tile([P, ND, TB], BF16, tag="xT", bufs=4, name=f"xT{tb}")
               for tb in range(NTB)]

        def xT(c, lo, hi):
            # feature-chunk c, token range [lo, hi) (within one tb block)
            tb, off = divmod(lo, TB)
            return xTb[tb][:, c, off : off + (hi - lo)]
        q2 = [big.tile([P, N], BF16, tag="qk", bufs=4, name=f"q2_{hp}")
              for hp in range(2)]
        k2 = [big.tile([P, N], BF16, tag="qk", bufs=4, name=f"k2_{hp}")
              for hp in range(2)]
        v_all = big.tile([P, 4 * NT * (DH + 1)], BF16, tag="vx",
                         name="v_all")
        va = v_all[:].rearrange("p (h t c) -> p h t c", h=4, t=NT)
        aT = [big.tile([P, NQ], BF16, tag="aT", bufs=8, name=f"aT_{c}")
              for c in range(ND)]
        partial = [big.tile([P, TB], F32, tag="part", bufs=8, name=f"part{i}")
                   for i in range(8)]
        bo_rep = const.tile([P, DIM], F32, tag="bo_rep")

        # DMA issue order: one DMA per weight tensor on the scalar (ACT)
        # HWDGE queue - 4 configs total so neither the ACT sequencer nor the
        # queue's in-flight window ever backs up; x transposes on SP.
        nc.sync.dma_start(xTb[0][:], xt_in.ap()[0])
        nc.scalar.dma_start(wk_all[:], wk.ap().rearrange("(c p) f -> p c f",
                                                         c=ND))
        nc.sync.dma_start(xTb[1][:], xt_in.ap()[1])
        nc.scalar.dma_start(wq_all[:], wq.ap().rearrange("(c p) f -> p c f",
                                                         c=ND))
        nc.sync.dma_start(xTb[2][:], xt_in.ap()[2])
        nc.sync.dma_start(xTb[3][:], xt_in.ap()[3])
        nc.scalar.dma_start(wv_all[:], wv.ap().rearrange("(c p) f -> p c f",
                                                         c=ND))
        nc.scalar.dma_start(wo_all[:], wo.ap().rearrange("(c p) f -> p c f",
                                                         c=ND))

        nc.vector.memset(va[:, :, :, DH], 1.0)

        # A2A buffers (DRAM), one per head: chunk j (64 rows) -> core j;
        # chunk content = this head's [64 d, 512 tok] for quarter j%4,
        # zeros unless j//4 == my batch.
        dram = ctx.enter_context(tc.tile_pool(name="dram", bufs=1, space="DRAM"))
        a2a_in = [dram.tile([8 * DH, NQ], BF16, tag="a2a_in", bufs=4,
                            name=f"a2a_in{h}") for h in range(4)]
        a2a_out = [dram.tile([8 * DH, NQ], BF16, tag="a2a_out", bufs=4,
                             name=f"a2a_out{h}") for h in range(4)]

        ps_s = ctx.enter_context(tc.tile_pool(name="ps_s", bufs=2, space="PSUM"))
        ps_av = ctx.enter_context(tc.tile_pool(name="ps_av", bufs=2, space="PSUM"))
        ps_m = ctx.enter_context(tc.tile_pool(name="ps_m", bufs=2, space="PSUM"))

        sm = ctx.enter_context(tc.tile_pool(name="sm", bufs=1))

        # bias broadcast rows (uses the scores PSUM arena before attention)
        for nb in range(DIM // TB):
            psb = ps_s.tile([P, 2 * TB], F32, tag="s", name="psb")
            _mm(nc, psb[:, 0:TB], ones_r[0:1, :],
                bo_sb[:, nb * TB : (nb + 1) * TB])
            nc.vector.tensor_copy(bo_rep[:, nb * TB : (nb + 1) * TB],
                                  psb[:, 0:TB])

        # ---- projection / wo step closures (popped one per exp slot) ----
        def proj_step(kind, hp, tb):
            def run():
                pj = ps_m.tile([P, TB], F32, tag="m", name="pj")
                wsb = wq_sb if kind == "q" else wk_sb
                dest = q2[hp] if kind == "q" else k2[hp]
                for c in range(ND):
                    _mm(nc, pj[:], wsb[c][:, hp * P : (hp + 1) * P],
                        xT(c, tb * TB, (tb + 1) * TB),
                        start=(c == 0), stop=(c == ND - 1))
                nc.vector.tensor_copy(dest[:, tb * TB : (tb + 1) * TB], pj[:])
            return run

        def proj_half(kind, hp, j):
            # half-range projection: tokens [256j, 256j+256)
            def run():
                pj = ps_m.tile([P, TB], F32, tag="m", name="pjh")
                wsb = wq_sb if kind == "q" else wk_sb
                dest = q2[hp] if kind == "q" else k2[hp]
                lo = j * 256
                for c in range(ND):
                    _mm(nc, pj[:, 0:256], wsb[c][:, hp * P : (hp + 1) * P],
                        xT(c, lo, lo + 256),
                        start=(c == 0), stop=(c == ND - 1))
                nc.vector.tensor_copy(dest[:, lo : lo + 256], pj[:, 0:256])
            return run

        def v_step(t):
            def run():
                pv = ps_m.tile([P, IC], F32, tag="m", name="pv")
                for c in range(ND):
                    _mm(nc, pv[:], xT(c, t * P, (t + 1) * P), wv_sb[c],
                        start=(c == 0), stop=(c == ND - 1))
                nc.vector.tensor_copy(
                    va[:, :, t, 0:DH],
                    pv[:].rearrange("p (h c) -> p h c", h=4))
            return run

        def wo_step(parity, t, nb, alt):
            def run():
                if alt:
                    pw_t = ps_s.tile([P, 2 * TB], F32, tag="s", name="pw")
                    pw = pw_t[:, 0:TB]
                else:
                    pw = ps_m.tile([P, TB], F32, tag="m", name="pw")[:]
                for m in range(4):
                    _mm(nc, pw,
                        aT[2 * m + parity][:, t * P : (t + 1) * P],
                        wo_sb[2 * m + parity][:, nb * TB : (nb + 1) * TB],
                        start=(m == 0), stop=(m == 3))
                if parity == 0:
                    nc.vector.tensor_add(
                        partial[t * 2 + nb][:], pw,
                        bo_rep[:, nb * TB : (nb + 1) * TB])
                else:
                    fo = sm.tile([P, TB], F32, tag="fo", bufs=6, name="fo")
                    nc.vector.tensor_add(fo[:], pw, partial[t * 2 + nb][:])
                    nc.sync.dma_start(
                        y.ap()[t * P : (t + 1) * P, nb * TB : (nb + 1) * TB],
                        fo[:])
            return run

        # stage 1: only the projections that gate the first scores
        proj_step("k", 0, 0)()
        proj_step("k", 0, 1)()
        proj_step("q", 0, 0)()
        proj_step("q", 0, 1)()
        proj_step("k", 0, 2)()
        proj_step("k", 0, 3)()

        # pop queue: exp-slot index -> closure (16 strips x 8 slots = 128).
        # v chunk t is consumed by the (full-strip-lagged) AV at absolute
        # slot 8 + t//2; popping v(t) at slot t-1 stays ahead (same-engine
        # program order guarantees no deadlock even at zero slack).
        popq = {0: [v_step(0), v_step(1)]}
        for t in range(2, NT):
            popq.setdefault(t - 1, []).append(v_step(t))
        popq.setdefault(15, []).append(proj_step("q", 0, 2))
        popq.setdefault(17, []).append(proj_step("q", 0, 3))
        for j in range(8):
            popq.setdefault(32 + 3 * j, []).append(proj_half("k", 1, j))
            popq.setdefault(56 + 3 * j, []).append(proj_half("q", 1, j))

        # ---- attention strips: one head at a time; AV for strip s runs
        # during strip s+1 (one pair per exp slot) so the interleaved
        # v/k/q projection steps never sit behind a waiting AV matmul.
        def do_av(st, jt):
            et = st["es"][jt // 2]
            off = (jt % 2) * TB
            # four accumulation groups share this bank: it was zeroed at
            # strip start and all matmuls accumulate with start=False so
            # their start bits can't clobber each other's partials
            av = st["av"]
            for c in range(4):
                nc.tensor.matmul(
                    av[:, c * (DH + 1) : (c + 1) * (DH + 1)],
                    et[:, off + c * P : off + (c + 1) * P],
                    va[:, st["h"], jt, :],
                    start=False, stop=(jt == NT - 1),
                    skip_group_check=True)

        def norm_dve(st):
            """Normalize (DVE); returns deferred PE-transpose + staging."""
            av, h, ib = st["av"], st["h"], st["ib"]
            rcp = sm.tile([P, 4], F32, tag="rcp", bufs=4, name="rcp")
            with nc.allow_low_precision("softmax denom"):
                nc.vector.reciprocal(rcp[:], av[:, DH :: DH + 1])
            onrm = sm.tile([P, 4 * DH], BF16, tag="onrm", bufs=4, name="onrm")
            for c in range(4):
                nc.vector.tensor_scalar_mul(
                    onrm[:, c * DH : (c + 1) * DH],
                    av[:, c * (DH + 1) : c * (DH + 1) + DH],
                    rcp[:, c : c + 1])

            def part2():
                oT = ps_m.tile([DH, TB], BF16, tag="m", name="oT")
                for c in range(4):
                    nc.tensor.transpose(oT[:, c * P : (c + 1) * P],
                                        onrm[:, c * DH : (c + 1) * DH],
                                        ident[:])
                for g in range(2):
                    stg = sm.tile([DH, TB], BF16, tag="stg", bufs=6,
                                  name="stg")
                    nc.vector.tensor_scalar_mul(stg[:], oT[:],
                                                gate_sb[0:DH, g : g + 1])
                    row = (4 * g + ib) * DH
                    nc.sync.dma_start(a2a_in[h][row : row + DH, :], stg[:])
                if ib == NTB - 1:
                    fire_a2a(h)
            return part2

        def fire_a2a(h):
            nc.gpsimd.collective_compute(
                "AllToAll",
                mybir.AluOpType.bypass,
                replica_groups=[[0, 1, 2, 3, 4, 5, 6, 7]],
                ins=[a2a_in[h].opt()],
                outs=[a2a_out[h].opt()],
            )

        def recv_a2a(h, use_dve=False):
            # receive: aT[2m+hp] rows [e*64, e*64+64) = chunk m + chunk 4+m
            # (exactly one is nonzero). SWDGE accumulate adds the second
            # half; the last head instead loads both halves and adds on DVE
            # (idle in the tail) to skip the serial SWDGE generation chain.
            # Emitted only after the LAST collective so the SP/Pool queues
            # never block a later collective's issue while waiting.
            hp, e = divmod(h, 2)
            for m in range(GROUP):
                dst = aT[2 * m + hp][e * DH : (e + 1) * DH, :]
                nc.sync.dma_start(dst, a2a_out[h][m * DH : (m + 1) * DH, :])
                if use_dve:
                    tmp = sm.tile([DH, NQ], BF16, tag="rtmp", bufs=4,
                                  name="rtmp")
                    nc.sync.dma_start(
                        tmp[:], a2a_out[h][(4 + m) * DH : (5 + m) * DH, :])
                    nc.vector.tensor_add(dst, dst, tmp[:])
                else:
                    nc.gpsimd.dma_start(
                        dst, a2a_out[h][(4 + m) * DH : (5 + m) * DH, :],
                        accum_op=mybir.AluOpType.add)

        # Strips 0-1 lag their AV by a full strip (the v projections are
        # still streaming in); strips 2+ run their own AV in-strip with a
        # 2-pair lag, so each head's A2A fires right at its last strip's
        # end and the four collectives never queue on each other.
        slot = 0
        prev = None
        pend2 = []
        for h in range(4):
            hp, e = divmod(h, 2)
            ksb = k2[hp][e * DH : (e + 1) * DH, :]
            qsb = q2[hp][e * DH : (e + 1) * DH, :]
            tp = (e * DH, 0)
            for ib in range(NTB):
                isl = slice(ib * TB, (ib + 1) * TB)
                s = 4 * h + ib
                in_strip = s >= 2
                cur = {"h": h, "ib": ib, "es": [None] * NJP, "av": None}
                cur["av"] = ps_av.tile([P, 4 * (DH + 1)], F32, tag="av",
                                       name="av")
                nc.vector.memset(cur["av"][:], 0.0)
                for jp in range(NJP):
                    psS = ps_s.tile([P, 2 * TB], F32, tag="s", name="psS")
                    for sub in range(2):
                        jt = 2 * jp + sub
                        jsl = slice(jt * P, (jt + 1) * P)
                        _mm(nc, psS[:, sub * TB : (sub + 1) * TB],
                            ksb[:, jsl], qsb[:, isl], tile_position=tp)
                        if mask_any:
                            nc.vector.tensor_scalar_add(
                                psS[:, sub * TB : (sub + 1) * TB],
                                psS[:, sub * TB : (sub + 1) * TB],
                                mb_sb[:, jt : jt + 1])
                    e_t = sm.tile([P, 2 * TB], BF16, tag="e", bufs=20,
                                  name="e")
                    nc.scalar.activation(e_t[:], psS[:], EXP)
                    cur["es"][jp] = e_t
                    for fn in popq.pop(slot, ()):
                        fn()
                    if jp == 2 and pend2:
                        for fn in pend2:
                            fn()
                        pend2 = []
                    if prev is not None:
                        do_av(prev, 2 * jp)
                        do_av(prev, 2 * jp + 1)
                    if in_strip and jp >= 3:
                        do_av(cur, 2 * (jp - 3))
                        do_av(cur, 2 * (jp - 3) + 1)
                    slot += 1
                if in_strip:
                    if prev is not None:
                        pend2.append(norm_dve(prev))
                        prev = None
                    for jp in range(NJP - 3, NJP):
                        do_av(cur, 2 * jp)
                        do_av(cur, 2 * jp + 1)
                    pc = norm_dve(cur)
                    if ib == NTB - 1:
                        pc()
                    else:
                        pend2.append(pc)
                else:
                    if prev is not None:
                        pend2.append(norm_dve(prev))
                    prev = cur

        for h in range(4):
            recv_a2a(h)

        # wo even stage: runs inside the final collective's window
        wi = 0
        for t in range(NQ // P):
            for nb in range(DIM // TB):
                wo_step(0, t, nb, alt=(wi % 2 == 1))()
                wi += 1
        # drain any unfired pop steps (none expected)
        for i in sorted(popq):
            for fn in popq.pop(i):
                fn()

        # wo odd stage (alternate PSUM arenas to avoid WAR stalls)
        wi = 0
        for t in range(NQ // P):
            for nb in range(DIM // TB):
                wo_step(1, t, nb, alt=(wi % 2 == 1))()
                wi += 1

    nc.compile()
    return nc


def _get_nc(mask_any: bool) -> bass.Bass:
    if mask_any not in _CACHE:
        _CACHE[mask_any] = _build(mask_any)
    return _CACHE[mask_any]


def _in_maps(x, mask, Wq, Wkv, Wo, bo, mask_any):
    bf = ml_dtypes.bfloat16
    bo2 = np.ascontiguousarray(np.asarray(bo, np.float32).reshape(1, DIM))
    wo_bf = np.ascontiguousarray(np.asarray(Wo, np.float32).astype(bf))
    maps = []
    for c in range(8):
        g, r = divmod(c, GROUP)
        csl = slice(r * IC, (r + 1) * IC)
        m = {
            "xt_b": np.ascontiguousarray(
                x[g].reshape(NTB, TB, ND, P).transpose(0, 3, 2, 1).astype(bf)),
            "wq_s": np.ascontiguousarray(
                (Wq[:, csl] * np.float32(SCALE)).astype(bf)),
            "wk_s": np.ascontiguousarray(Wkv[:, csl].astype(bf)),
            "wv_s": np.ascontiguousarray(
                Wkv[:, INNER + r * IC : INNER + (r + 1) * IC].astype(bf)),
            "wo": wo_bf,
            "bo": bo2,
            "gate": np.ascontiguousarray(
                np.tile(np.array([[1.0 - g, float(g)]], np.float32), (P, 1))),
        }
        if mask_any:
            mvec = np.where(mask[g], np.float32(NEG), np.float32(0.0)).astype(
                np.float32)
            m["mbias"] = np.ascontiguousarray(mvec.reshape(NT, P).T)
        maps.append(m)
    return maps


_RUNNER = {}


def _get_runner(mask_any: bool):
    """Cached jax-jitted SPMD executor for the Bass module."""
    if mask_any in _RUNNER:
        return _RUNNER[mask_any]
    import jax
    from jax.sharding import Mesh, PartitionSpec
    from jax.experimental.shard_map import shard_map
    from concourse import bass2jax

    nc = _get_nc(mask_any)
    bass2jax.install_neuronx_cc_hook()

    partition_name = (
        nc.partition_id_tensor.name if nc.partition_id_tensor else None
    )
    in_names, out_names, out_avals = [], [], []
    for alloc in nc.m.functions[0].allocations:
        if not isinstance(alloc, mybir.MemoryLocationSet):
            continue
        name = alloc.memorylocations[0].name
        if alloc.kind == "ExternalInput":
            if name != partition_name:
                in_names.append(name)
        elif alloc.kind == "ExternalOutput":
            shape = tuple(alloc.tensor_shape)
            dtype = mybir.dt.np(alloc.dtype)
            out_names.append(name)
            out_avals.append(jax.core.ShapedArray(shape, dtype))
    n_params = len(in_names)
    n_outs = len(out_avals)
    all_names = list(in_names) + list(out_names)
    if partition_name is not None:
        all_names.append(partition_name)
    donate = tuple(range(n_params, n_params + n_outs))

    def _body(*args):
        operands = list(args)
        if partition_name is not None:
            operands.append(bass2jax.partition_id_tensor())
        outs = bass2jax._bass_exec_p.bind(
            *operands,
            out_avals=tuple(out_avals),
            in_names=tuple(all_names),
            out_names=tuple(out_names),
            lowering_input_output_aliases=(),
            sim_require_finite=True,
            sim_require_nnan=True,
            nc=nc,
        )
        return tuple(outs)

    devices = jax.devices()[:8]
    mesh = Mesh(np.asarray(devices), ("core",))
    in_specs = (PartitionSpec("core"),) * (n_params + n_outs)
    out_specs = (PartitionSpec("core"),) * n_outs
    sharded = jax.jit(
        shard_map(_body, mesh=mesh, in_specs=in_specs, out_specs=out_specs,
                  check_rep=False),
        donate_argnums=donate,
        keep_unused=True,
    )
    zero_shapes = [tuple(a.shape) for a in out_avals]
    zero_dtypes = [a.dtype for a in out_avals]

    def call(maps):
        concat_in = [
            np.concatenate([np.asarray(maps[c][nm]) for c in range(8)], axis=0)
            for nm in in_names
        ]
        concat_zeros = [
            np.zeros((8 * s[0], *s[1:]), d)
            for s, d in zip(zero_shapes, zero_dtypes)
        ]
        out_arrs = sharded(*concat_in, *concat_zeros)
        return [
            {
                nm: np.asarray(out_arrs[i]).reshape(8, *zero_shapes[i])[c]
                for i, nm in enumerate(out_names)
            }
            for c in range(8)
        ]

    _RUNNER[mask_any] = call
    return call


def run(x, mask, Wq, Wkv, Wo, bo, trace=False):
    x = np.asarray(x, np.float32)
    mask = np.asarray(mask, bool)
    Wq = np.asarray(Wq, np.float32)
    Wkv = np.asarray(Wkv, np.float32)
    Wo = np.asarray(Wo, np.float32)
    bo = np.asarray(bo, np.float32)
    mask_any = bool(mask.any())
    maps = _in_maps(x, mask, Wq, Wkv, Wo, bo, mask_any)
    results = _get_runner(mask_any)(maps)
    out = np.empty((B, N, DIM), np.float32)
    for c in range(8):
        g, r = divmod(c, GROUP)
        out[g, r * NQ : (r + 1) * NQ, :] = results[c]["y"]
    return out, results


def kernel(x, mask, Wq, Wkv, Wo, bo):
    out, _ = run(x, mask, Wq, Wkv, Wo, bo)
    return out


# revision 8
# speedup vs baseline: 1.0643x; 1.0017x over previous
"""Distributed multi-head attention kernel for one TRN2 chip (8 NeuronCores).

Problem: y = Attention(x) with b=2, n=2048, dim=1024, heads=16, dim_head=64.

Sharding (data + tensor parallel):
  core c: batch g = c // 4, head-group r = c % 4 (4 heads = 256 inner dims).

Design (v2):
  - Host pre-converts x (transposed to feature-major xT[p, c, t] =
    x[t, 128c+p]) and Wq/Wk/Wv/Wo to bf16, with SCALE folded into Wq, so
    no on-device transposes or conversions are needed.
  - Attention in single-head strips (h, ib): scores psS [128 j, 2*512 i]
    (two j-chunks per PSUM tile pair), exp on ACT -> e bf16 tiles. The
    ACT engine (exp throughput) paces the attention phase; q/k/v
    projections are interleaved into the PE stream one step per exp slot.
  - AV in transposed layout: out [i=128 tok, 65] per i-chunk with a
    ones-column in the v tile giving the softmax denominator in col 64.
    Full 128-partition outputs halve PE cost vs the [65, i] layout, and
    normalization becomes a cheap per-partition DVE scalar op. The four
    i-chunk accumulation groups share one PSUM bank, so the bank is
    zeroed once and all matmuls accumulate with start=False.
  - AV runs in-strip with a 2-pair lag (strips 0-1 lag a full strip while
    the v projections stream in); normalize+transpose-back+staging for
    strip s is deferred into strip s+1 so the PE never stalls on the DVE
    normalization chain at strip boundaries.
  - Transpose-back to [d, tok] via PE with a bf16 identity; two
    gate-scaled DVE copies stage it into the A2A buffer (the wrong-batch
    copy writes zeros).
  - Exchange: four 8-core AllToAll collectives (one per head, fired
    inline as each head's last strip closes) so only the smallest
    possible transfer sits in the tail; receives (plain + SWDGE
    accumulate, summing the two batch halves) are deferred until after
    the last collective so no queue ever blocks a later collective's
    issue.
  - Output projection in two stages: even aT chunks (head pairs 0/1)
    run inside the later collectives' windows; odd chunks in the tail,
    alternating PSUM arenas to avoid write-after-read stalls.
"""

import sys

if "/opt/trn_rl_repo" not in sys.path:
    sys.path.insert(0, "/opt/trn_rl_repo")

from contextlib import ExitStack

import ml_dtypes
import numpy as np

import concourse.bass as bass
from concourse import bacc
import concourse.mybir as mybir
import concourse.tile as tile
from concourse.masks import make_identity

F32 = mybir.dt.float32
F32R = mybir.dt.float32r
BF16 = mybir.dt.bfloat16
EXP = mybir.ActivationFunctionType.Exp

B, N, DIM = 2, 2048, 1024
HEADS, DH = 16, 64
INNER = HEADS * DH            # 1024
SCALE = DH ** -0.5
GROUP = 4                     # tensor-parallel group size (cores per batch)
IC = INNER // GROUP           # 256 inner dims per core (4 heads)
NQ = N // GROUP               # 512 output tokens per core
NEG = -1.0e30

P = 128
TB = 512                      # i-block (moving-dim max)
NT = N // P                   # 16 j-chunks
NJP = NT // 2                 # 8 j-chunk pairs (one psS/exp per pair)
ND = DIM // P                 # 8 contraction chunks
NTB = N // TB                 # 4 token blocks
LAGP = 2                      # AV trails exp by LAGP j-chunk-pairs

_CACHE = {}


def _mm(nc, out, lhsT, rhs, start=True, stop=True, tile_position=None):
    nc.tensor.matmul(
        out, lhsT, rhs, start=start, stop=stop, tile_position=tile_position
    )


def _build(mask_any: bool) -> bass.Bass:
    nc = bacc.Bacc()

    xt_in = nc.declare_dram_parameter("xt_b", [NTB, P, ND, TB], BF16, False)
    wq = nc.declare_dram_parameter("wq_s", [DIM, IC], BF16, False)
    wk = nc.declare_dram_parameter("wk_s", [DIM, IC], BF16, False)
    wv = nc.declare_dram_parameter("wv_s", [DIM, IC], BF16, False)
    wo = nc.declare_dram_parameter("wo", [INNER, DIM], BF16, False)
    bo = nc.declare_dram_parameter("bo", [1, DIM], F32R, False)
    # gate[:, g] = 1.0 iff this core handles batch g (replicated down the
    # partition axis); scales the transpose-back identity so cross-batch
    # A2A chunks carry zeros.
    gate = nc.declare_dram_parameter("gate", [P, 2], F32, False)
    if mask_any:
        mb = nc.declare_dram_parameter("mbias", [P, NT], F32, False)
    y = nc.declare_dram_parameter("y", [NQ, DIM], F32, True)

    with ExitStack() as ctx:
        tc = ctx.enter_context(tile.TileContext(nc))

        const = ctx.enter_context(tc.tile_pool(name="const", bufs=1))
        ident = const.tile([P, P], BF16, tag="ident")
        make_identity(nc, ident[:])
        gate_sb = const.tile([P, 2], F32, tag="gate_sb")
        nc.scalar.dma_start(gate_sb[:], gate.ap())
        ones_f = const.tile([P, P], F32, tag="ones_f")
        nc.vector.memset(ones_f[:], 1.0)
        ones_r = const.tile([P, P], F32R, tag="ones_r")
        nc.vector.tensor_copy(ones_r[:], ones_f[:])
        bo_sb = const.tile([1, DIM], F32R, tag="bo_sb")
        nc.scalar.dma_start(bo_sb[:], bo.ap())
        if mask_any:
            mb_sb = const.tile([P, NT], F32, tag="mb_sb")
            nc.scalar.dma_start(mb_sb[:], mb.ap())

        # ---- persistent SBUF ----
        big = ctx.enter_context(tc.tile_pool(name="big", bufs=1))
        wk_all = big.tile([P, ND, IC], BF16, tag="w", bufs=3, name="wk_all")
        wq_all = big.tile([P, ND, IC], BF16, tag="w", bufs=3, name="wq_all")
        wv_all = big.tile([P, ND, IC], BF16, tag="w", bufs=3, name="wv_all")
        wo_all = big.tile([P, ND, DIM], BF16, tag="woa", name="wo_all")
        wk_sb = [wk_all[:, c, :] for c in range(ND)]
        wq_sb = [wq_all[:, c, :] for c in range(ND)]
        wv_sb = [wv_all[:, c, :] for c in range(ND)]
        wo_sb = [wo_all[:, c, :] for c in range(ND)]
        xTb = [big.tile([P, ND, TB], BF16, tag="xT", bufs=4, name=f"xT{tb}")
               for tb in range(NTB)]

        def xT(c, lo, hi):
            # feature-chunk c, token range [lo, hi) (within one tb block)
            tb, off = divmod(lo, TB)
            return xTb[tb][:, c, off : off + (hi - lo)]
        q2 = [big.tile([P, N], BF16, tag="qk", bufs=4, name=f"q2_{hp}")
              for hp in range(2)]
        k2 = [big.tile([P, N], BF16, tag="qk", bufs=4, name=f"k2_{hp}")
              for hp in range(2)]
        v_all = big.tile([P, 4 * NT * (DH + 1)], BF16, tag="vx",
                         name="v_all")
        va = v_all[:].rearrange("p (h t c) -> p h t c", h=4, t=NT)
        aT = [big.tile([P, NQ], BF16, tag="aT", bufs=8, name=f"aT_{c}")
              for c in range(ND)]
        partial = [big.tile([P, TB], F32, tag="part", bufs=8, name=f"part{i}")
                   for i in range(8)]
        bo_rep = const.tile([P, DIM], F32, tag="bo_rep")

        # DMA issue order: one DMA per weight tensor on the scalar (ACT)
        # HWDGE queue - 4 configs total so neither the ACT sequencer nor the
        # queue's in-flight window ever backs up; x transposes on SP.
        nc.sync.dma_start(xTb[0][:], xt_in.ap()[0])
        nc.scalar.dma_start(wk_all[:], wk.ap().rearrange("(c p) f -> p c f",
                                                         c=ND))
        nc.sync.dma_start(xTb[1][:], xt_in.ap()[1])
        nc.scalar.dma_start(wq_all[:], wq.ap().rearrange("(c p) f -> p c f",
                                                         c=ND))
        nc.sync.dma_start(xTb[2][:], xt_in.ap()[2])
        nc.sync.dma_start(xTb[3][:], xt_in.ap()[3])
        nc.scalar.dma_start(wv_all[:], wv.ap().rearrange("(c p) f -> p c f",
                                                         c=ND))
        nc.scalar.dma_start(wo_all[:], wo.ap().rearrange("(c p) f -> p c f",
                                                         c=ND))

        nc.vector.memset(va[:, :, :, DH], 1.0)

        # A2A buffers (DRAM), one per head: chunk j (64 rows) -> core j;
        # chunk content = this head's [64 d, 512 tok] for quarter j%4,
        # zeros unless j//4 == my batch.
        dram = ctx.enter_context(tc.tile_pool(name="dram", bufs=1, space="DRAM"))
        a2a_in = [dram.tile([8 * DH, NQ], BF16, tag="a2a_in", bufs=4,
                            name=f"a2a_in{h}") for h in range(4)]
        a2a_out = [dram.tile([8 * DH, NQ], BF16, tag="a2a_out", bufs=4,
                             name=f"a2a_out{h}") for h in range(4)]

        ps_s = ctx.enter_context(tc.tile_pool(name="ps_s", bufs=2, space="PSUM"))
        ps_av = ctx.enter_context(tc.tile_pool(name="ps_av", bufs=2, space="PSUM"))
        ps_m = ctx.enter_context(tc.tile_pool(name="ps_m", bufs=2, space="PSUM"))

        sm = ctx.enter_context(tc.tile_pool(name="sm", bufs=1))

        # bias broadcast rows (uses the scores PSUM arena before attention)
        for nb in range(DIM // TB):
            psb = ps_s.tile([P, 2 * TB], F32, tag="s", name="psb")
            _mm(nc, psb[:, 0:TB], ones_r[0:1, :],
                bo_sb[:, nb * TB : (nb + 1) * TB])
            nc.vector.tensor_copy(bo_rep[:, nb * TB : (nb + 1) * TB],
                                  psb[:, 0:TB])

        # ---- projection / wo step closures (popped one per exp slot) ----
        def proj_step(kind, hp, tb):
            def run():
                pj = ps_m.tile([P, TB], F32, tag="m", name="pj")
                wsb = wq_sb if kind == "q" else wk_sb
                dest = q2[hp] if kind == "q" else k2[hp]
                for c in range(ND):
                    _mm(nc, pj[:], wsb[c][:, hp * P : (hp + 1) * P],
                        xT(c, tb * TB, (tb + 1) * TB),
                        start=(c == 0), stop=(c == ND - 1))
                nc.vector.tensor_copy(dest[:, tb * TB : (tb + 1) * TB], pj[:])
            return run

        def proj_half(kind, hp, j):
            # half-range projection: tokens [256j, 256j+256)
            def run():
                pj = ps_m.tile([P, TB], F32, tag="m", name="pjh")
                wsb = wq_sb if kind == "q" else wk_sb
                dest = q2[hp] if kind == "q" else k2[hp]
                lo = j * 256
                for c in range(ND):
                    _mm(nc, pj[:, 0:256], wsb[c][:, hp * P : (hp + 1) * P],
                        xT(c, lo, lo + 256),
                        start=(c == 0), stop=(c == ND - 1))
                nc.vector.tensor_copy(dest[:, lo : lo + 256], pj[:, 0:256])
            return run

        def v_step(t):
            def run():
                pv = ps_m.tile([P, IC], F32, tag="m", name="pv")
                for c in range(ND):
                    _mm(nc, pv[:], xT(c, t * P, (t + 1) * P), wv_sb[c],
                        start=(c == 0), stop=(c == ND - 1))
                nc.vector.tensor_copy(
                    va[:, :, t, 0:DH],
                    pv[:].rearrange("p (h c) -> p h c", h=4))
            return run

        def wo_step(parity, t, nb, alt):
            def run():
                if alt:
                    pw_t = ps_s.tile([P, 2 * TB], F32, tag="s", name="pw")
                    pw = pw_t[:, 0:TB]
                else:
                    pw = ps_m.tile([P, TB], F32, tag="m", name="pw")[:]
                for m in range(4):
                    _mm(nc, pw,
                        aT[2 * m + parity][:, t * P : (t + 1) * P],
                        wo_sb[2 * m + parity][:, nb * TB : (nb + 1) * TB],
                        start=(m == 0), stop=(m == 3))
                if parity == 0:
                    nc.vector.tensor_add(
                        partial[t * 2 + nb][:], pw,
                        bo_rep[:, nb * TB : (nb + 1) * TB])
                else:
                    fo = sm.tile([P, TB], F32, tag="fo", bufs=6, name="fo")
                    nc.vector.tensor_add(fo[:], pw, partial[t * 2 + nb][:])
                    nc.sync.dma_start(
                        y.ap()[t * P : (t + 1) * P, nb * TB : (nb + 1) * TB],
                        fo[:])
            return run

        # stage 1: only the projections that gate the first scores
        proj_step("k", 0, 0)()
        proj_step("k", 0, 1)()
        proj_step("q", 0, 0)()
        proj_step("q", 0, 1)()
        proj_step("k", 0, 2)()
        proj_step("k", 0, 3)()

        # pop queue: exp-slot index -> closure (16 strips x 8 slots = 128).
        # v chunk t is consumed by the (full-strip-lagged) AV at absolute
        # slot 8 + t//2; popping v(t) at slot t-1 stays ahead (same-engine
        # program order guarantees no deadlock even at zero slack).
        popq = {0: [v_step(0), v_step(1)]}
        for t in range(2, NT):
            popq.setdefault(t - 1, []).append(v_step(t))
        popq.setdefault(12, []).append(proj_half("q", 0, 4))
        popq.setdefault(14, []).append(proj_half("q", 0, 5))
        popq.setdefault(16, []).append(proj_half("q", 0, 6))
        popq.setdefault(18, []).append(proj_half("q", 0, 7))
        for j in range(8):
            popq.setdefault(32 + 4 * j, []).append(proj_half("k", 1, j))
            popq.setdefault(50 + 4 * j, []).append(proj_half("q", 1, j))

        # ---- attention strips: one head at a time; AV for strip s runs
        # during strip s+1 (one pair per exp slot) so the interleaved
        # v/k/q projection steps never sit behind a waiting AV matmul.
        def do_av(st, jt):
            et = st["es"][jt // 2]
            off = (jt % 2) * TB
            # four accumulation groups share this bank: it was zeroed at
            # strip start and all matmuls accumulate with start=False so
            # their start bits can't clobber each other's partials
            av = st["av"]
            for c in range(4):
                nc.tensor.matmul(
                    av[:, c * (DH + 1) : (c + 1) * (DH + 1)],
                    et[:, off + c * P : off + (c + 1) * P],
                    va[:, st["h"], jt, :],
                    start=False, stop=(jt == NT - 1),
                    skip_group_check=True)

        def norm_dve(st):
            """Normalize (DVE); returns deferred PE-transpose + staging."""
            av, h, ib = st["av"], st["h"], st["ib"]
            rcp = sm.tile([P, 4], F32, tag="rcp", bufs=4, name="rcp")
            with nc.allow_low_precision("softmax denom"):
                nc.vector.reciprocal(rcp[:], av[:, DH :: DH + 1])
            onrm = sm.tile([P, 4 * DH], BF16, tag="onrm", bufs=4, name="onrm")
            for c in range(4):
                nc.vector.tensor_scalar_mul(
                    onrm[:, c * DH : (c + 1) * DH],
                    av[:, c * (DH + 1) : c * (DH + 1) + DH],
                    rcp[:, c : c + 1])

            def part2():
                oT = ps_m.tile([DH, TB], BF16, tag="m", name="oT")
                for c in range(4):
                    nc.tensor.transpose(oT[:, c * P : (c + 1) * P],
                                        onrm[:, c * DH : (c + 1) * DH],
                                        ident[:])
                for g in range(2):
                    stg = sm.tile([DH, TB], BF16, tag="stg", bufs=6,
                                  name="stg")
                    nc.vector.tensor_scalar_mul(stg[:], oT[:],
                                                gate_sb[0:DH, g : g + 1])
                    row = (4 * g + ib) * DH
                    nc.sync.dma_start(a2a_in[h][row : row + DH, :], stg[:])
                if ib == NTB - 1:
                    fire_a2a(h)
            return part2

        def fire_a2a(h):
            nc.gpsimd.collective_compute(
                "AllToAll",
                mybir.AluOpType.bypass,
                replica_groups=[[0, 1, 2, 3, 4, 5, 6, 7]],
                ins=[a2a_in[h].opt()],
                outs=[a2a_out[h].opt()],
            )

        def recv_a2a(h, use_dve=False):
            # receive: aT[2m+hp] rows [e*64, e*64+64) = chunk m + chunk 4+m
            # (exactly one is nonzero). SWDGE accumulate adds the second
            # half; the last head instead loads both halves and adds on DVE
            # (idle in the tail) to skip the serial SWDGE generation chain.
            # Emitted only after the LAST collective so the SP/Pool queues
            # never block a later collective's issue while waiting.
            hp, e = divmod(h, 2)
            for m in range(GROUP):
                dst = aT[2 * m + hp][e * DH : (e + 1) * DH, :]
                nc.sync.dma_start(dst, a2a_out[h][m * DH : (m + 1) * DH, :])
                if use_dve:
                    tmp = sm.tile([DH, NQ], BF16, tag="rtmp", bufs=4,
                                  name="rtmp")
                    nc.sync.dma_start(
                        tmp[:], a2a_out[h][(4 + m) * DH : (5 + m) * DH, :])
                    nc.vector.tensor_add(dst, dst, tmp[:])
                else:
                    nc.gpsimd.dma_start(
                        dst, a2a_out[h][(4 + m) * DH : (5 + m) * DH, :],
                        accum_op=mybir.AluOpType.add)

        # Strips 0-1 lag their AV by a full strip (the v projections are
        # still streaming in); strips 2+ run their own AV in-strip with a
        # 2-pair lag, so each head's A2A fires right at its last strip's
        # end and the four collectives never queue on each other.
        slot = 0
        prev = None
        pend2 = []
        for h in range(4):
            hp, e = divmod(h, 2)
            ksb = k2[hp][e * DH : (e + 1) * DH, :]
            qsb = q2[hp][e * DH : (e + 1) * DH, :]
            tp = (e * DH, 0)
            for ib in range(NTB):
                isl = slice(ib * TB, (ib + 1) * TB)
                s = 4 * h + ib
                in_strip = s >= 2
                cur = {"h": h, "ib": ib, "es": [None] * NJP, "av": None}
                cur["av"] = ps_av.tile([P, 4 * (DH + 1)], F32, tag="av",
                                       name="av")
                nc.vector.memset(cur["av"][:], 0.0)
                for jp in range(NJP):
                    psS = ps_s.tile([P, 2 * TB], F32, tag="s", name="psS")
                    for sub in range(2):
                        jt = 2 * jp + sub
                        jsl = slice(jt * P, (jt + 1) * P)
                        _mm(nc, psS[:, sub * TB : (sub + 1) * TB],
                            ksb[:, jsl], qsb[:, isl], tile_position=tp)
                        if mask_any:
                            nc.vector.tensor_scalar_add(
                                psS[:, sub * TB : (sub + 1) * TB],
                                psS[:, sub * TB : (sub + 1) * TB],
                                mb_sb[:, jt : jt + 1])
                    e_t = sm.tile([P, 2 * TB], BF16, tag="e", bufs=20,
                                  name="e")
                    nc.scalar.activation(e_t[:], psS[:], EXP)
                    cur["es"][jp] = e_t
                    for fn in popq.pop(slot, ()):
                        fn()
                    if jp == 2 and pend2:
                        for fn in pend2:
                            fn()
                        pend2 = []
                    if prev is not None:
                        do_av(prev, 2 * jp)
                        do_av(prev, 2 * jp + 1)
                    if in_strip and jp >= 3:
                        do_av(cur, 2 * (jp - 3))
                        do_av(cur, 2 * (jp - 3) + 1)
                    slot += 1
                if in_strip:
                    if prev is not None:
                        pend2.append(norm_dve(prev))
                        prev = None
                    for jp in range(NJP - 3, NJP):
                        do_av(cur, 2 * jp)
                        do_av(cur, 2 * jp + 1)
                    pc = norm_dve(cur)
                    if ib == NTB - 1:
                        pc()
                    else:
                        pend2.append(pc)
                else:
                    if prev is not None:
                        pend2.append(norm_dve(prev))
                    prev = cur

        for h in range(4):
            recv_a2a(h)

        # wo even stage: runs inside the final collective's window
        wi = 0
        for t in range(NQ // P):
            for nb in range(DIM // TB):
                wo_step(0, t, nb, alt=(wi % 2 == 1))()
                wi += 1
        # drain any unfired pop steps (none expected)
        for i in sorted(popq):
            for fn in popq.pop(i):
                fn()

        # wo odd stage (alternate PSUM arenas to avoid WAR stalls)
        wi = 0
        for t in range(NQ // P):
            for nb in range(DIM // TB):
                wo_step(1, t, nb, alt=(wi % 2 == 1))()
                wi += 1

    nc.compile()
    return nc


def _get_nc(mask_any: bool) -> bass.Bass:
    if mask_any not in _CACHE:
        _CACHE[mask_any] = _build(mask_any)
    return _CACHE[mask_any]


def _in_maps(x, mask, Wq, Wkv, Wo, bo, mask_any):
    bf = ml_dtypes.bfloat16
    bo2 = np.ascontiguousarray(np.asarray(bo, np.float32).reshape(1, DIM))
    wo_bf = np.ascontiguousarray(np.asarray(Wo, np.float32).astype(bf))
    maps = []
    for c in range(8):
        g, r = divmod(c, GROUP)
        csl = slice(r * IC, (r + 1) * IC)
        m = {
            "xt_b": np.ascontiguousarray(
                x[g].reshape(NTB, TB, ND, P).transpose(0, 3, 2, 1).astype(bf)),
            "wq_s": np.ascontiguousarray(
                (Wq[:, csl] * np.float32(SCALE)).astype(bf)),
            "wk_s": np.ascontiguousarray(Wkv[:, csl].astype(bf)),
            "wv_s": np.ascontiguousarray(
                Wkv[:, INNER + r * IC : INNER + (r + 1) * IC].astype(bf)),
            "wo": wo_bf,
            "bo": bo2,
            "gate": np.ascontiguousarray(
                np.tile(np.array([[1.0 - g, float(g)]], np.float32), (P, 1))),
        }
        if mask_any:
            mvec = np.where(mask[g], np.float32(NEG), np.float32(0.0)).astype(
                np.float32)
            m["mbias"] = np.ascontiguousarray(mvec.reshape(NT, P).T)
        maps.append(m)
    return maps


_RUNNER = {}


def _get_runner(mask_any: bool):
    """Cached jax-jitted SPMD executor for the Bass module."""
    if mask_any in _RUNNER:
        return _RUNNER[mask_any]
    import jax
    from jax.sharding import Mesh, PartitionSpec
    from jax.experimental.shard_map import shard_map
    from concourse import bass2jax

    nc = _get_nc(mask_any)
    bass2jax.install_neuronx_cc_hook()

    partition_name = (
        nc.partition_id_tensor.name if nc.partition_id_tensor else None
    )
    in_names, out_names, out_avals = [], [], []
    for alloc in nc.m.functions[0].allocations:
        if not isinstance(alloc, mybir.MemoryLocationSet):
            continue
        name = alloc.memorylocations[0].name
        if alloc.kind == "ExternalInput":
            if name != partition_name:
                in_names.append(name)
        elif alloc.kind == "ExternalOutput":
            shape = tuple(alloc.tensor_shape)
            dtype = mybir.dt.np(alloc.dtype)
            out_names.append(name)
            out_avals.append(jax.core.ShapedArray(shape, dtype))
    n_params = len(in_names)
    n_outs = len(out_avals)
    all_names = list(in_names) + list(out_names)
    if partition_name is not None:
        all_names.append(partition_name)
    donate = tuple(range(n_params, n_params + n_outs))

    def _body(*args):
        operands = list(args)
        if partition_name is not None:
            operands.append(bass2jax.partition_id_tensor())
        outs = bass2jax._bass_exec_p.bind(
            *operands,
            out_avals=tuple(out_avals),
            in_names=tuple(all_names),
            out_names=tuple(out_names),
            lowering_input_output_aliases=(),
            sim_require_finite=True,
            sim_require_nnan=True,
            nc=nc,
        )
        return tuple(outs)

    devices = jax.devices()[:8]
    mesh = Mesh(np.asarray(devices), ("core",))
    in_specs = (PartitionSpec("core"),) * (n_params + n_outs)
    out_specs = (PartitionSpec("core"),) * n_outs
    sharded = jax.jit(
        shard_map(_body, mesh=mesh, in_specs=in_specs, out_specs=out_specs,
                  check_rep=False),
        donate_argnums=donate,
        keep_unused=True,
    )
    zero_shapes = [tuple(a.shape) for a in out_avals]
    zero_dtypes = [a.dtype for a in out_avals]

    def call(maps):
        concat_in = [
            np.concatenate([np.asarray(maps[c][nm]) for c in range(8)], axis=0)
            for nm in in_names
        ]
        concat_zeros = [
            np.zeros((8 * s[0], *s[1:]), d)
            for s, d in zip(zero_shapes, zero_dtypes)
        ]
        out_arrs = sharded(*concat_in, *concat_zeros)
        return [
            {
                nm: np.asarray(out_arrs[i]).reshape(8, *zero_shapes[i])[c]
                for i, nm in enumerate(out_names)
            }
            for c in range(8)
        ]

    _RUNNER[mask_any] = call
    return call


def run(x, mask, Wq, Wkv, Wo, bo, trace=False):
    x = np.asarray(x, np.float32)
    mask = np.asarray(mask, bool)
    Wq = np.asarray(Wq, np.float32)
    Wkv = np.asarray(Wkv, np.float32)
    Wo = np.asarray(Wo, np.float32)
    bo = np.asarray(bo, np.float32)
    mask_any = bool(mask.any())
    maps = _in_maps(x, mask, Wq, Wkv, Wo, bo, mask_any)
    results = _get_runner(mask_any)(maps)
    out = np.empty((B, N, DIM), np.float32)
    for c in range(8):
        g, r = divmod(c, GROUP)
        out[g, r * NQ : (r + 1) * NQ, :] = results[c]["y"]
    return out, results


def kernel(x, mask, Wq, Wkv, Wo, bo):
    out, _ = run(x, mask, Wq, Wkv, Wo, bo)
    return out


# revision 9
# speedup vs baseline: 1.0644x; 1.0001x over previous
"""Distributed multi-head attention kernel for one TRN2 chip (8 NeuronCores).

Problem: y = Attention(x) with b=2, n=2048, dim=1024, heads=16, dim_head=64.

Sharding (data + tensor parallel):
  core c: batch g = c // 4, head-group r = c % 4 (4 heads = 256 inner dims).

Design (v2):
  - Host pre-converts x (transposed to feature-major xT[p, c, t] =
    x[t, 128c+p]) and Wq/Wk/Wv/Wo to bf16, with SCALE folded into Wq, so
    no on-device transposes or conversions are needed.
  - Attention in single-head strips (h, ib): scores psS [128 j, 2*512 i]
    (two j-chunks per PSUM tile pair), exp on ACT -> e bf16 tiles. The
    ACT engine (exp throughput) paces the attention phase; q/k/v
    projections are interleaved into the PE stream one step per exp slot.
  - AV in transposed layout: out [i=128 tok, 65] per i-chunk with a
    ones-column in the v tile giving the softmax denominator in col 64.
    Full 128-partition outputs halve PE cost vs the [65, i] layout, and
    normalization becomes a cheap per-partition DVE scalar op. The four
    i-chunk accumulation groups share one PSUM bank, so the bank is
    zeroed once and all matmuls accumulate with start=False.
  - AV runs in-strip with a 2-pair lag (strips 0-1 lag a full strip while
    the v projections stream in); normalize+transpose-back+staging for
    strip s is deferred into strip s+1 so the PE never stalls on the DVE
    normalization chain at strip boundaries.
  - Transpose-back to [d, tok] via PE with a bf16 identity; two
    gate-scaled DVE copies stage it into the A2A buffer (the wrong-batch
    copy writes zeros).
  - Exchange: four 8-core AllToAll collectives (one per head, fired
    inline as each head's last strip closes) so only the smallest
    possible transfer sits in the tail; receives (plain + SWDGE
    accumulate, summing the two batch halves) are deferred until after
    the last collective so no queue ever blocks a later collective's
    issue.
  - Output projection in two stages: even aT chunks (head pairs 0/1)
    run inside the later collectives' windows; odd chunks in the tail,
    alternating PSUM arenas to avoid write-after-read stalls.
"""

import sys

if "/opt/trn_rl_repo" not in sys.path:
    sys.path.insert(0, "/opt/trn_rl_repo")

from contextlib import ExitStack

import ml_dtypes
import numpy as np

import concourse.bass as bass
from concourse import bacc
import concourse.mybir as mybir
import concourse.tile as tile
from concourse.masks import make_identity

F32 = mybir.dt.float32
F32R = mybir.dt.float32r
BF16 = mybir.dt.bfloat16
EXP = mybir.ActivationFunctionType.Exp

B, N, DIM = 2, 2048, 1024
HEADS, DH = 16, 64
INNER = HEADS * DH            # 1024
SCALE = DH ** -0.5
GROUP = 4                     # tensor-parallel group size (cores per batch)
IC = INNER // GROUP           # 256 inner dims per core (4 heads)
NQ = N // GROUP               # 512 output tokens per core
NEG = -1.0e30

P = 128
TB = 512                      # i-block (moving-dim max)
NT = N // P                   # 16 j-chunks
NJP = NT // 2                 # 8 j-chunk pairs (one psS/exp per pair)
ND = DIM // P                 # 8 contraction chunks
NTB = N // TB                 # 4 token blocks
LAGP = 2                      # AV trails exp by LAGP j-chunk-pairs

_CACHE = {}


def _mm(nc, out, lhsT, rhs, start=True, stop=True, tile_position=None):
    nc.tensor.matmul(
        out, lhsT, rhs, start=start, stop=stop, tile_position=tile_position
    )


def _build(mask_any: bool) -> bass.Bass:
    nc = bacc.Bacc()

    xt_in = nc.declare_dram_parameter("xt_b", [NTB, P, ND, TB], BF16, False)
    wq = nc.declare_dram_parameter("wq_s", [DIM, IC], BF16, False)
    wk = nc.declare_dram_parameter("wk_s", [DIM, IC], BF16, False)
    wv = nc.declare_dram_parameter("wv_s", [DIM, IC], BF16, False)
    wo = nc.declare_dram_parameter("wo", [INNER, DIM], BF16, False)
    bo = nc.declare_dram_parameter("bo", [1, DIM], F32R, False)
    # gate[:, g] = 1.0 iff this core handles batch g (replicated down the
    # partition axis); scales the transpose-back identity so cross-batch
    # A2A chunks carry zeros.
    gate = nc.declare_dram_parameter("gate", [P, 2], F32, False)
    if mask_any:
        mb = nc.declare_dram_parameter("mbias", [P, NT], F32, False)
    y = nc.declare_dram_parameter("y", [NQ, DIM], F32, True)

    with ExitStack() as ctx:
        tc = ctx.enter_context(tile.TileContext(nc))

        const = ctx.enter_context(tc.tile_pool(name="const", bufs=1))
        ident = const.tile([P, P], BF16, tag="ident")
        make_identity(nc, ident[:])
        gate_sb = const.tile([P, 2], F32, tag="gate_sb")
        nc.scalar.dma_start(gate_sb[:], gate.ap())
        ones_f = const.tile([P, P], F32, tag="ones_f")
        nc.vector.memset(ones_f[:], 1.0)
        ones_r = const.tile([P, P], F32R, tag="ones_r")
        nc.vector.tensor_copy(ones_r[:], ones_f[:])
        bo_sb = const.tile([1, DIM], F32R, tag="bo_sb")
        nc.scalar.dma_start(bo_sb[:], bo.ap())
        if mask_any:
            mb_sb = const.tile([P, NT], F32, tag="mb_sb")
            nc.scalar.dma_start(mb_sb[:], mb.ap())

        # ---- persistent SBUF ----
        big = ctx.enter_context(tc.tile_pool(name="big", bufs=1))
        wk_all = big.tile([P, ND, IC], BF16, tag="w", bufs=3, name="wk_all")
        wq_all = big.tile([P, ND, IC], BF16, tag="w", bufs=3, name="wq_all")
        wv_all = big.tile([P, ND, IC], BF16, tag="w", bufs=3, name="wv_all")
        wo_all = big.tile([P, ND, DIM], BF16, tag="woa", name="wo_all")
        wk_sb = [wk_all[:, c, :] for c in range(ND)]
        wq_sb = [wq_all[:, c, :] for c in range(ND)]
        wv_sb = [wv_all[:, c, :] for c in range(ND)]
        wo_sb = [wo_all[:, c, :] for c in range(ND)]
        xTb = [big.tile([P, ND, TB], BF16, tag="xT", bufs=4, name=f"xT{tb}")
               for tb in range(NTB)]

        def xT(c, lo, hi):
            # feature-chunk c, token range [lo, hi) (within one tb block)
            tb, off = divmod(lo, TB)
            return xTb[tb][:, c, off : off + (hi - lo)]
        q2 = [big.tile([P, N], BF16, tag="qk", bufs=4, name=f"q2_{hp}")
              for hp in range(2)]
        k2 = [big.tile([P, N], BF16, tag="qk", bufs=4, name=f"k2_{hp}")
              for hp in range(2)]
        v_all = big.tile([P, 4 * NT * (DH + 1)], BF16, tag="vx",
                         name="v_all")
        va = v_all[:].rearrange("p (h t c) -> p h t c", h=4, t=NT)
        aT = [big.tile([P, NQ], BF16, tag="aT", bufs=8, name=f"aT_{c}")
              for c in range(ND)]
        partial = [big.tile([P, TB], F32, tag="part", bufs=8, name=f"part{i}")
                   for i in range(8)]
        bo_rep = const.tile([P, DIM], F32, tag="bo_rep")

        # DMA issue order: one DMA per weight tensor on the scalar (ACT)
        # HWDGE queue - 4 configs total so neither the ACT sequencer nor the
        # queue's in-flight window ever backs up; x transposes on SP.
        nc.sync.dma_start(xTb[0][:], xt_in.ap()[0])
        nc.scalar.dma_start(wk_all[:], wk.ap().rearrange("(c p) f -> p c f",
                                                         c=ND))
        nc.sync.dma_start(xTb[1][:], xt_in.ap()[1])
        nc.scalar.dma_start(wq_all[:], wq.ap().rearrange("(c p) f -> p c f",
                                                         c=ND))
        nc.sync.dma_start(xTb[2][:], xt_in.ap()[2])
        nc.sync.dma_start(xTb[3][:], xt_in.ap()[3])
        nc.scalar.dma_start(wv_all[:], wv.ap().rearrange("(c p) f -> p c f",
                                                         c=ND))
        nc.scalar.dma_start(wo_all[:], wo.ap().rearrange("(c p) f -> p c f",
                                                         c=ND))

        nc.vector.memset(va[:, :, :, DH], 1.0)

        # A2A buffers (DRAM), one per head: chunk j (64 rows) -> core j;
        # chunk content = this head's [64 d, 512 tok] for quarter j%4,
        # zeros unless j//4 == my batch.
        dram = ctx.enter_context(tc.tile_pool(name="dram", bufs=1, space="DRAM"))
        a2a_in = [dram.tile([8 * DH, NQ], BF16, tag="a2a_in", bufs=4,
                            name=f"a2a_in{h}") for h in range(4)]
        a2a_out = [dram.tile([8 * DH, NQ], BF16, tag="a2a_out", bufs=4,
                             name=f"a2a_out{h}") for h in range(4)]

        ps_s = ctx.enter_context(tc.tile_pool(name="ps_s", bufs=2, space="PSUM"))
        ps_av = ctx.enter_context(tc.tile_pool(name="ps_av", bufs=2, space="PSUM"))
        ps_m = ctx.enter_context(tc.tile_pool(name="ps_m", bufs=2, space="PSUM"))

        sm = ctx.enter_context(tc.tile_pool(name="sm", bufs=1))

        # bias broadcast rows (uses the scores PSUM arena before attention)
        for nb in range(DIM // TB):
            psb = ps_s.tile([P, 2 * TB], F32, tag="s", name="psb")
            _mm(nc, psb[:, 0:TB], ones_r[0:1, :],
                bo_sb[:, nb * TB : (nb + 1) * TB])
            nc.vector.tensor_copy(bo_rep[:, nb * TB : (nb + 1) * TB],
                                  psb[:, 0:TB])

        # ---- projection / wo step closures (popped one per exp slot) ----
        def proj_step(kind, hp, tb):
            def run():
                pj = ps_m.tile([P, TB], F32, tag="m", name="pj")
                wsb = wq_sb if kind == "q" else wk_sb
                dest = q2[hp] if kind == "q" else k2[hp]
                for c in range(ND):
                    _mm(nc, pj[:], wsb[c][:, hp * P : (hp + 1) * P],
                        xT(c, tb * TB, (tb + 1) * TB),
                        start=(c == 0), stop=(c == ND - 1))
                nc.vector.tensor_copy(dest[:, tb * TB : (tb + 1) * TB], pj[:])
            return run

        def proj_half(kind, hp, j):
            # half-range projection: tokens [256j, 256j+256)
            def run():
                pj = ps_m.tile([P, TB], F32, tag="m", name="pjh")
                wsb = wq_sb if kind == "q" else wk_sb
                dest = q2[hp] if kind == "q" else k2[hp]
                lo = j * 256
                for c in range(ND):
                    _mm(nc, pj[:, 0:256], wsb[c][:, hp * P : (hp + 1) * P],
                        xT(c, lo, lo + 256),
                        start=(c == 0), stop=(c == ND - 1))
                nc.vector.tensor_copy(dest[:, lo : lo + 256], pj[:, 0:256])
            return run

        def v_step(t):
            def run():
                pv = ps_m.tile([P, IC], F32, tag="m", name="pv")
                for c in range(ND):
                    _mm(nc, pv[:], xT(c, t * P, (t + 1) * P), wv_sb[c],
                        start=(c == 0), stop=(c == ND - 1))
                nc.vector.tensor_copy(
                    va[:, :, t, 0:DH],
                    pv[:].rearrange("p (h c) -> p h c", h=4))
            return run

        def wo_step(parity, t, nb, alt):
            def run():
                if alt:
                    pw_t = ps_s.tile([P, 2 * TB], F32, tag="s", name="pw")
                    pw = pw_t[:, 0:TB]
                else:
                    pw = ps_m.tile([P, TB], F32, tag="m", name="pw")[:]
                for m in range(4):
                    _mm(nc, pw,
                        aT[2 * m + parity][:, t * P : (t + 1) * P],
                        wo_sb[2 * m + parity][:, nb * TB : (nb + 1) * TB],
                        start=(m == 0), stop=(m == 3))
                if parity == 0:
                    nc.vector.tensor_add(
                        partial[t * 2 + nb][:], pw,
                        bo_rep[:, nb * TB : (nb + 1) * TB])
                else:
                    fo = sm.tile([P, TB], F32, tag="fo", bufs=6, name="fo")
                    nc.vector.tensor_add(fo[:], pw, partial[t * 2 + nb][:])
                    nc.sync.dma_start(
                        y.ap()[t * P : (t + 1) * P, nb * TB : (nb + 1) * TB],
                        fo[:])
            return run

        # stage 1: only the projections that gate the first scores
        proj_step("k", 0, 0)()
        proj_step("k", 0, 1)()
        proj_step("q", 0, 0)()
        proj_step("q", 0, 1)()
        proj_step("k", 0, 2)()
        proj_step("k", 0, 3)()

        # pop queue: exp-slot index -> closure (16 strips x 8 slots = 128).
        # v chunk t is consumed by the (full-strip-lagged) AV at absolute
        # slot 8 + t//2; popping v(t) at slot t-1 stays ahead (same-engine
        # program order guarantees no deadlock even at zero slack).
        popq = {0: [v_step(0), v_step(1)]}
        for t in range(2, NT):
            popq.setdefault(t - 1, []).append(v_step(t))
        popq.setdefault(12, []).append(proj_half("q", 0, 4))
        popq.setdefault(14, []).append(proj_half("q", 0, 5))
        popq.setdefault(16, []).append(proj_half("q", 0, 6))
        popq.setdefault(18, []).append(proj_half("q", 0, 7))
        for j in range(8):
            popq.setdefault(32 + 4 * j, []).append(proj_half("k", 1, j))
            popq.setdefault(50 + 4 * j, []).append(proj_half("q", 1, j))

        # ---- attention strips: one head at a time; AV for strip s runs
        # during strip s+1 (one pair per exp slot) so the interleaved
        # v/k/q projection steps never sit behind a waiting AV matmul.
        def do_av(st, jt):
            et = st["es"][jt // 2]
            off = (jt % 2) * TB
            # four accumulation groups share this bank: it was zeroed at
            # strip start and all matmuls accumulate with start=False so
            # their start bits can't clobber each other's partials
            av = st["av"]
            for c in range(4):
                nc.tensor.matmul(
                    av[:, c * (DH + 1) : (c + 1) * (DH + 1)],
                    et[:, off + c * P : off + (c + 1) * P],
                    va[:, st["h"], jt, :],
                    start=False, stop=(jt == NT - 1),
                    skip_group_check=True)

        def norm_dve(st):
            """Normalize (DVE); returns deferred PE-transpose + staging."""
            av, h, ib = st["av"], st["h"], st["ib"]
            rcp = sm.tile([P, 4], F32, tag="rcp", bufs=4, name="rcp")
            with nc.allow_low_precision("softmax denom"):
                nc.vector.reciprocal(rcp[:], av[:, DH :: DH + 1])
            onrm = sm.tile([P, 4 * DH], BF16, tag="onrm", bufs=4, name="onrm")
            for c in range(4):
                nc.vector.tensor_scalar_mul(
                    onrm[:, c * DH : (c + 1) * DH],
                    av[:, c * (DH + 1) : c * (DH + 1) + DH],
                    rcp[:, c : c + 1])

            def part2():
                oT = ps_m.tile([DH, TB], BF16, tag="m", name="oT")
                for c in range(4):
                    nc.tensor.transpose(oT[:, c * P : (c + 1) * P],
                                        onrm[:, c * DH : (c + 1) * DH],
                                        ident[:])
                for g in range(2):
                    stg = sm.tile([DH, TB], BF16, tag="stg", bufs=6,
                                  name="stg")
                    nc.vector.tensor_scalar_mul(stg[:], oT[:],
                                                gate_sb[0:DH, g : g + 1])
                    row = (4 * g + ib) * DH
                    nc.sync.dma_start(a2a_in[h][row : row + DH, :], stg[:])
                if ib == NTB - 1:
                    fire_a2a(h)
            return part2

        def fire_a2a(h):
            nc.gpsimd.collective_compute(
                "AllToAll",
                mybir.AluOpType.bypass,
                replica_groups=[[0, 1, 2, 3, 4, 5, 6, 7]],
                ins=[a2a_in[h].opt()],
                outs=[a2a_out[h].opt()],
            )

        def recv_a2a(h, use_dve=False):
            # receive: aT[2m+hp] rows [e*64, e*64+64) = chunk m + chunk 4+m
            # (exactly one is nonzero). SWDGE accumulate adds the second
            # half; the last head instead loads both halves and adds on DVE
            # (idle in the tail) to skip the serial SWDGE generation chain.
            # Emitted only after the LAST collective so the SP/Pool queues
            # never block a later collective's issue while waiting.
            hp, e = divmod(h, 2)
            for m in range(GROUP):
                dst = aT[2 * m + hp][e * DH : (e + 1) * DH, :]
                nc.sync.dma_start(dst, a2a_out[h][m * DH : (m + 1) * DH, :])
                if use_dve:
                    tmp = sm.tile([DH, NQ], BF16, tag="rtmp", bufs=4,
                                  name="rtmp")
                    nc.sync.dma_start(
                        tmp[:], a2a_out[h][(4 + m) * DH : (5 + m) * DH, :])
                    nc.vector.tensor_add(dst, dst, tmp[:])
                else:
                    nc.gpsimd.dma_start(
                        dst, a2a_out[h][(4 + m) * DH : (5 + m) * DH, :],
                        accum_op=mybir.AluOpType.add)

        # Strips 0-1 lag their AV by a full strip (the v projections are
        # still streaming in); strips 2+ run their own AV in-strip with a
        # 2-pair lag, so each head's A2A fires right at its last strip's
        # end and the four collectives never queue on each other.
        slot = 0
        prev = None
        pend2 = []
        for h in range(4):
            hp, e = divmod(h, 2)
            ksb = k2[hp][e * DH : (e + 1) * DH, :]
            qsb = q2[hp][e * DH : (e + 1) * DH, :]
            tp = (e * DH, 0)
            for ib in range(NTB):
                isl = slice(ib * TB, (ib + 1) * TB)
                s = 4 * h + ib
                in_strip = s >= 2
                cur = {"h": h, "ib": ib, "es": [None] * NJP, "av": None}
                cur["av"] = ps_av.tile([P, 4 * (DH + 1)], F32, tag="av",
                                       name="av")
                nc.vector.memset(cur["av"][:], 0.0)
                for jp in range(NJP):
                    psS = ps_s.tile([P, 2 * TB], F32, tag="s", name="psS")
                    for sub in range(2):
                        jt = 2 * jp + sub
                        jsl = slice(jt * P, (jt + 1) * P)
                        _mm(nc, psS[:, sub * TB : (sub + 1) * TB],
                            ksb[:, jsl], qsb[:, isl], tile_position=tp)
                        if mask_any:
                            nc.vector.tensor_scalar_add(
                                psS[:, sub * TB : (sub + 1) * TB],
                                psS[:, sub * TB : (sub + 1) * TB],
                                mb_sb[:, jt : jt + 1])
                    e_t = sm.tile([P, 2 * TB], BF16, tag="e", bufs=20,
                                  name="e")
                    nc.scalar.activation(e_t[:], psS[:], EXP)
                    cur["es"][jp] = e_t
                    for fn in popq.pop(slot, ()):
                        fn()
                    if jp == 3 and pend2:
                        for fn in pend2:
                            fn()
                        pend2 = []
                    if prev is not None:
                        do_av(prev, 2 * jp)
                        do_av(prev, 2 * jp + 1)
                    if in_strip and jp >= 3:
                        do_av(cur, 2 * (jp - 3))
                        do_av(cur, 2 * (jp - 3) + 1)
                    slot += 1
                if in_strip:
                    if prev is not None:
                        pend2.append(norm_dve(prev))
                        prev = None
                    for jp in range(NJP - 3, NJP):
                        do_av(cur, 2 * jp)
                        do_av(cur, 2 * jp + 1)
                    pc = norm_dve(cur)
                    if ib == NTB - 1:
                        pc()
                    else:
                        pend2.append(pc)
                else:
                    if prev is not None:
                        pend2.append(norm_dve(prev))
                    prev = cur

        for h in range(4):
            recv_a2a(h)

        # wo even stage: runs inside the final collective's window
        wi = 0
        for t in range(NQ // P):
            for nb in range(DIM // TB):
                wo_step(0, t, nb, alt=(wi % 2 == 1))()
                wi += 1
        # drain any unfired pop steps (none expected)
        for i in sorted(popq):
            for fn in popq.pop(i):
                fn()

        # wo odd stage (alternate PSUM arenas to avoid WAR stalls)
        wi = 0
        for t in range(NQ // P):
            for nb in range(DIM // TB):
                wo_step(1, t, nb, alt=(wi % 2 == 1))()
                wi += 1

    nc.compile()
    return nc


def _get_nc(mask_any: bool) -> bass.Bass:
    if mask_any not in _CACHE:
        _CACHE[mask_any] = _build(mask_any)
    return _CACHE[mask_any]


def _in_maps(x, mask, Wq, Wkv, Wo, bo, mask_any):
    bf = ml_dtypes.bfloat16
    bo2 = np.ascontiguousarray(np.asarray(bo, np.float32).reshape(1, DIM))
    wo_bf = np.ascontiguousarray(np.asarray(Wo, np.float32).astype(bf))
    maps = []
    for c in range(8):
        g, r = divmod(c, GROUP)
        csl = slice(r * IC, (r + 1) * IC)
        m = {
            "xt_b": np.ascontiguousarray(
                x[g].reshape(NTB, TB, ND, P).transpose(0, 3, 2, 1).astype(bf)),
            "wq_s": np.ascontiguousarray(
                (Wq[:, csl] * np.float32(SCALE)).astype(bf)),
            "wk_s": np.ascontiguousarray(Wkv[:, csl].astype(bf)),
            "wv_s": np.ascontiguousarray(
                Wkv[:, INNER + r * IC : INNER + (r + 1) * IC].astype(bf)),
            "wo": wo_bf,
            "bo": bo2,
            "gate": np.ascontiguousarray(
                np.tile(np.array([[1.0 - g, float(g)]], np.float32), (P, 1))),
        }
        if mask_any:
            mvec = np.where(mask[g], np.float32(NEG), np.float32(0.0)).astype(
                np.float32)
            m["mbias"] = np.ascontiguousarray(mvec.reshape(NT, P).T)
        maps.append(m)
    return maps


_RUNNER = {}


def _get_runner(mask_any: bool):
    """Cached jax-jitted SPMD executor for the Bass module."""
    if mask_any in _RUNNER:
        return _RUNNER[mask_any]
    import jax
    from jax.sharding import Mesh, PartitionSpec
    from jax.experimental.shard_map import shard_map
    from concourse import bass2jax

    nc = _get_nc(mask_any)
    bass2jax.install_neuronx_cc_hook()

    partition_name = (
        nc.partition_id_tensor.name if nc.partition_id_tensor else None
    )
    in_names, out_names, out_avals = [], [], []
    for alloc in nc.m.functions[0].allocations:
        if not isinstance(alloc, mybir.MemoryLocationSet):
            continue
        name = alloc.memorylocations[0].name
        if alloc.kind == "ExternalInput":
            if name != partition_name:
                in_names.append(name)
        elif alloc.kind == "ExternalOutput":
            shape = tuple(alloc.tensor_shape)
            dtype = mybir.dt.np(alloc.dtype)
            out_names.append(name)
            out_avals.append(jax.core.ShapedArray(shape, dtype))
    n_params = len(in_names)
    n_outs = len(out_avals)
    all_names = list(in_names) + list(out_names)
    if partition_name is not None:
        all_names.append(partition_name)
    donate = tuple(range(n_params, n_params + n_outs))

    def _body(*args):
        operands = list(args)
        if partition_name is not None:
            operands.append(bass2jax.partition_id_tensor())
        outs = bass2jax._bass_exec_p.bind(
            *operands,
            out_avals=tuple(out_avals),
            in_names=tuple(all_names),
            out_names=tuple(out_names),
            lowering_input_output_aliases=(),
            sim_require_finite=True,
            sim_require_nnan=True,
            nc=nc,
        )
        return tuple(outs)

    devices = jax.devices()[:8]
    mesh = Mesh(np.asarray(devices), ("core",))
    in_specs = (PartitionSpec("core"),) * (n_params + n_outs)
    out_specs = (PartitionSpec("core"),) * n_outs
    sharded = jax.jit(
        shard_map(_body, mesh=mesh, in_specs=in_specs, out_specs=out_specs,
                  check_rep=False),
        donate_argnums=donate,
        keep_unused=True,
    )
    zero_shapes = [tuple(a.shape) for a in out_avals]
    zero_dtypes = [a.dtype for a in out_avals]

    def call(maps):
        concat_in = [
            np.concatenate([np.asarray(maps[c][nm]) for c in range(8)], axis=0)
            for nm in in_names
        ]
        concat_zeros = [
            np.zeros((8 * s[0], *s[1:]), d)
            for s, d in zip(zero_shapes, zero_dtypes)
        ]
        out_arrs = sharded(*concat_in, *concat_zeros)
        return [
            {
                nm: np.asarray(out_arrs[i]).reshape(8, *zero_shapes[i])[c]
                for i, nm in enumerate(out_names)
            }
            for c in range(8)
        ]

    _RUNNER[mask_any] = call
    return call


def run(x, mask, Wq, Wkv, Wo, bo, trace=False):
    x = np.asarray(x, np.float32)
    mask = np.asarray(mask, bool)
    Wq = np.asarray(Wq, np.float32)
    Wkv = np.asarray(Wkv, np.float32)
    Wo = np.asarray(Wo, np.float32)
    bo = np.asarray(bo, np.float32)
    mask_any = bool(mask.any())
    maps = _in_maps(x, mask, Wq, Wkv, Wo, bo, mask_any)
    results = _get_runner(mask_any)(maps)
    out = np.empty((B, N, DIM), np.float32)
    for c in range(8):
        g, r = divmod(c, GROUP)
        out[g, r * NQ : (r + 1) * NQ, :] = results[c]["y"]
    return out, results


def kernel(x, mask, Wq, Wkv, Wo, bo):
    out, _ = run(x, mask, Wq, Wkv, Wo, bo)
    return out


# revision 10
# speedup vs baseline: 1.0646x; 1.0002x over previous
"""Distributed multi-head attention kernel for one TRN2 chip (8 NeuronCores).

Problem: y = Attention(x) with b=2, n=2048, dim=1024, heads=16, dim_head=64.

Sharding (data + tensor parallel):
  core c: batch g = c // 4, head-group r = c % 4 (4 heads = 256 inner dims).

Design (v2):
  - Host pre-converts x (transposed to feature-major xT[p, c, t] =
    x[t, 128c+p]) and Wq/Wk/Wv/Wo to bf16, with SCALE folded into Wq, so
    no on-device transposes or conversions are needed.
  - Attention in single-head strips (h, ib): scores psS [128 j, 2*512 i]
    (two j-chunks per PSUM tile pair), exp on ACT -> e bf16 tiles. The
    ACT engine (exp throughput) paces the attention phase; q/k/v
    projections are interleaved into the PE stream one step per exp slot.
  - AV in transposed layout: out [i=128 tok, 65] per i-chunk with a
    ones-column in the v tile giving the softmax denominator in col 64.
    Full 128-partition outputs halve PE cost vs the [65, i] layout, and
    normalization becomes a cheap per-partition DVE scalar op. The four
    i-chunk accumulation groups share one PSUM bank, so the bank is
    zeroed once and all matmuls accumulate with start=False.
  - AV runs in-strip with a 2-pair lag (strips 0-1 lag a full strip while
    the v projections stream in); normalize+transpose-back+staging for
    strip s is deferred into strip s+1 so the PE never stalls on the DVE
    normalization chain at strip boundaries.
  - Transpose-back to [d, tok] via PE with a bf16 identity; two
    gate-scaled DVE copies stage it into the A2A buffer (the wrong-batch
    copy writes zeros).
  - Exchange: four 8-core AllToAll collectives (one per head, fired
    inline as each head's last strip closes) so only the smallest
    possible transfer sits in the tail; receives (plain + SWDGE
    accumulate, summing the two batch halves) are deferred until after
    the last collective so no queue ever blocks a later collective's
    issue.
  - Output projection in two stages: even aT chunks (head pairs 0/1)
    run inside the later collectives' windows; odd chunks in the tail,
    alternating PSUM arenas to avoid write-after-read stalls.
"""

import sys

if "/opt/trn_rl_repo" not in sys.path:
    sys.path.insert(0, "/opt/trn_rl_repo")

from contextlib import ExitStack

import ml_dtypes
import numpy as np

import concourse.bass as bass
from concourse import bacc
import concourse.mybir as mybir
import concourse.tile as tile
from concourse.masks import make_identity

F32 = mybir.dt.float32
F32R = mybir.dt.float32r
BF16 = mybir.dt.bfloat16
EXP = mybir.ActivationFunctionType.Exp

B, N, DIM = 2, 2048, 1024
HEADS, DH = 16, 64
INNER = HEADS * DH            # 1024
SCALE = DH ** -0.5
GROUP = 4                     # tensor-parallel group size (cores per batch)
IC = INNER // GROUP           # 256 inner dims per core (4 heads)
NQ = N // GROUP               # 512 output tokens per core
NEG = -1.0e30

P = 128
TB = 512                      # i-block (moving-dim max)
NT = N // P                   # 16 j-chunks
NJP = NT // 2                 # 8 j-chunk pairs (one psS/exp per pair)
ND = DIM // P                 # 8 contraction chunks
NTB = N // TB                 # 4 token blocks
LAGP = 2                      # AV trails exp by LAGP j-chunk-pairs

_CACHE = {}


def _mm(nc, out, lhsT, rhs, start=True, stop=True, tile_position=None):
    nc.tensor.matmul(
        out, lhsT, rhs, start=start, stop=stop, tile_position=tile_position
    )


def _build(mask_any: bool) -> bass.Bass:
    nc = bacc.Bacc()

    xt_in = nc.declare_dram_parameter("xt_b", [NTB, P, ND, TB], BF16, False)
    wq = nc.declare_dram_parameter("wq_s", [DIM, IC], BF16, False)
    wk = nc.declare_dram_parameter("wk_s", [DIM, IC], BF16, False)
    wv = nc.declare_dram_parameter("wv_s", [DIM, IC], BF16, False)
    wo = nc.declare_dram_parameter("wo", [INNER, DIM], BF16, False)
    bo = nc.declare_dram_parameter("bo", [1, DIM], F32R, False)
    # gate[:, g] = 1.0 iff this core handles batch g (replicated down the
    # partition axis); scales the transpose-back identity so cross-batch
    # A2A chunks carry zeros.
    gate = nc.declare_dram_parameter("gate", [P, 2], F32, False)
    if mask_any:
        mb = nc.declare_dram_parameter("mbias", [P, NT], F32, False)
    y = nc.declare_dram_parameter("y", [NQ, DIM], F32, True)

    with ExitStack() as ctx:
        tc = ctx.enter_context(tile.TileContext(nc))

        const = ctx.enter_context(tc.tile_pool(name="const", bufs=1))
        ident = const.tile([P, P], BF16, tag="ident")
        make_identity(nc, ident[:])
        gate_sb = const.tile([P, 2], F32, tag="gate_sb")
        nc.scalar.dma_start(gate_sb[:], gate.ap())
        ones_f = const.tile([P, P], F32, tag="ones_f")
        nc.vector.memset(ones_f[:], 1.0)
        ones_r = const.tile([P, P], F32R, tag="ones_r")
        nc.vector.tensor_copy(ones_r[:], ones_f[:])
        bo_sb = const.tile([1, DIM], F32R, tag="bo_sb")
        nc.scalar.dma_start(bo_sb[:], bo.ap())
        if mask_any:
            mb_sb = const.tile([P, NT], F32, tag="mb_sb")
            nc.scalar.dma_start(mb_sb[:], mb.ap())

        # ---- persistent SBUF ----
        big = ctx.enter_context(tc.tile_pool(name="big", bufs=1))
        wk_all = big.tile([P, ND, IC], BF16, tag="w", bufs=3, name="wk_all")
        wq_all = big.tile([P, ND, IC], BF16, tag="w", bufs=3, name="wq_all")
        wv_all = big.tile([P, ND, IC], BF16, tag="w", bufs=3, name="wv_all")
        wo_all = big.tile([P, ND, DIM], BF16, tag="woa", name="wo_all")
        wk_sb = [wk_all[:, c, :] for c in range(ND)]
        wq_sb = [wq_all[:, c, :] for c in range(ND)]
        wv_sb = [wv_all[:, c, :] for c in range(ND)]
        wo_sb = [wo_all[:, c, :] for c in range(ND)]
        xTb = [big.tile([P, ND, TB], BF16, tag="xT", bufs=4, name=f"xT{tb}")
               for tb in range(NTB)]

        def xT(c, lo, hi):
            # feature-chunk c, token range [lo, hi) (within one tb block)
            tb, off = divmod(lo, TB)
            return xTb[tb][:, c, off : off + (hi - lo)]
        q2 = [big.tile([P, N], BF16, tag="qk", bufs=4, name=f"q2_{hp}")
              for hp in range(2)]
        k2 = [big.tile([P, N], BF16, tag="qk", bufs=4, name=f"k2_{hp}")
              for hp in range(2)]
        v_all = big.tile([P, 4 * NT * (DH + 1)], BF16, tag="vx",
                         name="v_all")
        va = v_all[:].rearrange("p (h t c) -> p h t c", h=4, t=NT)
        aT = [big.tile([P, NQ], BF16, tag="aT", bufs=8, name=f"aT_{c}")
              for c in range(ND)]
        partial = [big.tile([P, TB], F32, tag="part", bufs=8, name=f"part{i}")
                   for i in range(8)]
        bo_rep = const.tile([P, DIM], F32, tag="bo_rep")

        # DMA issue order: one DMA per weight tensor on the scalar (ACT)
        # HWDGE queue - 4 configs total so neither the ACT sequencer nor the
        # queue's in-flight window ever backs up; x transposes on SP.
        nc.sync.dma_start(xTb[0][:], xt_in.ap()[0])
        nc.scalar.dma_start(wk_all[:], wk.ap().rearrange("(c p) f -> p c f",
                                                         c=ND))
        nc.sync.dma_start(xTb[1][:], xt_in.ap()[1])
        nc.scalar.dma_start(wq_all[:], wq.ap().rearrange("(c p) f -> p c f",
                                                         c=ND))
        nc.sync.dma_start(xTb[2][:], xt_in.ap()[2])
        nc.sync.dma_start(xTb[3][:], xt_in.ap()[3])
        nc.scalar.dma_start(wv_all[:], wv.ap().rearrange("(c p) f -> p c f",
                                                         c=ND))
        nc.scalar.dma_start(wo_all[:], wo.ap().rearrange("(c p) f -> p c f",
                                                         c=ND))

        nc.vector.memset(va[:, :, :, DH], 1.0)

        # A2A buffers (DRAM), one per head: chunk j (64 rows) -> core j;
        # chunk content = this head's [64 d, 512 tok] for quarter j%4,
        # zeros unless j//4 == my batch.
        dram = ctx.enter_context(tc.tile_pool(name="dram", bufs=1, space="DRAM"))
        a2a_in = [dram.tile([8 * DH, NQ], BF16, tag="a2a_in", bufs=4,
                            name=f"a2a_in{h}") for h in range(4)]
        a2a_out = [dram.tile([8 * DH, NQ], BF16, tag="a2a_out", bufs=4,
                             name=f"a2a_out{h}") for h in range(4)]

        ps_s = ctx.enter_context(tc.tile_pool(name="ps_s", bufs=2, space="PSUM"))
        ps_av = ctx.enter_context(tc.tile_pool(name="ps_av", bufs=2, space="PSUM"))
        ps_m = ctx.enter_context(tc.tile_pool(name="ps_m", bufs=2, space="PSUM"))

        sm = ctx.enter_context(tc.tile_pool(name="sm", bufs=1))

        # bias broadcast rows (uses the scores PSUM arena before attention)
        for nb in range(DIM // TB):
            psb = ps_s.tile([P, 2 * TB], F32, tag="s", name="psb")
            _mm(nc, psb[:, 0:TB], ones_r[0:1, :],
                bo_sb[:, nb * TB : (nb + 1) * TB])
            nc.vector.tensor_copy(bo_rep[:, nb * TB : (nb + 1) * TB],
                                  psb[:, 0:TB])

        # ---- projection / wo step closures (popped one per exp slot) ----
        def proj_step(kind, hp, tb):
            def run():
                pj = ps_m.tile([P, TB], F32, tag="m", name="pj")
                wsb = wq_sb if kind == "q" else wk_sb
                dest = q2[hp] if kind == "q" else k2[hp]
                for c in range(ND):
                    _mm(nc, pj[:], wsb[c][:, hp * P : (hp + 1) * P],
                        xT(c, tb * TB, (tb + 1) * TB),
                        start=(c == 0), stop=(c == ND - 1))
                nc.vector.tensor_copy(dest[:, tb * TB : (tb + 1) * TB], pj[:])
            return run

        def proj_half(kind, hp, j):
            # half-range projection: tokens [256j, 256j+256)
            def run():
                pj = ps_m.tile([P, TB], F32, tag="m", name="pjh")
                wsb = wq_sb if kind == "q" else wk_sb
                dest = q2[hp] if kind == "q" else k2[hp]
                lo = j * 256
                for c in range(ND):
                    _mm(nc, pj[:, 0:256], wsb[c][:, hp * P : (hp + 1) * P],
                        xT(c, lo, lo + 256),
                        start=(c == 0), stop=(c == ND - 1))
                nc.vector.tensor_copy(dest[:, lo : lo + 256], pj[:, 0:256])
            return run

        def v_step(t):
            def run():
                pv = ps_m.tile([P, IC], F32, tag="m", name="pv")
                for c in range(ND):
                    _mm(nc, pv[:], xT(c, t * P, (t + 1) * P), wv_sb[c],
                        start=(c == 0), stop=(c == ND - 1))
                nc.vector.tensor_copy(
                    va[:, :, t, 0:DH],
                    pv[:].rearrange("p (h c) -> p h c", h=4))
            return run

        def wo_step(parity, t, nb, alt):
            def run():
                if alt:
                    pw_t = ps_s.tile([P, 2 * TB], F32, tag="s", name="pw")
                    pw = pw_t[:, 0:TB]
                else:
                    pw = ps_m.tile([P, TB], F32, tag="m", name="pw")[:]
                for m in range(4):
                    _mm(nc, pw,
                        aT[2 * m + parity][:, t * P : (t + 1) * P],
                        wo_sb[2 * m + parity][:, nb * TB : (nb + 1) * TB],
                        start=(m == 0), stop=(m == 3))
                if parity == 0:
                    nc.vector.tensor_add(
                        partial[t * 2 + nb][:], pw,
                        bo_rep[:, nb * TB : (nb + 1) * TB])
                else:
                    fo = sm.tile([P, TB], F32, tag="fo", bufs=6, name="fo")
                    nc.vector.tensor_add(fo[:], pw, partial[t * 2 + nb][:])
                    nc.sync.dma_start(
                        y.ap()[t * P : (t + 1) * P, nb * TB : (nb + 1) * TB],
                        fo[:])
            return run

        # stage 1: only the projections that gate the first scores
        proj_step("k", 0, 0)()
        proj_step("k", 0, 1)()
        proj_step("q", 0, 0)()
        proj_step("q", 0, 1)()
        proj_step("k", 0, 2)()
        proj_step("k", 0, 3)()

        # pop queue: exp-slot index -> closure (16 strips x 8 slots = 128).
        # v chunk t is consumed by the (full-strip-lagged) AV at absolute
        # slot 8 + t//2; popping v(t) at slot t-1 stays ahead (same-engine
        # program order guarantees no deadlock even at zero slack).
        popq = {0: [v_step(0)], 1: [v_step(1)]}
        for t in range(2, NT):
            popq.setdefault(t, []).append(v_step(t))
        popq.setdefault(12, []).append(proj_half("q", 0, 4))
        popq.setdefault(14, []).append(proj_half("q", 0, 5))
        popq.setdefault(16, []).append(proj_half("q", 0, 6))
        popq.setdefault(18, []).append(proj_half("q", 0, 7))
        for j in range(8):
            popq.setdefault(32 + 4 * j, []).append(proj_half("k", 1, j))
            popq.setdefault(50 + 4 * j, []).append(proj_half("q", 1, j))

        # ---- attention strips: one head at a time; AV for strip s runs
        # during strip s+1 (one pair per exp slot) so the interleaved
        # v/k/q projection steps never sit behind a waiting AV matmul.
        def do_av(st, jt):
            et = st["es"][jt // 2]
            off = (jt % 2) * TB
            # four accumulation groups share this bank: it was zeroed at
            # strip start and all matmuls accumulate with start=False so
            # their start bits can't clobber each other's partials
            av = st["av"]
            for c in range(4):
                nc.tensor.matmul(
                    av[:, c * (DH + 1) : (c + 1) * (DH + 1)],
                    et[:, off + c * P : off + (c + 1) * P],
                    va[:, st["h"], jt, :],
                    start=False, stop=(jt == NT - 1),
                    skip_group_check=True)

        def norm_dve(st):
            """Normalize (DVE); returns deferred PE-transpose + staging."""
            av, h, ib = st["av"], st["h"], st["ib"]
            rcp = sm.tile([P, 4], F32, tag="rcp", bufs=4, name="rcp")
            with nc.allow_low_precision("softmax denom"):
                nc.vector.reciprocal(rcp[:], av[:, DH :: DH + 1])
            onrm = sm.tile([P, 4 * DH], BF16, tag="onrm", bufs=4, name="onrm")
            for c in range(4):
                nc.vector.tensor_scalar_mul(
                    onrm[:, c * DH : (c + 1) * DH],
                    av[:, c * (DH + 1) : c * (DH + 1) + DH],
                    rcp[:, c : c + 1])

            def part2():
                oT = ps_m.tile([DH, TB], BF16, tag="m", name="oT")
                for c in range(4):
                    nc.tensor.transpose(oT[:, c * P : (c + 1) * P],
                                        onrm[:, c * DH : (c + 1) * DH],
                                        ident[:])
                for g in range(2):
                    stg = sm.tile([DH, TB], BF16, tag="stg", bufs=6,
                                  name="stg")
                    nc.vector.tensor_scalar_mul(stg[:], oT[:],
                                                gate_sb[0:DH, g : g + 1])
                    row = (4 * g + ib) * DH
                    nc.sync.dma_start(a2a_in[h][row : row + DH, :], stg[:])
                if ib == NTB - 1:
                    fire_a2a(h)
            return part2

        def fire_a2a(h):
            nc.gpsimd.collective_compute(
                "AllToAll",
                mybir.AluOpType.bypass,
                replica_groups=[[0, 1, 2, 3, 4, 5, 6, 7]],
                ins=[a2a_in[h].opt()],
                outs=[a2a_out[h].opt()],
            )

        def recv_a2a(h, use_dve=False):
            # receive: aT[2m+hp] rows [e*64, e*64+64) = chunk m + chunk 4+m
            # (exactly one is nonzero). SWDGE accumulate adds the second
            # half; the last head instead loads both halves and adds on DVE
            # (idle in the tail) to skip the serial SWDGE generation chain.
            # Emitted only after the LAST collective so the SP/Pool queues
            # never block a later collective's issue while waiting.
            hp, e = divmod(h, 2)
            for m in range(GROUP):
                dst = aT[2 * m + hp][e * DH : (e + 1) * DH, :]
                nc.sync.dma_start(dst, a2a_out[h][m * DH : (m + 1) * DH, :])
                if use_dve:
                    tmp = sm.tile([DH, NQ], BF16, tag="rtmp", bufs=4,
                                  name="rtmp")
                    nc.sync.dma_start(
                        tmp[:], a2a_out[h][(4 + m) * DH : (5 + m) * DH, :])
                    nc.vector.tensor_add(dst, dst, tmp[:])
                else:
                    nc.gpsimd.dma_start(
                        dst, a2a_out[h][(4 + m) * DH : (5 + m) * DH, :],
                        accum_op=mybir.AluOpType.add)

        # Strips 0-1 lag their AV by a full strip (the v projections are
        # still streaming in); strips 2+ run their own AV in-strip with a
        # 2-pair lag, so each head's A2A fires right at its last strip's
        # end and the four collectives never queue on each other.
        slot = 0
        prev = None
        pend2 = []
        for h in range(4):
            hp, e = divmod(h, 2)
            ksb = k2[hp][e * DH : (e + 1) * DH, :]
            qsb = q2[hp][e * DH : (e + 1) * DH, :]
            tp = (e * DH, 0)
            for ib in range(NTB):
                isl = slice(ib * TB, (ib + 1) * TB)
                s = 4 * h + ib
                in_strip = s >= 2
                cur = {"h": h, "ib": ib, "es": [None] * NJP, "av": None}
                cur["av"] = ps_av.tile([P, 4 * (DH + 1)], F32, tag="av",
                                       name="av")
                nc.vector.memset(cur["av"][:], 0.0)
                for jp in range(NJP):
                    psS = ps_s.tile([P, 2 * TB], F32, tag="s", name="psS")
                    for sub in range(2):
                        jt = 2 * jp + sub
                        jsl = slice(jt * P, (jt + 1) * P)
                        _mm(nc, psS[:, sub * TB : (sub + 1) * TB],
                            ksb[:, jsl], qsb[:, isl], tile_position=tp)
                        if mask_any:
                            nc.vector.tensor_scalar_add(
                                psS[:, sub * TB : (sub + 1) * TB],
                                psS[:, sub * TB : (sub + 1) * TB],
                                mb_sb[:, jt : jt + 1])
                    e_t = sm.tile([P, 2 * TB], BF16, tag="e", bufs=20,
                                  name="e")
                    nc.scalar.activation(e_t[:], psS[:], EXP)
                    cur["es"][jp] = e_t
                    for fn in popq.pop(slot, ()):
                        fn()
                    if jp == 4 and pend2:
                        for fn in pend2:
                            fn()
                        pend2 = []
                    if prev is not None:
                        do_av(prev, 2 * jp)
                        do_av(prev, 2 * jp + 1)
                    if in_strip and jp >= 3:
                        do_av(cur, 2 * (jp - 3))
                        do_av(cur, 2 * (jp - 3) + 1)
                    slot += 1
                if in_strip:
                    if prev is not None:
                        pend2.append(norm_dve(prev))
                        prev = None
                    for jp in range(NJP - 3, NJP):
                        do_av(cur, 2 * jp)
                        do_av(cur, 2 * jp + 1)
                    pc = norm_dve(cur)
                    if ib == NTB - 1:
                        pc()
                    else:
                        pend2.append(pc)
                else:
                    if prev is not None:
                        pend2.append(norm_dve(prev))
                    prev = cur

        for h in range(4):
            recv_a2a(h)

        # wo even stage: runs inside the final collective's window
        wi = 0
        for t in range(NQ // P):
            for nb in range(DIM // TB):
                wo_step(0, t, nb, alt=(wi % 2 == 1))()
                wi += 1
        # drain any unfired pop steps (none expected)
        for i in sorted(popq):
            for fn in popq.pop(i):
                fn()

        # wo odd stage (alternate PSUM arenas to avoid WAR stalls)
        wi = 0
        for t in range(NQ // P):
            for nb in range(DIM // TB):
                wo_step(1, t, nb, alt=(wi % 2 == 1))()
                wi += 1

    nc.compile()
    return nc


def _get_nc(mask_any: bool) -> bass.Bass:
    if mask_any not in _CACHE:
        _CACHE[mask_any] = _build(mask_any)
    return _CACHE[mask_any]


def _in_maps(x, mask, Wq, Wkv, Wo, bo, mask_any):
    bf = ml_dtypes.bfloat16
    bo2 = np.ascontiguousarray(np.asarray(bo, np.float32).reshape(1, DIM))
    wo_bf = np.ascontiguousarray(np.asarray(Wo, np.float32).astype(bf))
    maps = []
    for c in range(8):
        g, r = divmod(c, GROUP)
        csl = slice(r * IC, (r + 1) * IC)
        m = {
            "xt_b": np.ascontiguousarray(
                x[g].reshape(NTB, TB, ND, P).transpose(0, 3, 2, 1).astype(bf)),
            "wq_s": np.ascontiguousarray(
                (Wq[:, csl] * np.float32(SCALE)).astype(bf)),
            "wk_s": np.ascontiguousarray(Wkv[:, csl].astype(bf)),
            "wv_s": np.ascontiguousarray(
                Wkv[:, INNER + r * IC : INNER + (r + 1) * IC].astype(bf)),
            "wo": wo_bf,
            "bo": bo2,
            "gate": np.ascontiguousarray(
                np.tile(np.array([[1.0 - g, float(g)]], np.float32), (P, 1))),
        }
        if mask_any:
            mvec = np.where(mask[g], np.float32(NEG), np.float32(0.0)).astype(
                np.float32)
            m["mbias"] = np.ascontiguousarray(mvec.reshape(NT, P).T)
        maps.append(m)
    return maps


_RUNNER = {}


def _get_runner(mask_any: bool):
    """Cached jax-jitted SPMD executor for the Bass module."""
    if mask_any in _RUNNER:
        return _RUNNER[mask_any]
    import jax
    from jax.sharding import Mesh, PartitionSpec
    from jax.experimental.shard_map import shard_map
    from concourse import bass2jax

    nc = _get_nc(mask_any)
    bass2jax.install_neuronx_cc_hook()

    partition_name = (
        nc.partition_id_tensor.name if nc.partition_id_tensor else None
    )
    in_names, out_names, out_avals = [], [], []
    for alloc in nc.m.functions[0].allocations:
        if not isinstance(alloc, mybir.MemoryLocationSet):
            continue
        name = alloc.memorylocations[0].name
        if alloc.kind == "ExternalInput":
            if name != partition_name:
                in_names.append(name)
        elif alloc.kind == "ExternalOutput":
            shape = tuple(alloc.tensor_shape)
            dtype = mybir.dt.np(alloc.dtype)
            out_names.append(name)
            out_avals.append(jax.core.ShapedArray(shape, dtype))
    n_params = len(in_names)
    n_outs = len(out_avals)
    all_names = list(in_names) + list(out_names)
    if partition_name is not None:
        all_names.append(partition_name)
    donate = tuple(range(n_params, n_params + n_outs))

    def _body(*args):
        operands = list(args)
        if partition_name is not None:
            operands.append(bass2jax.partition_id_tensor())
        outs = bass2jax._bass_exec_p.bind(
            *operands,
            out_avals=tuple(out_avals),
            in_names=tuple(all_names),
            out_names=tuple(out_names),
            lowering_input_output_aliases=(),
            sim_require_finite=True,
            sim_require_nnan=True,
            nc=nc,
        )
        return tuple(outs)

    devices = jax.devices()[:8]
    mesh = Mesh(np.asarray(devices), ("core",))
    in_specs = (PartitionSpec("core"),) * (n_params + n_outs)
    out_specs = (PartitionSpec("core"),) * n_outs
    sharded = jax.jit(
        shard_map(_body, mesh=mesh, in_specs=in_specs, out_specs=out_specs,
                  check_rep=False),
        donate_argnums=donate,
        keep_unused=True,
    )
    zero_shapes = [tuple(a.shape) for a in out_avals]
    zero_dtypes = [a.dtype for a in out_avals]

    def call(maps):
        concat_in = [
            np.concatenate([np.asarray(maps[c][nm]) for c in range(8)], axis=0)
            for nm in in_names
        ]
        concat_zeros = [
            np.zeros((8 * s[0], *s[1:]), d)
            for s, d in zip(zero_shapes, zero_dtypes)
        ]
        out_arrs = sharded(*concat_in, *concat_zeros)
        return [
            {
                nm: np.asarray(out_arrs[i]).reshape(8, *zero_shapes[i])[c]
                for i, nm in enumerate(out_names)
            }
            for c in range(8)
        ]

    _RUNNER[mask_any] = call
    return call


def run(x, mask, Wq, Wkv, Wo, bo, trace=False):
    x = np.asarray(x, np.float32)
    mask = np.asarray(mask, bool)
    Wq = np.asarray(Wq, np.float32)
    Wkv = np.asarray(Wkv, np.float32)
    Wo = np.asarray(Wo, np.float32)
    bo = np.asarray(bo, np.float32)
    mask_any = bool(mask.any())
    maps = _in_maps(x, mask, Wq, Wkv, Wo, bo, mask_any)
    results = _get_runner(mask_any)(maps)
    out = np.empty((B, N, DIM), np.float32)
    for c in range(8):
        g, r = divmod(c, GROUP)
        out[g, r * NQ : (r + 1) * NQ, :] = results[c]["y"]
    return out, results


def kernel(x, mask, Wq, Wkv, Wo, bo):
    out, _ = run(x, mask, Wq, Wkv, Wo, bo)
    return out
